# revision 1
# baseline (speedup 1.0000x reference)
"""TRN2 Bass/Tile kernel: causal self-attention with RoPE.

Sharding across 8 NeuronCores: batch (2) x head-groups (4 groups of 4 heads,
tensor-parallel). Each core computes, for its batch and its 4 heads:
Q/K/V projections (RoPE folded into doubled Q/K weight matmuls), causal
softmax attention in transposed (scores^T) orientation with the softmax
denominator obtained via an extra ones-column in V, and a partial output
projection. The host sums the 4 partial outputs per batch.

All matmuls run in float32r (TF32-like, full-rate for free dim >= 256,
fp32 PSUM accumulation); measured end-to-end rel error ~3e-4.
"""
import numpy as np
import ml_dtypes
import concourse.bass as bass
from concourse import bacc
import concourse.mybir as mybir
import concourse.tile as tile
from concourse.bass_utils import run_bass_kernel_spmd

B, S, D = 2, 2048, 1024
H, DK = 16, 64
THETA = 10000.0
ST = 512              # q-band / projection s-tile width
NSC = S // 128        # 16 s-chunks of 128
f32 = mybir.dt.float32
f32r = mybir.dt.float32r
bf16 = mybir.dt.bfloat16
AF = mybir.ActivationFunctionType
Alu = mybir.AluOpType

# v_aug layout per s-chunk, per head pair: A head [v(64) | one],
# B head [one | zeros(63) | v(64)] (places attn rows at psum partitions 64:128)
VA = 65
VB = 128
VHP = VA + VB        # 193
VSC = 2 * VHP        # 386

_NC = None
_CONSTS = None


def _build():
    import os
    phases = os.environ.get("K_PHASES", "ABC")
    nc = bacc.Bacc()
    xT = nc.dram_tensor("xT", [D, S], f32r, kind="ExternalInput")
    pw = nc.dram_tensor("pw", [D, 1280], f32r, kind="ExternalInput")
    woT = nc.dram_tensor("woT", [256, D], f32r, kind="ExternalInput")
    cossin = nc.dram_tensor("cossin", [128, 2 * S], f32, kind="ExternalInput")
    masks = nc.dram_tensor("masks", [128, 4096], bf16, kind="ExternalInput")
    vpat = nc.dram_tensor("vpat", [128, NSC * VSC], f32r, kind="ExternalInput")
    onesd = nc.dram_tensor("onesd", [128, 128], f32r, kind="ExternalInput")
    out = nc.dram_tensor("out", [S, D], f32, kind="ExternalOutput")

    with tile.TileContext(nc) as tc:
        with tc.tile_pool(name="persist", bufs=1) as pp:
            qT = [pp.tile([128, S], f32r, tag=f"qT{i}", name=f"qT{i}") for i in range(2)]
            kT = [pp.tile([128, S], f32r, tag=f"kT{i}", name=f"kT{i}") for i in range(2)]
            v_aug = pp.tile([128, NSC * VSC], f32r, tag="vaug")
            concatT = [pp.tile([128, S], f32r, tag=f"cT{i}", name=f"cT{i}") for i in range(2)]
            woT_sb = pp.tile([128, 2, D], f32r, tag="woT")
            ones_sb = pp.tile([128, 128], f32r, tag="ones")
            masks_sb = pp.tile([128, 4096], bf16, tag="masks")

            nc.sync.dma_start(masks_sb[:], masks[:])
            nc.sync.dma_start(ones_sb[:], onesd[:])
            nc.sync.dma_start(v_aug[:], vpat[:])
            nc.sync.dma_start(woT_sb[:],
                              woT[:].rearrange("(k p) m -> p k m", p=128))

            va_sc = v_aug[:].rearrange("p (c h r) -> p c h r", c=NSC, r=VHP)

            # ---- Phase A: projections + RoPE + V ----
            with tc.tile_pool(name="pa", bufs=1) as pa, \
                 tc.tile_pool(name="pax", bufs=2) as pax, \
                 tc.tile_pool(name="prope", bufs=4) as prope, \
                 tc.tile_pool(name="psA", bufs=6, space="PSUM") as psA, \
                 tc.tile_pool(name="psV", bufs=2, space="PSUM") as psV:
                pw_sb = pa.tile([128, 8, 1280], f32r, tag="pw")
                cs_sb = pa.tile([128, 2, S], f32, tag="cs")
                nc.sync.dma_start(pw_sb[:],
                                  pw[:].rearrange("(k p) m -> p k m", p=128))
                nc.sync.dma_start(cs_sb[:],
                                  cossin[:].rearrange("p (c s) -> p c s", c=2))

                for st in range(4):
                    xs = pax.tile([128, 8, ST], f32r, tag="xs")
                    nc.sync.dma_start(
                        xs[:],
                        xT[:, st * ST:(st + 1) * ST]
                        .rearrange("(k p) m -> p k m", p=128))
                    sl = slice(st * ST, (st + 1) * ST)
                    for hp in range(2):
                        for aoff, boff, dst in ((0, 256, qT), (512, 768, kT)):
                            pa_ps = psA.tile([128, ST], f32, tag="proj")
                            pb_ps = psA.tile([128, ST], f32, tag="proj")
                            ao = aoff + 128 * hp
                            bo = boff + 128 * hp
                            for kt in range(8):
                                nc.tensor.matmul(pa_ps[:],
                                                 pw_sb[:, kt, ao:ao + 128],
                                                 xs[:, kt, :],
                                                 start=(kt == 0), stop=(kt == 7))
                            for kt in range(8):
                                nc.tensor.matmul(pb_ps[:],
                                                 pw_sb[:, kt, bo:bo + 128],
                                                 xs[:, kt, :],
                                                 start=(kt == 0), stop=(kt == 7))
                            t1 = prope.tile([128, ST], f32r, tag="ropea")
                            t2 = prope.tile([128, ST], f32r, tag="ropeb")
                            nc.vector.tensor_tensor(t1[:], pa_ps[:],
                                                    cs_sb[:, 0, sl], Alu.mult)
                            nc.vector.tensor_tensor(t2[:], pb_ps[:],
                                                    cs_sb[:, 1, sl], Alu.mult)
                            nc.vector.tensor_tensor(dst[hp][:, sl], t1[:],
                                                    t2[:], Alu.add)
                    for scl in range(4):
                        sc = st * 4 + scl
                        vp = psV.tile([128, 256], f32, tag="vproj")
                        for kt in range(8):
                            nc.tensor.matmul(vp[:],
                                             xs[:, kt, scl * 128:(scl + 1) * 128],
                                             pw_sb[:, kt, 1024:1280],
                                             start=(kt == 0), stop=(kt == 7))
                        vp_r = vp[:].rearrange("p (g t e) -> p g t e", g=2, t=2)
                        nc.vector.tensor_copy(va_sc[:, sc, :, 0:64],
                                              vp_r[:, :, 0, :])
                        nc.vector.tensor_copy(va_sc[:, sc, :, VA + 64:VHP],
                                              vp_r[:, :, 1, :])

            # ---- Phase B: attention (scores^T -> exp -> PV -> normalize) ----
            if "B" not in phases:
                return _finish(nc)
            with tc.tile_pool(name="wtp", bufs=5) as wtp, \
                 tc.tile_pool(name="dnp", bufs=2) as dnp, \
                 tc.tile_pool(name="bcp", bufs=2) as bcp, \
                 tc.tile_pool(name="psS", bufs=2, space="PSUM") as psS, \
                 tc.tile_pool(name="psP", bufs=2, space="PSUM") as psP, \
                 tc.tile_pool(name="psB", bufs=1, space="PSUM") as psB:
                for band in range(4):
                    qsl = slice(band * ST, (band + 1) * ST)
                    nkt = 4 * band + 4
                    # diagonal k-tiles first: their mask multiply runs on
                    # gpsimd while PE/ACT stream the full (unmasked) k-tiles
                    kts = list(range(4 * band, nkt)) + list(range(0, 4 * band))
                    for hp in range(2):
                        pvA = psP.tile([65, ST], f32, tag="pv")
                        pvB = psP.tile([128, ST], f32, tag="pv")
                        for i, kt in enumerate(kts):
                            ksl = slice(kt * 128, (kt + 1) * 128)
                            scp = psS.tile([128, 1024], f32, tag="sc")
                            nc.tensor.matmul(scp[:, 0:512],
                                             kT[hp][0:64, ksl],
                                             qT[hp][0:64, qsl],
                                             start=True, stop=True)
                            nc.tensor.matmul(scp[:, 512:1024],
                                             kT[hp][64:128, ksl],
                                             qT[hp][64:128, qsl],
                                             start=True, stop=True)
                            wt = wtp.tile([128, 1024], f32r, tag="wt")
                            nc.scalar.activation(wt[:], scp[:], AF.Exp,
                                                 scale=0.125)
                            j = kt - 4 * band
                            if j >= 0:
                                eng = nc.vector if band == 0 else nc.gpsimd
                                eng.tensor_tensor(
                                    wt[:], wt[:],
                                    masks_sb[:, j * 1024:(j + 1) * 1024],
                                    Alu.mult)
                            nc.tensor.matmul(pvA[:],
                                             va_sc[:, kt, hp, 0:VA],
                                             wt[:, 0:512],
                                             start=(i == 0),
                                             stop=(i == nkt - 1),
                                             skip_group_check=True)
                            nc.tensor.matmul(pvB[:],
                                             va_sc[:, kt, hp, VA:VHP],
                                             wt[:, 512:1024],
                                             start=(i == 0),
                                             stop=(i == nkt - 1),
                                             skip_group_check=True)
                        # normalize head A (denominator at pvA row 64)
                        dnA = dnp.tile([65, ST], f32r, tag="dna")
                        nc.scalar.copy(dnA[64:65, :], pvA[64:65, :])
                        bcA_ps = psB.tile([64, ST], f32, tag="bca")
                        nc.tensor.matmul(bcA_ps[:], ones_sb[64:65, 0:64],
                                         dnA[64:65, :], start=True, stop=True)
                        bcA = bcp.tile([64, ST], f32, tag="bca")
                        nc.vector.reciprocal_approx_fast(bcA[:], bcA_ps[:])
                        nc.vector.tensor_tensor(concatT[hp][0:64, qsl],
                                                pvA[0:64, :], bcA[:], Alu.mult)
                        # normalize head B (denominator at pvB row 0,
                        # attn rows at 64:128)
                        rB = dnp.tile([65, ST], f32, tag="rb")
                        nc.vector.reciprocal_approx_fast(rB[0:1, :],
                                                         pvB[0:1, :])
                        dnB = dnp.tile([65, ST], f32r, tag="dnb")
                        nc.scalar.copy(dnB[0:1, :], rB[0:1, :])
                        bcB_ps = psB.tile([128, ST], f32, tag="bcb")
                        nc.tensor.matmul(bcB_ps[:], ones_sb[0:1, :],
                                         dnB[0:1, :], start=True, stop=True)
                        bcB = bcp.tile([128, ST], f32, tag="bcb")
                        nc.scalar.copy(bcB[64:128, :], bcB_ps[64:128, :])
                        nc.vector.tensor_tensor(concatT[hp][64:128, qsl],
                                                pvB[64:128, :], bcB[64:128, :],
                                                Alu.mult)

            # ---- Phase C: output projection (partial) ----
            if "C" not in phases:
                return _finish(nc)
            with tc.tile_pool(name="outp", bufs=3) as outp, \
                 tc.tile_pool(name="psO", bufs=2, space="PSUM") as psO:
                for sc in range(NSC):
                    ssl = slice(sc * 128, (sc + 1) * 128)
                    op_ps = psO.tile([128, D], f32, tag="op")
                    for ds in range(2):
                        dsl = slice(ds * 512, (ds + 1) * 512)
                        for ot in range(2):
                            nc.tensor.matmul(op_ps[:, dsl],
                                             concatT[ot][:, ssl],
                                             woT_sb[:, ot, dsl],
                                             start=(ot == 0), stop=(ot == 1))
                    ob = outp.tile([128, D], f32, tag="ob")
                    nc.vector.tensor_copy(ob[:], op_ps[:])
                    nc.sync.dma_start(out[ssl, :], ob[:])
    nc.finalize()
    return nc


def _rope_tables():
    inv_freq = 1.0 / (THETA ** (np.arange(0, DK, 2, dtype=np.float64) / DK))
    t = np.arange(S, dtype=np.float64)
    freqs = np.outer(t, inv_freq)
    emb = np.stack((freqs, freqs), axis=-1).reshape(S, DK)
    return np.cos(emb).astype(np.float32), np.sin(emb).astype(np.float32)


def _sgn_shuf(w):
    ws = np.empty_like(w)
    ws[0::2] = -w[1::2]
    ws[1::2] = w[0::2]
    return ws


def _host_consts():
    f_idx = np.arange(512)
    p_idx = np.arange(128)
    mblocks = []
    for j in range(4):
        mj = (f_idx[None, :] >= p_idx[:, None] + 128 * j).astype(np.float32)
        mblocks.append(np.tile(mj, (1, 2)))
    masks_np = np.concatenate(mblocks, axis=1).astype(ml_dtypes.bfloat16)

    vpat_np = np.zeros((128, NSC * VSC), np.float32)
    for sc in range(NSC):
        for r in range(2):
            base = sc * VSC + r * VHP
            vpat_np[:, base + 64] = 1.0   # A ones column
            vpat_np[:, base + VA] = 1.0   # B ones column

    onesd_np = np.zeros((128, 128), np.float32)
    onesd_np[64, 0:64] = 1.0              # lhsT for head-A broadcast
    onesd_np[0, 64:128] = 1.0             # lhsT for head-B broadcast
    return masks_np, vpat_np, onesd_np


def kernel(x, token_positions, W_q, W_k, W_v, W_o):
    global _NC
    if _NC is None:
        _NC = _build()
    x = np.asarray(x, dtype=np.float32)
    token_positions = np.asarray(token_positions)
    W_q = np.asarray(W_q, dtype=np.float32)
    W_k = np.asarray(W_k, dtype=np.float32)
    W_v = np.asarray(W_v, dtype=np.float32)
    W_o = np.asarray(W_o, dtype=np.float32)

    global _CONSTS
    if _CONSTS is None:
        _CONSTS = (*_rope_tables(), *_host_consts())
    cos_t, sin_t, masks_np, vpat_np, onesd_np = _CONSTS

    in_maps = []
    for c in range(8):
        b, g = divmod(c, 4)
        rows = slice(256 * g, 256 * (g + 1))
        wq, wk, wv = W_q[rows], W_k[rows], W_v[rows]
        pw_np = np.ascontiguousarray(np.concatenate(
            [wq.T, _sgn_shuf(wq).T, wk.T, _sgn_shuf(wk).T, wv.T], axis=1))
        woT_np = np.ascontiguousarray(W_o[:, rows].T)
        pos = np.asarray(token_positions[b], dtype=np.int64)
        cosT = np.tile(cos_t[pos].T, (2, 1))
        sinT = np.tile(sin_t[pos].T, (2, 1))
        cossin_np = np.ascontiguousarray(
            np.concatenate([cosT, sinT], axis=1), dtype=np.float32)
        xT_np = np.ascontiguousarray(x[b].T)
        in_maps.append({
            "xT": xT_np, "pw": pw_np, "woT": woT_np, "cossin": cossin_np,
            "masks": masks_np, "vpat": vpat_np, "onesd": onesd_np,
        })

    res = run_bass_kernel_spmd(_NC, in_maps, core_ids=list(range(8)))
    outs = [res.results[c]["out"] for c in range(8)]
    o0 = outs[0] + outs[1] + outs[2] + outs[3]
    o1 = outs[4] + outs[5] + outs[6] + outs[7]
    return np.stack([o0, o1]).astype(np.float32)



# revision 14
# speedup vs baseline: 1.5286x; 1.5286x over previous
"""TRN2 Bass/Tile kernel: causal self-attention with RoPE (bf16 pipeline).

Sharding across 8 NeuronCores: batch (2) x head-groups (4 groups of 4 heads).
Per core, for its batch and 4 heads (2 head-pairs "hp" of 2 heads each):

- Phase A: Q/K/V projections in bf16. RoPE is applied as
  q_rope = q*cos + rot(q*sin) where rot is a fixed pair-swap/sign
  permutation executed as a single [128x128] matmul on the PE (the
  interleaved cos/sin tables are pair-equal so rot commutes with them).
- Phase B: causal attention in scores^T orientation ([keys, q] tiles).
  Diagonal key-tiles are restricted to their live q-range and masked with
  one [128,128] triangle on DVE; fully-masked regions are never computed.
  Softmax denominators come from ones-columns in the augmented V (extra
  PSUM rows are free: matmul cost depends only on the moving free size).
- Phase C: output projection partials, summed on the host.

All matmuls bf16 with fp32 PSUM accumulation.
"""
import numpy as np
import ml_dtypes
import concourse.bass as bass
from concourse import bacc
import concourse.mybir as mybir
import concourse.tile as tile
from concourse.bass_utils import run_bass_kernel_spmd

B, S, D = 2, 2048, 1024
H, DK = 16, 64
THETA = 10000.0
ST = 512              # q-band width
NSC = S // 128        # 16 key chunks of 128
f32 = mybir.dt.float32
bf16 = mybir.dt.bfloat16
AF = mybir.ActivationFunctionType
Alu = mybir.AluOpType

# v_aug layout per key-chunk, per head pair:
# A head [v(64) | one] = 65 cols -> psum rows 0:64 attn, row 64 denom
# B head [one | zeros(63) | v(64)] = 128 cols -> row 0 denom, rows 64:128 attn
VA = 65
VB = 128
VHP = VA + VB         # 193
VSC = 2 * VHP         # 386

# consts layout: [tri(128) | mrotT(128) | onesA(128) | onesB(128)]
C_TRI = 0
C_ROT = 128
C_ONA = 256
C_ONB = 384
C_W = 512

_NC = None
_CONSTS = None
_LAST_RES = None


def _build():
    nc = bacc.Bacc()
    xT = nc.dram_tensor("xT", [D, S], bf16, kind="ExternalInput")
    pw = nc.dram_tensor("pw", [D, 768], bf16, kind="ExternalInput")
    woT = nc.dram_tensor("woT", [256, D], bf16, kind="ExternalInput")
    cossin = nc.dram_tensor("cossin", [128, 2 * S], bf16, kind="ExternalInput")
    consts = nc.dram_tensor("consts", [128, C_W], bf16, kind="ExternalInput")
    vpat = nc.dram_tensor("vpat", [128, NSC * VSC], bf16, kind="ExternalInput")
    onesf = nc.dram_tensor("onesf", [1, 256], mybir.dt.float32r,
                           kind="ExternalInput")
    out = nc.dram_tensor("out", [S, D], bf16, kind="ExternalOutput")
    import os
    dbg = os.environ.get("K_DEBUG", "") == "1"
    if dbg:
        qdbg = nc.dram_tensor("qdbg", [128, 2 * S], bf16, kind="ExternalOutput")
        kdbg = nc.dram_tensor("kdbg", [128, 2 * S], bf16, kind="ExternalOutput")
        vdbg = nc.dram_tensor("vdbg", [128, NSC * VSC], bf16,
                              kind="ExternalOutput")
        cdbg = nc.dram_tensor("cdbg", [128, 2 * S], bf16, kind="ExternalOutput")
        bdbg = nc.dram_tensor("bdbg", [128, 2 * ST], f32, kind="ExternalOutput")
        rdbg = nc.dram_tensor("rdbg", [128, 4 * ST], f32, kind="ExternalOutput")

    with tile.TileContext(nc) as tc:
        with tc.tile_pool(name="persist", bufs=1) as pp, \
             tc.tile_pool(name="rope", bufs=4) as rp, \
             tc.tile_pool(name="wtp", bufs=8) as wtp, \
             tc.tile_pool(name="rcp", bufs=4) as rcp, \
             tc.tile_pool(name="obp", bufs=3) as obp, \
             tc.tile_pool(name="psW", bufs=6, space="PSUM") as psW, \
             tc.tile_pool(name="psA", bufs=1, space="PSUM") as psA, \
             tc.tile_pool(name="psB", bufs=1, space="PSUM") as psB:
            qT = [pp.tile([128, S], bf16, tag=f"qT{i}", name=f"qT{i}") for i in range(2)]
            kT = [pp.tile([128, S], bf16, tag=f"kT{i}", name=f"kT{i}") for i in range(2)]
            v_aug = pp.tile([128, NSC * VSC], bf16, tag="vaug")
            concatT = [pp.tile([128, S], bf16, tag=f"cT{i}", name=f"cT{i}") for i in range(2)]
            woT_sb = pp.tile([128, 2, D], bf16, tag="woT")
            cs_sb = pp.tile([128, 2, S], bf16, tag="cs")
            co_sb = pp.tile([128, C_W], bf16, tag="consts")
            pw_sb = pp.tile([128, 8, 768], bf16, tag="pw")
            onesf_sb = pp.tile([1, 256], mybir.dt.float32r, tag="onesf")
            xs = pp.tile([128, 8, S], bf16, tag="xs")

            # input DMAs, in need-order (SP queue drains FIFO)
            nc.sync.dma_start(co_sb[:], consts[:])
            nc.sync.dma_start(onesf_sb[:], onesf[:])
            pw_r = pw[:].rearrange("(k p) m -> p k m", p=128)
            xs_r = xT[:].rearrange("(k p) m -> p k m", p=128)
            nc.sync.dma_start(pw_sb[:, :, 0:256], pw_r[:, :, 0:256])
            nc.sync.dma_start(xs[:, :, 0:ST], xs_r[:, :, 0:ST])
            nc.sync.dma_start(pw_sb[:, :, 256:512], pw_r[:, :, 256:512])
            nc.sync.dma_start(cs_sb[:],
                              cossin[:].rearrange("p (c s) -> p c s", c=2))
            nc.sync.dma_start(pw_sb[:, :, 512:768], pw_r[:, :, 512:768])
            nc.sync.dma_start(xs[:, :, ST:2 * ST], xs_r[:, :, ST:2 * ST])
            nc.sync.dma_start(v_aug[:], vpat[:])
            nc.sync.dma_start(xs[:, :, 2 * ST:3 * ST], xs_r[:, :, 2 * ST:3 * ST])
            nc.sync.dma_start(xs[:, :, 3 * ST:4 * ST], xs_r[:, :, 3 * ST:4 * ST])
            nc.sync.dma_start(woT_sb[:],
                              woT[:].rearrange("(k p) m -> p k m", p=128))

            if dbg:
                bdbg_sb = pp.tile([128, 2 * ST], f32, tag="bdbg")
                rdbg_sb = pp.tile([128, 4 * ST], f32, tag="rdbg")
                nc.vector.memset(bdbg_sb[:], 0.0)
                nc.vector.memset(rdbg_sb[:], 0.0)
            va_sc = v_aug[:].rearrange("p (c h r) -> p c h r", c=NSC, r=VHP)
            tri = co_sb[:, C_TRI:C_TRI + 128]
            mrotT = co_sb[:, C_ROT:C_ROT + 128]
            onesA = co_sb[0:1, C_ONA:C_ONA + 128]
            onesB = co_sb[0:1, C_ONB:C_ONB + 128]

            def proj(st):
                sl = slice(st * ST, (st + 1) * ST)
                units = []
                # all Q/K contraction matmuls first; cos/sin multiplies on
                # DVE trail each unit; rot matmuls + adds issued after so PE
                # never waits mid-stream on DVE results
                for hp in range(2):
                    for coff, dstT in ((0, qT), (256, kT)):
                        ps = psW.tile([128, ST], f32, tag="w")
                        o = coff + 128 * hp
                        for kt in range(8):
                            nc.tensor.matmul(ps[:], pw_sb[:, kt, o:o + 128],
                                             xs[:, kt, sl],
                                             start=(kt == 0), stop=(kt == 7))
                        t1 = rp.tile([128, ST], bf16, tag="t1")
                        ts = rp.tile([128, ST], bf16, tag="ts")
                        nc.vector.tensor_tensor(t1[:], ps[:], cs_sb[:, 0, sl],
                                                Alu.mult)
                        nc.vector.tensor_tensor(ts[:], ps[:], cs_sb[:, 1, sl],
                                                Alu.mult)
                        if dbg and st == 0 and hp == 0 and coff == 0:
                            nc.vector.tensor_copy(rdbg_sb[:, 0:ST], ps[:])
                            nc.vector.tensor_copy(rdbg_sb[:, ST:2 * ST], t1[:])
                            nc.vector.tensor_copy(rdbg_sb[:, 2 * ST:3 * ST],
                                                  ts[:])
                        units.append((t1, ts, dstT, hp))
                for ui, (t1, ts, dstT, hp) in enumerate(units):
                    rot = psW.tile([128, ST], f32, tag="w")
                    nc.tensor.matmul(rot[:], mrotT, ts[:],
                                     start=True, stop=True)
                    if dbg and st == 0 and ui == 0:
                        nc.vector.tensor_copy(rdbg_sb[:, 3 * ST:4 * ST],
                                              rot[:])
                        nc.sync.dma_start(rdbg[:], rdbg_sb[:])
                    nc.vector.tensor_tensor(dstT[hp][:, sl], t1[:], rot[:],
                                            Alu.add)
                for scl in range(4):
                    sc = st * 4 + scl
                    vp = psW.tile([128, ST], f32, tag="w")
                    lo = st * ST + scl * 128
                    for kt in range(8):
                        nc.tensor.matmul(vp[:, 0:256],
                                         xs[:, kt, lo:lo + 128],
                                         pw_sb[:, kt, 512:768],
                                         start=(kt == 0), stop=(kt == 7))
                    vr = vp[:, 0:256].rearrange("p (g t e) -> p g t e",
                                                g=2, t=2)
                    nc.vector.tensor_copy(va_sc[:, sc, :, 0:64], vr[:, :, 0, :])
                    nc.vector.tensor_copy(va_sc[:, sc, :, 129:193],
                                          vr[:, :, 1, :])

            def band(b, mid=None):
                qb = b * ST
                nkt = 4 * b + 4
                # diagonal tiles first, ascending j: tile j=0 claims the full
                # PSUM bank with a full-width start (the start bit arms zeroing
                # at 2KB-bank granularity, so sub-range starts are illegal);
                # later diagonal tiles accumulate their live sub-range only
                tiles = [4 * b, 4 * b + 1, 4 * b + 2, 4 * b + 3] \
                    + list(range(0, 4 * b))
                for hp in range(2):
                    if hp == 1 and mid is not None:
                        mid()
                    pvA = psA.tile([128, ST], f32, tag="pva")
                    pvB = psB.tile([128, ST], f32, tag="pvb")
                    wts = {}

                    def emit_pv(i):
                        kt = tiles[i]
                        j = kt - 4 * b
                        last = (i == nkt - 1)
                        lo = 128 * j if j > 0 else 0
                        st_ = (i == 0)
                        wA, wB = wts.pop(i)
                        nc.tensor.matmul(pvA[0:VA, lo:512],
                                         va_sc[:, kt, hp, 0:VA],
                                         wA[:, lo:512],
                                         start=st_, stop=last,
                                         skip_group_check=True)
                        nc.tensor.matmul(pvB[:, lo:512],
                                         va_sc[:, kt, hp, VA:VHP],
                                         wB[:, lo:512],
                                         start=st_, stop=last,
                                         skip_group_check=True)

                    for i, kt in enumerate(tiles):
                        j = kt - 4 * b
                        qlo = 128 * j if j >= 0 else 0
                        ksl = slice(kt * 128, (kt + 1) * 128)
                        ws = []
                        for h in range(2):
                            scp = psW.tile([128, ST], f32, tag="w")
                            nc.tensor.matmul(
                                scp[:, qlo:512],
                                kT[hp][64 * h:64 * h + 64, ksl],
                                qT[hp][64 * h:64 * h + 64,
                                       qb + qlo:qb + 512],
                                start=True, stop=True)
                            wt = wtp.tile([128, ST], bf16, tag="wt")
                            nc.scalar.activation(wt[:, qlo:512],
                                                 scp[:, qlo:512],
                                                 AF.Exp, scale=0.125)
                            if j >= 0:
                                nc.vector.tensor_tensor(
                                    wt[:, qlo:qlo + 128],
                                    wt[:, qlo:qlo + 128], tri, Alu.mult)
                            ws.append(wt)
                        wts[i] = ws
                        if i >= 2:
                            emit_pv(i - 2)
                    if nkt >= 2:
                        emit_pv(nkt - 2)
                    emit_pv(nkt - 1)

                    # softmax normalization: denominator rows -> SBUF bf16,
                    # broadcast across partitions via a PE matmul with ones
                    # vectors, one full-tile reciprocal, then one
                    # psum-x-sbuf multiply per head half
                    dnA = rcp.tile([1, ST], bf16, tag="r")
                    dnB = rcp.tile([1, ST], bf16, tag="r")
                    nc.vector.tensor_copy(dnA[:], pvA[64:65, :])
                    nc.vector.tensor_copy(dnB[:], pvB[0:1, :])
                    bcp = psW.tile([128, ST], f32, tag="w")
                    nc.tensor.matmul(bcp[:], onesA, dnA[:],
                                     start=True, stop=False)
                    nc.tensor.matmul(bcp[:], onesB, dnB[:],
                                     start=False, stop=True)
                    bc = rcp.tile([128, ST], f32, tag="bc")
                    nc.vector.reciprocal_approx_fast(bc[:], bcp[:])
                    qsl = slice(qb, qb + ST)
                    nc.vector.tensor_tensor(concatT[hp][0:64, qsl],
                                            pvA[0:64, :], bc[0:64, :],
                                            Alu.mult)
                    nc.vector.tensor_tensor(concatT[hp][64:128, qsl],
                                            pvB[64:128, :], bc[64:128, :],
                                            Alu.mult)
                    if dbg and b == 0 and hp == 0:
                        nc.vector.tensor_copy(bdbg_sb[0:65, 0:ST],
                                              pvA[0:65, :])
                        nc.vector.tensor_copy(bdbg_sb[:, ST:2 * ST], pvB[:])
                        nc.sync.dma_start(bdbg[:], bdbg_sb[:])

            def phase_c(b):
                for sc in range(4 * b, 4 * b + 4):
                    ssl = slice(sc * 128, (sc + 1) * 128)
                    ob = obp.tile([128, D], bf16, tag="ob")
                    opl = psW.tile([128, ST], f32, tag="w")
                    oph = psW.tile([128, ST], f32, tag="w")
                    for dh, op in ((0, opl), (1, oph)):
                        dsl = slice(dh * 512, (dh + 1) * 512)
                        nc.tensor.matmul(op[:], concatT[0][:, ssl],
                                         woT_sb[:, 0, dsl],
                                         start=True, stop=False)
                        nc.tensor.matmul(op[:], concatT[1][:, ssl],
                                         woT_sb[:, 1, dsl],
                                         start=False, stop=True)
                    nc.scalar.copy(ob[:, 0:512], opl[:])
                    nc.vector.tensor_copy(ob[:, 512:1024], oph[:])
                    nc.gpsimd.dma_start(out[ssl, :], ob[:])

            for st in range(4):
                proj(st)
                if st > 0:
                    band(st, mid=lambda s=st: phase_c(s - 1))
                else:
                    band(st)
            phase_c(3)
            if dbg:
                for i in range(2):
                    nc.sync.dma_start(qdbg[:, i * S:(i + 1) * S], qT[i][:])
                    nc.sync.dma_start(kdbg[:, i * S:(i + 1) * S], kT[i][:])
                    nc.sync.dma_start(cdbg[:, i * S:(i + 1) * S], concatT[i][:])
                nc.sync.dma_start(vdbg[:], v_aug[:])
    nc.finalize()
    return nc


def _rope_tables():
    inv_freq = 1.0 / (THETA ** (np.arange(0, DK, 2, dtype=np.float64) / DK))
    t = np.arange(S, dtype=np.float64)
    freqs = np.outer(t, inv_freq)
    emb = np.stack((freqs, freqs), axis=-1).reshape(S, DK)
    return np.cos(emb).astype(np.float32), np.sin(emb).astype(np.float32)


def _host_consts():
    consts = np.zeros((128, C_W), np.float32)
    p = np.arange(128)
    consts[:, C_TRI:C_TRI + 128] = (p[None, :] >= p[:, None])
    # rot lhsT: lhsT[2i+1, 2i] = -1, lhsT[2i, 2i+1] = +1
    m = np.zeros((128, 128), np.float32)
    i2 = np.arange(0, 128, 2)
    m[i2 + 1, i2] = -1.0
    m[i2, i2 + 1] = 1.0
    consts[:, C_ROT:C_ROT + 128] = m
    consts[0, C_ONA:C_ONA + 64] = 1.0
    consts[0, C_ONB + 64:C_ONB + 128] = 1.0

    onesf_np = np.zeros((1, 256), np.float32)
    onesf_np[0, 0:64] = 1.0
    onesf_np[0, 192:256] = 1.0

    vpat_np = np.zeros((128, NSC * VSC), np.float32)
    for sc in range(NSC):
        for r in range(2):
            base = sc * VSC + r * VHP
            vpat_np[:, base + 64] = 1.0   # A ones column
            vpat_np[:, base + VA] = 1.0   # B ones column
    return (consts.astype(ml_dtypes.bfloat16),
            vpat_np.astype(ml_dtypes.bfloat16), onesf_np)


def kernel(x, token_positions, W_q, W_k, W_v, W_o):
    global _NC, _CONSTS
    if _NC is None:
        _NC = _build()
    x = np.asarray(x, dtype=np.float32)
    token_positions = np.asarray(token_positions)
    W_q = np.asarray(W_q, dtype=np.float32)
    W_k = np.asarray(W_k, dtype=np.float32)
    W_v = np.asarray(W_v, dtype=np.float32)
    W_o = np.asarray(W_o, dtype=np.float32)

    if _CONSTS is None:
        _CONSTS = (*_rope_tables(), *_host_consts())
    cos_t, sin_t, consts_np, vpat_np, onesf_np = _CONSTS

    in_maps = []
    for c in range(8):
        b, g = divmod(c, 4)
        rows = slice(256 * g, 256 * (g + 1))
        pw_np = np.concatenate(
            [W_q[rows].T, W_k[rows].T, W_v[rows].T], axis=1
        ).astype(ml_dtypes.bfloat16)
        woT_np = W_o[:, rows].T.astype(ml_dtypes.bfloat16)
        pos = np.asarray(token_positions[b], dtype=np.int64)
        cosT = np.tile(cos_t[pos].T, (2, 1))
        sinT = np.tile(sin_t[pos].T, (2, 1))
        cossin_np = np.concatenate([cosT, sinT], axis=1).astype(
            ml_dtypes.bfloat16)
        xT_np = x[b].T.astype(ml_dtypes.bfloat16)
        in_maps.append({
            "xT": np.ascontiguousarray(xT_np),
            "pw": np.ascontiguousarray(pw_np),
            "woT": np.ascontiguousarray(woT_np),
            "cossin": np.ascontiguousarray(cossin_np),
            "consts": consts_np, "vpat": vpat_np, "onesf": onesf_np,
        })

    global _LAST_RES
    res = run_bass_kernel_spmd(_NC, in_maps, core_ids=list(range(8)))
    _LAST_RES = res
    outs = [res.results[c]["out"] for c in range(8)]
    o0 = outs[0] + outs[1] + outs[2] + outs[3]
    o1 = outs[4] + outs[5] + outs[6] + outs[7]
    return np.stack([o0, o1]).astype(np.float32)


# revision 23
# speedup vs baseline: 1.7230x; 1.1271x over previous
"""TRN2 Bass/Tile kernel: causal self-attention with RoPE (bf16 pipeline).

Sharding across 8 NeuronCores: batch (2) x head-groups (4 groups of 4 heads).
Per core, for its batch and 4 heads (2 head-pairs "hp" of 2 heads each):

- Phase A: Q/K/V projections in bf16. RoPE is applied as
  q_rope = q*cos + rot(q*sin) where rot is a fixed pair-swap/sign
  permutation executed as a single [128x128] matmul on the PE (the
  interleaved cos/sin tables are pair-equal so rot commutes with them).
- Phase B: causal attention in scores^T orientation ([keys, q] tiles).
  Diagonal key-tiles are restricted to their live q-range and masked with
  one [128,128] triangle on DVE; fully-masked regions are never computed.
  Softmax denominators come from ones-columns in the augmented V (extra
  PSUM rows are free: matmul cost depends only on the moving free size).
- Phase C: output projection partials, summed on the host.

All matmuls bf16 with fp32 PSUM accumulation.
"""
import numpy as np
import ml_dtypes
import concourse.bass as bass
from concourse import bacc
import concourse.mybir as mybir
import concourse.tile as tile
from concourse.bass_utils import run_bass_kernel_spmd

B, S, D = 2, 2048, 1024
H, DK = 16, 64
THETA = 10000.0
ST = 512              # q-band width
NSC = S // 128        # 16 key chunks of 128
f32 = mybir.dt.float32
bf16 = mybir.dt.bfloat16
AF = mybir.ActivationFunctionType
Alu = mybir.AluOpType

# v_aug layout per key-chunk, per head pair:
# A head [v(64) | one] = 65 cols -> psum rows 0:64 attn, row 64 denom
# B head [one | zeros(63) | v(64)] = 128 cols -> row 0 denom, rows 64:128 attn
VA = 65
VB = 128
VHP = VA + VB         # 193
VSC = 2 * VHP         # 386

# consts layout: [tri(256 = 2 copies) | mrotT(128) | onesA(128) | onesB(128)]
C_TRI = 0
C_ROT = 256
C_ONA = 384
C_ONB = 512
C_W = 640

_NC = None
_CONSTS = None
_LAST_RES = None


def _build():
    nc = bacc.Bacc()
    xT = nc.dram_tensor("xT", [D, S], bf16, kind="ExternalInput")
    pw = nc.dram_tensor("pw", [D, 768], bf16, kind="ExternalInput")
    woT = nc.dram_tensor("woT", [256, D], bf16, kind="ExternalInput")
    cossin = nc.dram_tensor("cossin", [128, 2 * S], bf16, kind="ExternalInput")
    consts = nc.dram_tensor("consts", [128, C_W], bf16, kind="ExternalInput")
    vpat = nc.dram_tensor("vpat", [128, NSC * VSC], bf16, kind="ExternalInput")
    onesf = nc.dram_tensor("onesf", [1, 256], mybir.dt.float32r,
                           kind="ExternalInput")
    out = nc.dram_tensor("out", [S, D], bf16, kind="ExternalOutput")
    import os
    dbg = os.environ.get("K_DEBUG", "") == "1"
    if dbg:
        qdbg = nc.dram_tensor("qdbg", [128, 2 * S], bf16, kind="ExternalOutput")
        kdbg = nc.dram_tensor("kdbg", [128, 2 * S], bf16, kind="ExternalOutput")
        vdbg = nc.dram_tensor("vdbg", [128, NSC * VSC], bf16,
                              kind="ExternalOutput")
        cdbg = nc.dram_tensor("cdbg", [128, 2 * S], bf16, kind="ExternalOutput")
        bdbg = nc.dram_tensor("bdbg", [128, 2 * ST], f32, kind="ExternalOutput")

    with tile.TileContext(nc) as tc:
        with tc.tile_pool(name="persist", bufs=1) as pp, \
             tc.tile_pool(name="rope", bufs=4) as rp, \
             tc.tile_pool(name="wtp", bufs=6) as wtp, \
             tc.tile_pool(name="rcp", bufs=4) as rcp, \
             tc.tile_pool(name="obp", bufs=3) as obp, \
             tc.tile_pool(name="psS", bufs=2, space="PSUM") as psS, \
             tc.tile_pool(name="psW", bufs=2, space="PSUM") as psW, \
             tc.tile_pool(name="psA", bufs=1, space="PSUM") as psA, \
             tc.tile_pool(name="psB", bufs=1, space="PSUM") as psB:
            qT = [pp.tile([128, S], bf16, tag=f"qT{i}", name=f"qT{i}") for i in range(2)]
            kT = [pp.tile([128, S], bf16, tag=f"kT{i}", name=f"kT{i}") for i in range(2)]
            v_aug = pp.tile([128, NSC * VSC], bf16, tag="vaug")
            concatT = [pp.tile([128, S], bf16, tag=f"cT{i}", name=f"cT{i}") for i in range(2)]
            woT_sb = pp.tile([128, 2, D], bf16, tag="woT")
            cs_sb = pp.tile([128, 2, S], bf16, tag="cs")
            co_sb = pp.tile([128, C_W], bf16, tag="consts")
            pw_sb = pp.tile([128, 8, 768], bf16, tag="pw")
            onesf_sb = pp.tile([1, 256], mybir.dt.float32r, tag="onesf")
            xs = pp.tile([128, 8, S], bf16, tag="xs")

            # input DMAs, in need-order (SP queue drains FIFO)
            nc.sync.dma_start(co_sb[:], consts[:])
            nc.sync.dma_start(onesf_sb[:], onesf[:])
            pw_r = pw[:].rearrange("(k p) m -> p k m", p=128)
            xs_r = xT[:].rearrange("(k p) m -> p k m", p=128)
            nc.sync.dma_start(pw_sb[:, :, 0:128], pw_r[:, :, 0:128])
            nc.sync.dma_start(xs[:, 0:4, 0:ST], xs_r[:, 0:4, 0:ST])
            nc.sync.dma_start(xs[:, 4:8, 0:ST], xs_r[:, 4:8, 0:ST])
            nc.sync.dma_start(pw_sb[:, :, 128:256], pw_r[:, :, 128:256])
            nc.sync.dma_start(pw_sb[:, :, 256:512], pw_r[:, :, 256:512])
            nc.sync.dma_start(cs_sb[:],
                              cossin[:].rearrange("p (c s) -> p c s", c=2))
            nc.sync.dma_start(pw_sb[:, :, 512:768], pw_r[:, :, 512:768])
            nc.sync.dma_start(xs[:, :, ST:2 * ST], xs_r[:, :, ST:2 * ST])
            nc.sync.dma_start(v_aug[:], vpat[:])
            nc.sync.dma_start(xs[:, :, 2 * ST:3 * ST], xs_r[:, :, 2 * ST:3 * ST])
            nc.sync.dma_start(xs[:, :, 3 * ST:4 * ST], xs_r[:, :, 3 * ST:4 * ST])
            nc.sync.dma_start(woT_sb[:],
                              woT[:].rearrange("(k p) m -> p k m", p=128))

            if dbg:
                bdbg_sb = pp.tile([128, 2 * ST], f32, tag="bdbg")
                nc.vector.memset(bdbg_sb[:], 0.0)
            va_sc = v_aug[:].rearrange("p (c h r) -> p c h r", c=NSC, r=VHP)
            tri2 = co_sb[:, C_TRI:C_TRI + 256].rearrange(
                "p (h e) -> p h e", h=2)
            mrotT = co_sb[:, C_ROT:C_ROT + 128]
            onesA = co_sb[0:1, C_ONA:C_ONA + 128]
            onesB = co_sb[0:1, C_ONB:C_ONB + 128]

            def proj_chunks(st):
                """Projection work for band st as a list of issue-chunks so
                band(st-1) can interleave them into its PE stream: 4 Q/K
                units, one rots+adds chunk, 4 V chunks."""
                sl = slice(st * ST, (st + 1) * ST)
                units = []

                def mk_qk(hp, coff):
                    def f():
                        dstT = qT if coff == 0 else kT
                        ps = psW.tile([128, ST], f32, tag="w", name="ps")
                        o = coff + 128 * hp
                        for kt in range(8):
                            nc.tensor.matmul(ps[:], pw_sb[:, kt, o:o + 128],
                                             xs[:, kt, sl],
                                             start=(kt == 0), stop=(kt == 7))
                        t1 = rp.tile([128, ST], bf16, tag="t1", name="t1")
                        ts = rp.tile([128, ST], bf16, tag="ts", name="ts")
                        nc.vector.tensor_tensor(t1[:], ps[:], cs_sb[:, 0, sl],
                                                Alu.mult)
                        nc.vector.tensor_tensor(ts[:], ps[:], cs_sb[:, 1, sl],
                                                Alu.mult)
                        units.append((t1, ts, dstT, hp))
                    return f

                def rots():
                    for t1, ts, dstT, hp in units:
                        rot = psW.tile([128, ST], f32, tag="w", name="rot")
                        nc.tensor.matmul(rot[:], mrotT, ts[:],
                                         start=True, stop=True)
                        nc.vector.tensor_tensor(dstT[hp][:, sl], t1[:],
                                                rot[:], Alu.add)

                def mk_v(scl):
                    def f():
                        sc = st * 4 + scl
                        vp = psW.tile([128, ST], f32, tag="w", name="vp")
                        lo = st * ST + scl * 128
                        for kt in range(8):
                            nc.tensor.matmul(vp[:, 0:256],
                                             xs[:, kt, lo:lo + 128],
                                             pw_sb[:, kt, 512:768],
                                             start=(kt == 0), stop=(kt == 7))
                        vr = vp[:, 0:256].rearrange("p (g t e) -> p g t e",
                                                    g=2, t=2)
                        nc.vector.tensor_copy(va_sc[:, sc, :, 0:64],
                                              vr[:, :, 0, :])
                        nc.vector.tensor_copy(va_sc[:, sc, :, 129:193],
                                              vr[:, :, 1, :])
                    return f

                return [mk_qk(0, 0), mk_qk(0, 256), mk_qk(1, 0),
                        mk_qk(1, 256), rots,
                        mk_v(0), mk_v(1), mk_v(2), mk_v(3)]

            def band(b, mid=None, feed=()):
                feed = list(feed)
                ntile = 2 * (4 * b + 4)
                nfeed = len(feed)
                tctr = 0

                def pace():
                    nonlocal tctr
                    tctr += 1
                    want = tctr * nfeed // ntile
                    while feed and (nfeed - len(feed)) < want:
                        feed.pop(0)()
                qb = b * ST
                nkt = 4 * b + 4
                # diagonal tiles first, ascending j: tile j=0 claims the full
                # PSUM bank with a full-width start (the start bit arms zeroing
                # at 2KB-bank granularity, so sub-range starts are illegal);
                # later diagonal tiles accumulate their live sub-range only
                tiles = [4 * b, 4 * b + 1, 4 * b + 2, 4 * b + 3] \
                    + list(range(0, 4 * b))
                for hp in range(2):
                    if hp == 1 and mid is not None:
                        mid()
                    pvA = psA.tile([128, ST], f32, tag="pva")
                    pvB = psB.tile([128, ST], f32, tag="pvb")
                    wts = {}

                    def emit_pv(i):
                        kt = tiles[i]
                        j = kt - 4 * b
                        last = (i == nkt - 1)
                        lo = 128 * j if j > 0 else 0
                        st_ = (i == 0)
                        wt = wts.pop(i)
                        nc.tensor.matmul(pvA[0:VA, lo:512],
                                         va_sc[:, kt, hp, 0:VA],
                                         wt[:, lo:512],
                                         start=st_, stop=last,
                                         skip_group_check=True)
                        nc.tensor.matmul(pvB[:, lo:512],
                                         va_sc[:, kt, hp, VA:VHP],
                                         wt[:, 512 + lo:1024],
                                         start=st_, stop=last,
                                         skip_group_check=True)

                    for i, kt in enumerate(tiles):
                        j = kt - 4 * b
                        qlo = 128 * j if j >= 0 else 0
                        ksl = slice(kt * 128, (kt + 1) * 128)
                        scp = psS.tile([128, 2 * ST], f32, tag="sc")
                        for h in range(2):
                            nc.tensor.matmul(
                                scp[:, 512 * h + qlo:512 * h + 512],
                                kT[hp][64 * h:64 * h + 64, ksl],
                                qT[hp][64 * h:64 * h + 64,
                                       qb + qlo:qb + 512],
                                start=True, stop=True)
                        wt = wtp.tile([128, 2 * ST], bf16, tag="wt")
                        scv = scp[:].rearrange("p (h q) -> p h q", h=2)
                        wtv = wt[:].rearrange("p (h q) -> p h q", h=2)
                        nc.scalar.activation(wtv[:, :, qlo:512],
                                             scv[:, :, qlo:512],
                                             AF.Exp, scale=0.125)
                        if j >= 0:
                            nc.vector.tensor_tensor(
                                wtv[:, :, qlo:qlo + 128],
                                wtv[:, :, qlo:qlo + 128], tri2, Alu.mult)
                        wts[i] = wt
                        if i >= 2:
                            emit_pv(i - 2)
                        pace()
                    if nkt >= 2:
                        emit_pv(nkt - 2)
                    emit_pv(nkt - 1)

                    # softmax normalization: denominator rows -> SBUF bf16,
                    # broadcast across partitions via a PE matmul with ones
                    # vectors, one full-tile reciprocal, then one
                    # psum-x-sbuf multiply per head half
                    dnA = rcp.tile([1, ST], bf16, tag="r")
                    dnB = rcp.tile([1, ST], bf16, tag="r")
                    nc.scalar.copy(dnA[:], pvA[64:65, :])
                    nc.scalar.copy(dnB[:], pvB[0:1, :])
                    bcp = psW.tile([128, ST], f32, tag="w")
                    nc.tensor.matmul(bcp[:], onesA, dnA[:],
                                     start=True, stop=False)
                    nc.tensor.matmul(bcp[:], onesB, dnB[:],
                                     start=False, stop=True)
                    bc = rcp.tile([128, ST], f32, tag="bc")
                    nc.vector.reciprocal_approx_fast(bc[:], bcp[:])
                    qsl = slice(qb, qb + ST)
                    nc.vector.tensor_tensor(concatT[hp][0:64, qsl],
                                            pvA[0:64, :], bc[0:64, :],
                                            Alu.mult)
                    nc.vector.tensor_tensor(concatT[hp][64:128, qsl],
                                            pvB[64:128, :], bc[64:128, :],
                                            Alu.mult)
                    if hp == 1:
                        for f in feed:
                            f()
                        feed = []
                    if dbg and b == 0 and hp == 0:
                        nc.vector.tensor_copy(bdbg_sb[0:65, 0:ST],
                                              pvA[0:65, :])
                        nc.vector.tensor_copy(bdbg_sb[:, ST:2 * ST], pvB[:])
                        nc.sync.dma_start(bdbg[:], bdbg_sb[:])

            def phase_c(b):
                for sc in range(4 * b, 4 * b + 4):
                    ssl = slice(sc * 128, (sc + 1) * 128)
                    ob = obp.tile([128, D], bf16, tag="ob")
                    op = psS.tile([128, D], f32, tag="sc")
                    for dh in range(2):
                        dsl = slice(dh * 512, (dh + 1) * 512)
                        nc.tensor.matmul(op[:, dsl], concatT[0][:, ssl],
                                         woT_sb[:, 0, dsl],
                                         start=True, stop=False)
                        nc.tensor.matmul(op[:, dsl], concatT[1][:, ssl],
                                         woT_sb[:, 1, dsl],
                                         start=False, stop=True)
                    nc.scalar.copy(ob[:, 0:512], op[:, 0:512])
                    nc.vector.tensor_copy(ob[:, 512:1024], op[:, 512:1024])
                    nc.sync.dma_start(out[ssl, :], ob[:])

            for f in proj_chunks(0):
                f()
            for st in range(4):
                feed = proj_chunks(st + 1) if st < 3 else []
                if st > 0:
                    band(st, mid=lambda s=st: phase_c(s - 1), feed=feed)
                else:
                    band(st, feed=feed)
            phase_c(3)
            if dbg:
                for i in range(2):
                    nc.sync.dma_start(qdbg[:, i * S:(i + 1) * S], qT[i][:])
                    nc.sync.dma_start(kdbg[:, i * S:(i + 1) * S], kT[i][:])
                    nc.sync.dma_start(cdbg[:, i * S:(i + 1) * S], concatT[i][:])
                nc.sync.dma_start(vdbg[:], v_aug[:])
    nc.finalize()
    return nc


def _rope_tables():
    inv_freq = 1.0 / (THETA ** (np.arange(0, DK, 2, dtype=np.float64) / DK))
    t = np.arange(S, dtype=np.float64)
    freqs = np.outer(t, inv_freq)
    emb = np.stack((freqs, freqs), axis=-1).reshape(S, DK)
    return np.cos(emb).astype(np.float32), np.sin(emb).astype(np.float32)


def _host_consts():
    consts = np.zeros((128, C_W), np.float32)
    p = np.arange(128)
    tri = (p[None, :] >= p[:, None])
    consts[:, C_TRI:C_TRI + 128] = tri
    consts[:, C_TRI + 128:C_TRI + 256] = tri
    # rot lhsT: lhsT[2i+1, 2i] = -1, lhsT[2i, 2i+1] = +1
    m = np.zeros((128, 128), np.float32)
    i2 = np.arange(0, 128, 2)
    m[i2 + 1, i2] = -1.0
    m[i2, i2 + 1] = 1.0
    consts[:, C_ROT:C_ROT + 128] = m
    consts[0, C_ONA:C_ONA + 64] = 1.0
    consts[0, C_ONB + 64:C_ONB + 128] = 1.0

    onesf_np = np.zeros((1, 256), np.float32)
    onesf_np[0, 0:64] = 1.0
    onesf_np[0, 192:256] = 1.0

    vpat_np = np.zeros((128, NSC * VSC), np.float32)
    for sc in range(NSC):
        for r in range(2):
            base = sc * VSC + r * VHP
            vpat_np[:, base + 64] = 1.0   # A ones column
            vpat_np[:, base + VA] = 1.0   # B ones column
    return (consts.astype(ml_dtypes.bfloat16),
            vpat_np.astype(ml_dtypes.bfloat16), onesf_np)


def kernel(x, token_positions, W_q, W_k, W_v, W_o):
    global _NC, _CONSTS
    if _NC is None:
        _NC = _build()
    x = np.asarray(x, dtype=np.float32)
    token_positions = np.asarray(token_positions)
    W_q = np.asarray(W_q, dtype=np.float32)
    W_k = np.asarray(W_k, dtype=np.float32)
    W_v = np.asarray(W_v, dtype=np.float32)
    W_o = np.asarray(W_o, dtype=np.float32)

    if _CONSTS is None:
        _CONSTS = (*_rope_tables(), *_host_consts())
    cos_t, sin_t, consts_np, vpat_np, onesf_np = _CONSTS

    in_maps = []
    for c in range(8):
        b, g = divmod(c, 4)
        rows = slice(256 * g, 256 * (g + 1))
        pw_np = np.concatenate(
            [W_q[rows].T, W_k[rows].T, W_v[rows].T], axis=1
        ).astype(ml_dtypes.bfloat16)
        woT_np = W_o[:, rows].T.astype(ml_dtypes.bfloat16)
        pos = np.asarray(token_positions[b], dtype=np.int64)
        cosT = np.tile(cos_t[pos].T, (2, 1))
        sinT = np.tile(sin_t[pos].T, (2, 1))
        cossin_np = np.concatenate([cosT, sinT], axis=1).astype(
            ml_dtypes.bfloat16)
        xT_np = x[b].T.astype(ml_dtypes.bfloat16)
        in_maps.append({
            "xT": np.ascontiguousarray(xT_np),
            "pw": np.ascontiguousarray(pw_np),
            "woT": np.ascontiguousarray(woT_np),
            "cossin": np.ascontiguousarray(cossin_np),
            "consts": consts_np, "vpat": vpat_np, "onesf": onesf_np,
        })

    global _LAST_RES
    res = run_bass_kernel_spmd(_NC, in_maps, core_ids=list(range(8)))
    _LAST_RES = res
    outs = [res.results[c]["out"] for c in range(8)]
    o0 = outs[0] + outs[1] + outs[2] + outs[3]
    o1 = outs[4] + outs[5] + outs[6] + outs[7]
    return np.stack([o0, o1]).astype(np.float32)


# revision 34
# speedup vs baseline: 1.8243x; 1.0588x over previous
"""TRN2 Bass/Tile kernel: causal self-attention with RoPE (bf16 pipeline).

Sharding across 8 NeuronCores: batch (2) x head-groups (4 groups of 4 heads).
Per core, for its batch and 4 heads (2 head-pairs "hp" of 2 heads each):

- Phase A: Q/K/V projections in bf16. RoPE is applied as
  q_rope = q*cos + rot(q*sin) where rot is a fixed pair-swap/sign
  permutation executed as a single [128x128] matmul on the PE (the
  interleaved cos/sin tables are pair-equal so rot commutes with them).
- Phase B: causal attention in scores^T orientation ([keys, q] tiles).
  Diagonal key-tiles are restricted to their live q-range and masked with
  one [128,128] triangle on DVE; fully-masked regions are never computed.
  Softmax denominators come from ones-columns in the augmented V (extra
  PSUM rows are free: matmul cost depends only on the moving free size).
- Phase C: output projection partials, summed on the host.

All matmuls bf16 with fp32 PSUM accumulation.
"""
import numpy as np
import ml_dtypes
import concourse.bass as bass
from concourse import bacc
import concourse.mybir as mybir
import concourse.tile as tile
from concourse.bass_utils import run_bass_kernel_spmd

B, S, D = 2, 2048, 1024
H, DK = 16, 64
THETA = 10000.0
ST = 512              # q-band width
NSC = S // 128        # 16 key chunks of 128
f32 = mybir.dt.float32
bf16 = mybir.dt.bfloat16
AF = mybir.ActivationFunctionType
Alu = mybir.AluOpType

# v_aug layout per key-chunk, per head pair:
# A head [v(64) | one] = 65 cols -> psum rows 0:64 attn, row 64 denom
# B head [one | zeros(63) | v(64)] = 128 cols -> row 0 denom, rows 64:128 attn
VA = 65
VB = 128
VHP = VA + VB         # 193
VSC = 2 * VHP         # 386

# consts layout: [tri(256 = 2 copies) | mrotT(128) | onesA(128) | onesB(128)]
C_TRI = 0
C_ROT = 256
C_ONA = 384
C_ONB = 512
C_W = 640

_NC = None
_CONSTS = None
_LAST_RES = None


def _build():
    nc = bacc.Bacc()
    xT = nc.dram_tensor("xT", [D, S], bf16, kind="ExternalInput")
    pw = nc.dram_tensor("pw", [D, 768], bf16, kind="ExternalInput")
    woT = nc.dram_tensor("woT", [256, D], bf16, kind="ExternalInput")
    cossin = nc.dram_tensor("cossin", [128, 2 * S], bf16, kind="ExternalInput")
    consts = nc.dram_tensor("consts", [128, C_W + VSC], bf16,
                            kind="ExternalInput")
    out = nc.dram_tensor("out", [S, D], bf16, kind="ExternalOutput")
    import os
    dbg = os.environ.get("K_DEBUG", "") == "1"
    LOOK = int(os.environ.get("K_LOOK", "3"))
    HOLD = os.environ.get("K_HOLD", "0") == "1"
    if dbg:
        qdbg = nc.dram_tensor("qdbg", [128, 2 * S], bf16, kind="ExternalOutput")
        kdbg = nc.dram_tensor("kdbg", [128, 2 * S], bf16, kind="ExternalOutput")
        vdbg = nc.dram_tensor("vdbg", [128, NSC * VSC], bf16,
                              kind="ExternalOutput")
        cdbg = nc.dram_tensor("cdbg", [128, 2 * S], bf16, kind="ExternalOutput")
        bdbg = nc.dram_tensor("bdbg", [128, 2 * ST], f32, kind="ExternalOutput")

    with tile.TileContext(nc) as tc:
        with tc.tile_pool(name="persist", bufs=1) as pp, \
             tc.tile_pool(name="rope", bufs=4) as rp, \
             tc.tile_pool(name="wtp", bufs=6) as wtp, \
             tc.tile_pool(name="rcp", bufs=4) as rcp, \
             tc.tile_pool(name="obp", bufs=3) as obp, \
             tc.tile_pool(name="psS", bufs=2, space="PSUM") as psS, \
             tc.tile_pool(name="psW", bufs=2, space="PSUM") as psW, \
             tc.tile_pool(name="psA", bufs=1, space="PSUM") as psA, \
             tc.tile_pool(name="psB", bufs=1, space="PSUM") as psB:
            qT = [pp.tile([128, S], bf16, tag=f"qT{i}", name=f"qT{i}") for i in range(2)]
            kT = [pp.tile([128, S], bf16, tag=f"kT{i}", name=f"kT{i}") for i in range(2)]
            v_aug = pp.tile([128, NSC * VSC], bf16, tag="vaug")
            concatT = [pp.tile([128, S], bf16, tag=f"cT{i}", name=f"cT{i}") for i in range(2)]
            woT_sb = pp.tile([128, 2, D], bf16, tag="woT")
            cs_sb = pp.tile([128, 2, S], bf16, tag="cs")
            co_sb = pp.tile([128, C_W + VSC], bf16, tag="consts")
            pw_sb = pp.tile([128, 8, 768], bf16, tag="pw")
            xs = pp.tile([128, 8, S], bf16, tag="xs")

            # input DMAs, in need-order (SP queue drains FIFO)
            pw_r = pw[:].rearrange("(k p) m -> p k m", p=128)
            xs_r = xT[:].rearrange("(k p) m -> p k m", p=128)
            va_c = v_aug[:].rearrange("p (c r) -> p c r", c=NSC)
            nc.sync.dma_start(pw_sb[:, :, 0:256], pw_r[:, :, 0:256])
            nc.scalar.dma_start(xs[:, :, 0:ST], xs_r[:, :, 0:ST])
            nc.sync.dma_start(co_sb[:], consts[:])
            nc.vector.tensor_copy(va_c[:, 0, :], co_sb[:, C_W:C_W + VSC])
            nc.sync.dma_start(pw_sb[:, :, 256:768], pw_r[:, :, 256:768])
            nc.scalar.dma_start(cs_sb[:],
                                cossin[:].rearrange("p (c s) -> p c s", c=2))
            nc.scalar.dma_start(xs[:, :, ST:2 * ST], xs_r[:, :, ST:2 * ST])
            nc.scalar.dma_start(xs[:, :, 2 * ST:3 * ST],
                                xs_r[:, :, 2 * ST:3 * ST])
            nc.scalar.dma_start(xs[:, :, 3 * ST:4 * ST],
                                xs_r[:, :, 3 * ST:4 * ST])
            nc.sync.dma_start(woT_sb[:],
                              woT[:].rearrange("(k p) m -> p k m", p=128))
            # replicate the ones/zeros pattern to the other 15 key-chunks on
            # the otherwise-idle Pool engine (SBUF->SBUF)
            for c in range(1, NSC):
                nc.gpsimd.tensor_copy(va_c[:, c, :], va_c[:, 0, :])

            if dbg:
                bdbg_sb = pp.tile([128, 2 * ST], f32, tag="bdbg")
                nc.vector.memset(bdbg_sb[:], 0.0)
            va_sc = v_aug[:].rearrange("p (c h r) -> p c h r", c=NSC, r=VHP)
            tri2 = co_sb[:, C_TRI:C_TRI + 256].rearrange(
                "p (h e) -> p h e", h=2)
            mrotT = co_sb[:, C_ROT:C_ROT + 128]
            onesA = co_sb[0:1, C_ONA:C_ONA + 128]
            onesB = co_sb[0:1, C_ONB:C_ONB + 128]

            def proj_chunks(st):
                """Projection work for band st as a list of issue-chunks so
                band(st-1) can interleave them into its PE stream: 4 Q/K
                units, one rots+adds chunk, 4 V chunks."""
                sl = slice(st * ST, (st + 1) * ST)
                units = []

                def mk_qk(hp, coff):
                    def f():
                        dstT = qT if coff == 0 else kT
                        ps = psW.tile([128, ST], f32, tag="w", name="ps")
                        o = coff + 128 * hp
                        for kt in range(8):
                            nc.tensor.matmul(ps[:], pw_sb[:, kt, o:o + 128],
                                             xs[:, kt, sl],
                                             start=(kt == 0), stop=(kt == 7))
                        t1 = rp.tile([128, ST], bf16, tag="t1", name="t1")
                        ts = rp.tile([128, ST], bf16, tag="ts", name="ts")
                        nc.vector.tensor_tensor(t1[:], ps[:], cs_sb[:, 0, sl],
                                                Alu.mult)
                        nc.vector.tensor_tensor(ts[:], ps[:], cs_sb[:, 1, sl],
                                                Alu.mult)
                        units.append((t1, ts, dstT, hp))
                    return f

                def rots():
                    for t1, ts, dstT, hp in units:
                        rot = psW.tile([128, ST], f32, tag="w", name="rot")
                        nc.tensor.matmul(rot[:], mrotT, ts[:],
                                         start=True, stop=True)
                        nc.vector.tensor_tensor(dstT[hp][:, sl], t1[:],
                                                rot[:], Alu.add)

                def mk_v(scl):
                    def f():
                        sc = st * 4 + scl
                        vp = psW.tile([128, ST], f32, tag="w", name="vp")
                        lo = st * ST + scl * 128
                        for kt in range(8):
                            nc.tensor.matmul(vp[:, 0:256],
                                             xs[:, kt, lo:lo + 128],
                                             pw_sb[:, kt, 512:768],
                                             start=(kt == 0), stop=(kt == 7))
                        vr = vp[:, 0:256].rearrange("p (g t e) -> p g t e",
                                                    g=2, t=2)
                        nc.vector.tensor_copy(va_sc[:, sc, :, 0:64],
                                              vr[:, :, 0, :])
                        nc.vector.tensor_copy(va_sc[:, sc, :, 129:193],
                                              vr[:, :, 1, :])
                    return f

                return [mk_qk(0, 0), mk_qk(1, 0), mk_qk(0, 256),
                        mk_qk(1, 256), rots,
                        mk_v(0), mk_v(1), mk_v(2), mk_v(3)]

            def band(b, feed=(), tail_chunks=()):
                feed = list(feed)
                ntile = 2 * (4 * b + 4)
                nfeed = len(feed)
                tctr = 0

                def pace(last=False):
                    nonlocal tctr
                    tctr += 1
                    if last:
                        return
                    want = tctr * nfeed // ntile
                    while feed and (nfeed - len(feed)) < want:
                        feed.pop(0)()
                qb = b * ST
                nkt = 4 * b + 4
                # diagonal tiles first, ascending j: tile j=0 claims the full
                # PSUM bank with a full-width start (the start bit arms zeroing
                # at 2KB-bank granularity, so sub-range starts are illegal);
                # later diagonal tiles accumulate their live sub-range only
                tiles = [4 * b, 4 * b + 1, 4 * b + 2, 4 * b + 3] \
                    + list(range(0, 4 * b))
                for hp in range(2):
                    pvA = psA.tile([128, ST], f32, tag="pva")
                    pvB = psB.tile([128, ST], f32, tag="pvb")
                    wts = {}

                    def emit_pv(i):
                        kt = tiles[i]
                        j = kt - 4 * b
                        last = (i == nkt - 1)
                        lo = 128 * j if j > 0 else 0
                        st_ = (i == 0)
                        wt = wts.pop(i)
                        nc.tensor.matmul(pvA[0:VA, lo:512],
                                         va_sc[:, kt, hp, 0:VA],
                                         wt[:, lo:512],
                                         start=st_, stop=last,
                                         skip_group_check=True)
                        nc.tensor.matmul(pvB[:, lo:512],
                                         va_sc[:, kt, hp, VA:VHP],
                                         wt[:, 512 + lo:1024],
                                         start=st_, stop=last,
                                         skip_group_check=True)

                    for i, kt in enumerate(tiles):
                        j = kt - 4 * b
                        qlo = 128 * j if j >= 0 else 0
                        ksl = slice(kt * 128, (kt + 1) * 128)
                        scp = psS.tile([128, 2 * ST], f32, tag="sc")
                        for h in range(2):
                            nc.tensor.matmul(
                                scp[:, 512 * h + qlo:512 * h + 512],
                                kT[hp][64 * h:64 * h + 64, ksl],
                                qT[hp][64 * h:64 * h + 64,
                                       qb + qlo:qb + 512],
                                start=True, stop=True)
                        wt = wtp.tile([128, 2 * ST], bf16, tag="wt")
                        scv = scp[:].rearrange("p (h q) -> p h q", h=2)
                        wtv = wt[:].rearrange("p (h q) -> p h q", h=2)
                        nc.scalar.activation(wtv[:, :, qlo:512],
                                             scv[:, :, qlo:512],
                                             AF.Exp, scale=0.125)
                        if j >= 0:
                            nc.vector.tensor_tensor(
                                wtv[:, :, qlo:qlo + 128],
                                wtv[:, :, qlo:qlo + 128], tri2, Alu.mult)
                        wts[i] = wt
                        if i >= LOOK:
                            emit_pv(i - LOOK)
                        pace(last=HOLD and (i >= nkt - 2))
                    for i in range(max(0, nkt - LOOK), nkt):
                        emit_pv(i)

                    # softmax normalization: denominator rows -> SBUF bf16,
                    # broadcast across partitions via a PE matmul with ones
                    # vectors, one full-tile reciprocal, then one
                    # psum-x-sbuf multiply per head half
                    dnA = rcp.tile([1, ST], bf16, tag="r")
                    dnB = rcp.tile([1, ST], bf16, tag="r")
                    nc.scalar.copy(dnA[:], pvA[64:65, :])
                    nc.vector.tensor_copy(dnB[:], pvB[0:1, :])
                    bcp = psW.tile([128, ST], f32, tag="w")
                    nc.tensor.matmul(bcp[:], onesA, dnA[:],
                                     start=True, stop=False)
                    nc.tensor.matmul(bcp[:], onesB, dnB[:],
                                     start=False, stop=True)
                    bc = rcp.tile([128, ST], f32, tag="bc")
                    nc.vector.reciprocal_approx_fast(bc[:], bcp[:])
                    tail_c = tail_chunks if hp == 1 else []
                    for ci in range(2):
                        csl = slice(ci * 256, (ci + 1) * 256)
                        qsl = slice(qb + ci * 256, qb + (ci + 1) * 256)
                        nc.vector.tensor_tensor(concatT[hp][0:64, qsl],
                                                pvA[0:64, csl], bc[0:64, csl],
                                                Alu.mult)
                        nc.vector.tensor_tensor(concatT[hp][64:128, qsl],
                                                pvB[64:128, csl],
                                                bc[64:128, csl],
                                                Alu.mult)
                        for f in tail_c[2 * ci:2 * ci + 2]:
                            f()
                    if hp == 1:
                        for f in feed:
                            f()
                        feed = []
                    if dbg and b == 0 and hp == 0:
                        nc.vector.tensor_copy(bdbg_sb[0:65, 0:ST],
                                              pvA[0:65, :])
                        nc.vector.tensor_copy(bdbg_sb[:, ST:2 * ST], pvB[:])
                        nc.sync.dma_start(bdbg[:], bdbg_sb[:])

            def phase_c_chunks(b):
                def mk(sc):
                    def f():
                        ssl = slice(sc * 128, (sc + 1) * 128)
                        ob = obp.tile([128, D], bf16, tag="ob", name="ob")
                        op = psS.tile([128, D], f32, tag="sc", name="op")
                        for dh in range(2):
                            dsl = slice(dh * 512, (dh + 1) * 512)
                            nc.tensor.matmul(op[:, dsl], concatT[0][:, ssl],
                                             woT_sb[:, 0, dsl],
                                             start=True, stop=False)
                            nc.tensor.matmul(op[:, dsl], concatT[1][:, ssl],
                                             woT_sb[:, 1, dsl],
                                             start=False, stop=True)
                        nc.scalar.copy(ob[:, 0:512], op[:, 0:512])
                        nc.sync.dma_start(out[ssl, 0:512], ob[:, 0:512])
                        nc.vector.tensor_copy(ob[:, 512:1024], op[:, 512:1024])
                        nc.sync.dma_start(out[ssl, 512:1024], ob[:, 512:1024])
                    return f
                return [mk(sc) for sc in range(4 * b, 4 * b + 4)]

            for f in proj_chunks(0):
                f()
            for st in range(4):
                pj = proj_chunks(st + 1) if st < 3 else []
                cc = phase_c_chunks(st - 1) if st > 0 else []
                # interleave: C chunks between early QK chunks
                feed = []
                while pj or cc:
                    if pj:
                        feed.append(pj.pop(0))
                    if cc:
                        feed.append(cc.pop(0))
                band(st, feed=feed,
                     tail_chunks=phase_c_chunks(3) if st == 3 else ())
            if dbg:
                for i in range(2):
                    nc.sync.dma_start(qdbg[:, i * S:(i + 1) * S], qT[i][:])
                    nc.sync.dma_start(kdbg[:, i * S:(i + 1) * S], kT[i][:])
                    nc.sync.dma_start(cdbg[:, i * S:(i + 1) * S], concatT[i][:])
                nc.sync.dma_start(vdbg[:], v_aug[:])
    nc.finalize()
    return nc


def _rope_tables():
    inv_freq = 1.0 / (THETA ** (np.arange(0, DK, 2, dtype=np.float64) / DK))
    t = np.arange(S, dtype=np.float64)
    freqs = np.outer(t, inv_freq)
    emb = np.stack((freqs, freqs), axis=-1).reshape(S, DK)
    return np.cos(emb).astype(np.float32), np.sin(emb).astype(np.float32)


def _host_consts():
    consts = np.zeros((128, C_W), np.float32)  # vpat appended below
    p = np.arange(128)
    tri = (p[None, :] >= p[:, None])
    consts[:, C_TRI:C_TRI + 128] = tri
    consts[:, C_TRI + 128:C_TRI + 256] = tri
    # rot lhsT: lhsT[2i+1, 2i] = -1, lhsT[2i, 2i+1] = +1
    m = np.zeros((128, 128), np.float32)
    i2 = np.arange(0, 128, 2)
    m[i2 + 1, i2] = -1.0
    m[i2, i2 + 1] = 1.0
    consts[:, C_ROT:C_ROT + 128] = m
    consts[0, C_ONA:C_ONA + 64] = 1.0
    consts[0, C_ONB + 64:C_ONB + 128] = 1.0

    vpat_np = np.zeros((128, VSC), np.float32)
    for r in range(2):
        base = r * VHP
        vpat_np[:, base + 64] = 1.0   # A ones column
        vpat_np[:, base + VA] = 1.0   # B ones column
    return (np.concatenate([consts, vpat_np], axis=1)
            .astype(ml_dtypes.bfloat16),)


def kernel(x, token_positions, W_q, W_k, W_v, W_o):
    global _NC, _CONSTS
    if _NC is None:
        _NC = _build()
    x = np.asarray(x, dtype=np.float32)
    token_positions = np.asarray(token_positions)
    W_q = np.asarray(W_q, dtype=np.float32)
    W_k = np.asarray(W_k, dtype=np.float32)
    W_v = np.asarray(W_v, dtype=np.float32)
    W_o = np.asarray(W_o, dtype=np.float32)

    if _CONSTS is None:
        _CONSTS = (*_rope_tables(), *_host_consts())
    cos_t, sin_t, consts_np = _CONSTS

    in_maps = []
    for c in range(8):
        b, g = divmod(c, 4)
        rows = slice(256 * g, 256 * (g + 1))
        pw_np = np.concatenate(
            [W_q[rows].T, W_k[rows].T, W_v[rows].T], axis=1
        ).astype(ml_dtypes.bfloat16)
        woT_np = W_o[:, rows].T.astype(ml_dtypes.bfloat16)
        pos = np.asarray(token_positions[b], dtype=np.int64)
        cosT = np.tile(cos_t[pos].T, (2, 1))
        sinT = np.tile(sin_t[pos].T, (2, 1))
        cossin_np = np.concatenate([cosT, sinT], axis=1).astype(
            ml_dtypes.bfloat16)
        xT_np = x[b].T.astype(ml_dtypes.bfloat16)
        in_maps.append({
            "xT": np.ascontiguousarray(xT_np),
            "pw": np.ascontiguousarray(pw_np),
            "woT": np.ascontiguousarray(woT_np),
            "cossin": np.ascontiguousarray(cossin_np),
            "consts": consts_np,
        })

    global _LAST_RES
    res = run_bass_kernel_spmd(_NC, in_maps, core_ids=list(range(8)))
    _LAST_RES = res
    outs = [res.results[c]["out"] for c in range(8)]
    o0 = outs[0] + outs[1] + outs[2] + outs[3]
    o1 = outs[4] + outs[5] + outs[6] + outs[7]
    return np.stack([o0, o1]).astype(np.float32)


# revision 51
# speedup vs baseline: 1.8569x; 1.0179x over previous
"""TRN2 Bass/Tile kernel: causal self-attention with RoPE (bf16 pipeline).

Sharding across 8 NeuronCores: batch (2) x head-groups (4 groups of 4 heads).
Per core, for its batch and 4 heads (2 head-pairs "hp" of 2 heads each):

- Phase A: Q/K/V projections in bf16. RoPE is applied as
  q_rope = q*cos + rot(q*sin) where rot is a fixed pair-swap/sign
  permutation executed as a single [128x128] matmul on the PE (the
  interleaved cos/sin tables are pair-equal so rot commutes with them).
- Phase B: causal attention in scores^T orientation ([keys, q] tiles).
  Diagonal key-tiles are restricted to their live q-range and masked with
  one [128,128] triangle on DVE; fully-masked regions are never computed.
  Softmax denominators come from ones-columns in the augmented V (extra
  PSUM rows are free: matmul cost depends only on the moving free size).
- Phase C: output projection partials, summed on the host.

All matmuls bf16 with fp32 PSUM accumulation.
"""
import numpy as np
import ml_dtypes
import concourse.bass as bass
from concourse import bacc
import concourse.mybir as mybir
import concourse.tile as tile
from concourse.bass_utils import run_bass_kernel_spmd

B, S, D = 2, 2048, 1024
H, DK = 16, 64
THETA = 10000.0
ST = 512              # q-band width
NSC = S // 128        # 16 key chunks of 128
f32 = mybir.dt.float32
bf16 = mybir.dt.bfloat16
AF = mybir.ActivationFunctionType
Alu = mybir.AluOpType

# v_aug layout per key-chunk, per head pair:
# A head [v(64) | one] = 65 cols -> psum rows 0:64 attn, row 64 denom
# B head [one | zeros(63) | v(64)] = 128 cols -> row 0 denom, rows 64:128 attn
VA = 65
VB = 128
VHP = VA + VB         # 193
VSC = 2 * VHP         # 386

# consts layout: [tri(256 = 2 copies) | mrotT(128) | onesA(128) | onesB(128)]
C_TRI = 0
C_ROT = 256
C_ONA = 384
C_ONB = 512
C_W = 640

_NC = None
_CONSTS = None
_LAST_RES = None


def _build():
    nc = bacc.Bacc()
    xT = nc.dram_tensor("xT", [D, S], bf16, kind="ExternalInput")
    pw = nc.dram_tensor("pw", [D, 768], bf16, kind="ExternalInput")
    woT = nc.dram_tensor("woT", [256, D], bf16, kind="ExternalInput")
    cossin = nc.dram_tensor("cossin", [128, 2 * S], bf16, kind="ExternalInput")
    consts = nc.dram_tensor("consts", [128, C_W + VSC], bf16,
                            kind="ExternalInput")
    out = nc.dram_tensor("out", [S, D], bf16, kind="ExternalOutput")
    import os
    dbg = os.environ.get("K_DEBUG", "") == "1"
    LOOK = int(os.environ.get("K_LOOK", "3"))
    HOLD = os.environ.get("K_HOLD", "0") == "1"
    if dbg:
        qdbg = nc.dram_tensor("qdbg", [128, 2 * S], bf16, kind="ExternalOutput")
        kdbg = nc.dram_tensor("kdbg", [128, 2 * S], bf16, kind="ExternalOutput")
        vdbg = nc.dram_tensor("vdbg", [128, NSC * VSC], bf16,
                              kind="ExternalOutput")
        cdbg = nc.dram_tensor("cdbg", [128, 2 * S], bf16, kind="ExternalOutput")
        bdbg = nc.dram_tensor("bdbg", [128, 2 * ST], f32, kind="ExternalOutput")

    with tile.TileContext(nc) as tc:
        with tc.tile_pool(name="persist", bufs=1) as pp, \
             tc.tile_pool(name="rope", bufs=4) as rp, \
             tc.tile_pool(name="wtp", bufs=6) as wtp, \
             tc.tile_pool(name="rcp", bufs=4) as rcp, \
             tc.tile_pool(name="obp", bufs=3) as obp, \
             tc.tile_pool(name="psS", bufs=2, space="PSUM") as psS, \
             tc.tile_pool(name="psW", bufs=2, space="PSUM") as psW, \
             tc.tile_pool(name="psA", bufs=1, space="PSUM") as psA, \
             tc.tile_pool(name="psB", bufs=1, space="PSUM") as psB:
            qT = [pp.tile([128, S], bf16, tag=f"qT{i}", name=f"qT{i}") for i in range(2)]
            kT = [pp.tile([128, S], bf16, tag=f"kT{i}", name=f"kT{i}") for i in range(2)]
            v_aug = pp.tile([128, NSC * VSC], bf16, tag="vaug")
            concatT = [pp.tile([128, S], bf16, tag=f"cT{i}", name=f"cT{i}") for i in range(2)]
            woT_sb = pp.tile([128, 2, D], bf16, tag="woT")
            cs_sb = pp.tile([128, 2, S], bf16, tag="cs")
            co_sb = pp.tile([128, C_W + VSC], bf16, tag="consts")
            pw_sb = pp.tile([128, 8, 768], bf16, tag="pw")
            xs = pp.tile([128, 8, S], bf16, tag="xs")

            # input DMAs, in need-order (SP queue drains FIFO)
            pw_r = pw[:].rearrange("(k p) m -> p k m", p=128)
            xs_r = xT[:].rearrange("(k p) m -> p k m", p=128)
            va_c = v_aug[:].rearrange("p (c r) -> p c r", c=NSC)
            nc.sync.dma_start(pw_sb[:, :, 0:256], pw_r[:, :, 0:256])
            nc.scalar.dma_start(xs[:, :, 0:ST], xs_r[:, :, 0:ST])
            nc.sync.dma_start(co_sb[:], consts[:])
            nc.vector.tensor_copy(va_c[:, 0, :], co_sb[:, C_W:C_W + VSC])
            nc.sync.dma_start(pw_sb[:, :, 256:768], pw_r[:, :, 256:768])
            nc.scalar.dma_start(cs_sb[:],
                                cossin[:].rearrange("p (c s) -> p c s", c=2))
            nc.scalar.dma_start(xs[:, :, ST:2 * ST], xs_r[:, :, ST:2 * ST])
            nc.scalar.dma_start(xs[:, :, 2 * ST:3 * ST],
                                xs_r[:, :, 2 * ST:3 * ST])
            nc.scalar.dma_start(xs[:, :, 3 * ST:4 * ST],
                                xs_r[:, :, 3 * ST:4 * ST])
            nc.sync.dma_start(woT_sb[:],
                              woT[:].rearrange("(k p) m -> p k m", p=128))
            # replicate the ones/zeros pattern to the other 15 key-chunks on
            # the otherwise-idle Pool engine (SBUF->SBUF)
            for c in range(1, NSC):
                nc.gpsimd.tensor_copy(va_c[:, c, :], va_c[:, 0, :])

            if dbg:
                bdbg_sb = pp.tile([128, 2 * ST], f32, tag="bdbg")
                nc.vector.memset(bdbg_sb[:], 0.0)
            va_sc = v_aug[:].rearrange("p (c h r) -> p c h r", c=NSC, r=VHP)
            tri2 = co_sb[:, C_TRI:C_TRI + 256].rearrange(
                "p (h e) -> p h e", h=2)
            mrotT = co_sb[:, C_ROT:C_ROT + 128]
            onesA = co_sb[0:1, C_ONA:C_ONA + 128]
            onesB = co_sb[0:1, C_ONB:C_ONB + 128]

            def proj_chunks(st):
                """Projection work for band st as a list of issue-chunks so
                band(st-1) can interleave them into its PE stream: 4 Q/K
                units, one rots+adds chunk, 4 V chunks."""
                sl = slice(st * ST, (st + 1) * ST)
                units = []

                def mk_qk(hp, coff):
                    def f():
                        dstT = qT if coff == 0 else kT
                        ps = psW.tile([128, ST], f32, tag="w", name="ps")
                        o = coff + 128 * hp
                        for kt in range(8):
                            nc.tensor.matmul(ps[:], pw_sb[:, kt, o:o + 128],
                                             xs[:, kt, sl],
                                             start=(kt == 0), stop=(kt == 7))
                        t1 = rp.tile([128, ST], bf16, tag="t1", name="t1")
                        ts = rp.tile([128, ST], bf16, tag="ts", name="ts")
                        nc.vector.tensor_tensor(t1[:], ps[:], cs_sb[:, 0, sl],
                                                Alu.mult)
                        nc.vector.tensor_tensor(ts[:], ps[:], cs_sb[:, 1, sl],
                                                Alu.mult)
                        units.append((t1, ts, dstT, hp))
                    return f

                def rots():
                    for t1, ts, dstT, hp in units:
                        rot = psW.tile([128, ST], f32, tag="w", name="rot")
                        nc.tensor.matmul(rot[:], mrotT, ts[:],
                                         start=True, stop=True)
                        nc.vector.tensor_tensor(dstT[hp][:, sl], t1[:],
                                                rot[:], Alu.add)

                def mk_v(scl):
                    def f():
                        sc = st * 4 + scl
                        vp = psW.tile([128, ST], f32, tag="w", name="vp")
                        lo = st * ST + scl * 128
                        for kt in range(8):
                            nc.tensor.matmul(vp[:, 0:256],
                                             xs[:, kt, lo:lo + 128],
                                             pw_sb[:, kt, 512:768],
                                             start=(kt == 0), stop=(kt == 7))
                        vr = vp[:, 0:256].rearrange("p (g t e) -> p g t e",
                                                    g=2, t=2)
                        nc.vector.tensor_copy(va_sc[:, sc, :, 0:64],
                                              vr[:, :, 0, :])
                        nc.vector.tensor_copy(va_sc[:, sc, :, 129:193],
                                              vr[:, :, 1, :])
                    return f

                return [mk_qk(0, 0), mk_qk(1, 0), mk_qk(0, 256),
                        mk_qk(1, 256), rots,
                        mk_v(0), mk_v(1), mk_v(2), mk_v(3)]

            def band(b, feed=(), tail_chunks=()):
                feed = list(feed)
                ntile = 2 * (4 * b + 4)
                nfeed = len(feed)
                tctr = 0

                def pace(last=False):
                    nonlocal tctr
                    tctr += 1
                    if last:
                        return
                    want = tctr * nfeed // ntile
                    while feed and (nfeed - len(feed)) < want:
                        feed.pop(0)()
                qb = b * ST
                nkt = 4 * b + 4
                # diagonal tiles first, ascending j: tile j=0 claims the full
                # PSUM bank with a full-width start (the start bit arms zeroing
                # at 2KB-bank granularity, so sub-range starts are illegal);
                # later diagonal tiles accumulate their live sub-range only
                tiles = [4 * b, 4 * b + 1, 4 * b + 2, 4 * b + 3] \
                    + list(range(0, 4 * b))
                for hp in range(2):
                    pvA = psA.tile([128, ST], f32, tag="pva")
                    pvB = psB.tile([128, ST], f32, tag="pvb")
                    wts = {}

                    def emit_pv(i):
                        kt = tiles[i]
                        j = kt - 4 * b
                        last = (i == nkt - 1)
                        lo = 128 * j if j > 0 else 0
                        st_ = (i == 0)
                        wt = wts.pop(i)
                        nc.tensor.matmul(pvA[0:VA, lo:512],
                                         va_sc[:, kt, hp, 0:VA],
                                         wt[:, lo:512],
                                         start=st_, stop=last,
                                         skip_group_check=True)
                        nc.tensor.matmul(pvB[:, lo:512],
                                         va_sc[:, kt, hp, VA:VHP],
                                         wt[:, 512 + lo:1024],
                                         start=st_, stop=last,
                                         skip_group_check=True)

                    for i, kt in enumerate(tiles):
                        j = kt - 4 * b
                        qlo = 128 * j if j >= 0 else 0
                        ksl = slice(kt * 128, (kt + 1) * 128)
                        scp = psS.tile([128, 2 * ST], f32, tag="sc")
                        for h in range(2):
                            nc.tensor.matmul(
                                scp[:, 512 * h + qlo:512 * h + 512],
                                kT[hp][64 * h:64 * h + 64, ksl],
                                qT[hp][64 * h:64 * h + 64,
                                       qb + qlo:qb + 512],
                                start=True, stop=True)
                        wt = wtp.tile([128, 2 * ST], bf16, tag="wt")
                        scv = scp[:].rearrange("p (h q) -> p h q", h=2)
                        wtv = wt[:].rearrange("p (h q) -> p h q", h=2)
                        nc.scalar.activation(wtv[:, :, qlo:512],
                                             scv[:, :, qlo:512],
                                             AF.Exp, scale=0.125)
                        if j >= 0:
                            nc.vector.tensor_tensor(
                                wtv[:, :, qlo:qlo + 128],
                                wtv[:, :, qlo:qlo + 128], tri2, Alu.mult)
                        wts[i] = wt
                        if i >= LOOK:
                            emit_pv(i - LOOK)
                        pace(last=HOLD and (i >= nkt - 2))
                    for i in range(max(0, nkt - LOOK), nkt):
                        emit_pv(i)

                    # softmax normalization: denominator rows -> SBUF bf16,
                    # broadcast across partitions via a PE matmul with ones
                    # vectors, one full-tile reciprocal, then one
                    # psum-x-sbuf multiply per head half
                    dnA = rcp.tile([1, ST], bf16, tag="r")
                    dnB = rcp.tile([1, ST], bf16, tag="r")
                    nc.scalar.copy(dnA[:], pvA[64:65, :])
                    nc.vector.tensor_copy(dnB[:], pvB[0:1, :])
                    bcp = psW.tile([128, ST], f32, tag="w")
                    nc.tensor.matmul(bcp[:], onesA, dnA[:],
                                     start=True, stop=False)
                    nc.tensor.matmul(bcp[:], onesB, dnB[:],
                                     start=False, stop=True)
                    bc = rcp.tile([128, ST], f32, tag="bc")
                    nc.vector.reciprocal_approx_fast(bc[:], bcp[:])
                    tail_c = tail_chunks if hp == 1 else []
                    for ci in range(2):
                        csl = slice(ci * 256, (ci + 1) * 256)
                        qsl = slice(qb + ci * 256, qb + (ci + 1) * 256)
                        nc.vector.tensor_tensor(concatT[hp][0:64, qsl],
                                                pvA[0:64, csl], bc[0:64, csl],
                                                Alu.mult)
                        nc.vector.tensor_tensor(concatT[hp][64:128, qsl],
                                                pvB[64:128, csl],
                                                bc[64:128, csl],
                                                Alu.mult)
                        for f in tail_c[2 * ci:2 * ci + 2]:
                            f()
                    if hp == 1:
                        for f in feed:
                            f()
                        feed = []
                    if dbg and b == 0 and hp == 0:
                        nc.vector.tensor_copy(bdbg_sb[0:65, 0:ST],
                                              pvA[0:65, :])
                        nc.vector.tensor_copy(bdbg_sb[:, ST:2 * ST], pvB[:])
                        nc.sync.dma_start(bdbg[:], bdbg_sb[:])

            def phase_c_chunks(b, narrow=False, alternate=False):
                def mk(sc, narrow=narrow):
                    def f():
                        ssl = slice(sc * 128, (sc + 1) * 128)
                        ob = obp.tile([128, D], bf16, tag="ob", name="ob")
                        if narrow:
                            ops = [psW.tile([128, ST], f32, tag="w", name="op")
                                   for _ in range(2)]
                        else:
                            opw = psS.tile([128, D], f32, tag="sc", name="op")
                            ops = [opw[:, 0:512], opw[:, 512:1024]]
                        for dh in range(2):
                            dsl = slice(dh * 512, (dh + 1) * 512)
                            o = ops[dh][:] if narrow else ops[dh]
                            nc.tensor.matmul(o, concatT[0][:, ssl],
                                             woT_sb[:, 0, dsl],
                                             start=True, stop=False)
                            nc.tensor.matmul(o, concatT[1][:, ssl],
                                             woT_sb[:, 1, dsl],
                                             start=False, stop=True)
                        o0 = ops[0][:] if narrow else ops[0]
                        o1 = ops[1][:] if narrow else ops[1]
                        nc.scalar.copy(ob[:, 0:512], o0)
                        nc.sync.dma_start(out[ssl, 0:512], ob[:, 0:512])
                        nc.vector.tensor_copy(ob[:, 512:1024], o1)
                        nc.sync.dma_start(out[ssl, 512:1024], ob[:, 512:1024])
                    return f
                if alternate:
                    return [mk(sc, narrow=(sc % 2 == 1))
                            for sc in range(4 * b, 4 * b + 4)]
                return [mk(sc) for sc in range(4 * b, 4 * b + 4)]

            for f in proj_chunks(0):
                f()
            for st in range(4):
                pj = proj_chunks(st + 1) if st < 3 else []
                if st == 1:
                    cc = phase_c_chunks(0)
                elif st == 3:
                    cc = phase_c_chunks(1, narrow=True) \
                        + phase_c_chunks(2, narrow=True)
                else:
                    cc = []
                feed = []
                while pj or cc:
                    if pj:
                        feed.append(pj.pop(0))
                    if cc:
                        feed.append(cc.pop(0))
                band(st, feed=feed,
                     tail_chunks=phase_c_chunks(3) if st == 3 else ())
            if dbg:
                for i in range(2):
                    nc.sync.dma_start(qdbg[:, i * S:(i + 1) * S], qT[i][:])
                    nc.sync.dma_start(kdbg[:, i * S:(i + 1) * S], kT[i][:])
                    nc.sync.dma_start(cdbg[:, i * S:(i + 1) * S], concatT[i][:])
                nc.sync.dma_start(vdbg[:], v_aug[:])
    nc.finalize()
    return nc


def _rope_tables():
    inv_freq = 1.0 / (THETA ** (np.arange(0, DK, 2, dtype=np.float64) / DK))
    t = np.arange(S, dtype=np.float64)
    freqs = np.outer(t, inv_freq)
    emb = np.stack((freqs, freqs), axis=-1).reshape(S, DK)
    return np.cos(emb).astype(np.float32), np.sin(emb).astype(np.float32)


def _host_consts():
    consts = np.zeros((128, C_W), np.float32)  # vpat appended below
    p = np.arange(128)
    tri = (p[None, :] >= p[:, None])
    consts[:, C_TRI:C_TRI + 128] = tri
    consts[:, C_TRI + 128:C_TRI + 256] = tri
    # rot lhsT: lhsT[2i+1, 2i] = -1, lhsT[2i, 2i+1] = +1
    m = np.zeros((128, 128), np.float32)
    i2 = np.arange(0, 128, 2)
    m[i2 + 1, i2] = -1.0
    m[i2, i2 + 1] = 1.0
    consts[:, C_ROT:C_ROT + 128] = m
    consts[0, C_ONA:C_ONA + 64] = 1.0
    consts[0, C_ONB + 64:C_ONB + 128] = 1.0

    vpat_np = np.zeros((128, VSC), np.float32)
    for r in range(2):
        base = r * VHP
        vpat_np[:, base + 64] = 1.0   # A ones column
        vpat_np[:, base + VA] = 1.0   # B ones column
    return (np.concatenate([consts, vpat_np], axis=1)
            .astype(ml_dtypes.bfloat16),)


def kernel(x, token_positions, W_q, W_k, W_v, W_o):
    global _NC, _CONSTS
    if _NC is None:
        _NC = _build()
    x = np.asarray(x, dtype=np.float32)
    token_positions = np.asarray(token_positions)
    W_q = np.asarray(W_q, dtype=np.float32)
    W_k = np.asarray(W_k, dtype=np.float32)
    W_v = np.asarray(W_v, dtype=np.float32)
    W_o = np.asarray(W_o, dtype=np.float32)

    if _CONSTS is None:
        _CONSTS = (*_rope_tables(), *_host_consts())
    cos_t, sin_t, consts_np = _CONSTS

    in_maps = []
    for c in range(8):
        b, g = divmod(c, 4)
        rows = slice(256 * g, 256 * (g + 1))
        pw_np = np.concatenate(
            [W_q[rows].T, W_k[rows].T, W_v[rows].T], axis=1
        ).astype(ml_dtypes.bfloat16)
        woT_np = W_o[:, rows].T.astype(ml_dtypes.bfloat16)
        pos = np.asarray(token_positions[b], dtype=np.int64)
        cosT = np.tile(cos_t[pos].T, (2, 1))
        sinT = np.tile(sin_t[pos].T, (2, 1))
        cossin_np = np.concatenate([cosT, sinT], axis=1).astype(
            ml_dtypes.bfloat16)
        xT_np = x[b].T.astype(ml_dtypes.bfloat16)
        in_maps.append({
            "xT": np.ascontiguousarray(xT_np),
            "pw": np.ascontiguousarray(pw_np),
            "woT": np.ascontiguousarray(woT_np),
            "cossin": np.ascontiguousarray(cossin_np),
            "consts": consts_np,
        })

    global _LAST_RES
    res = run_bass_kernel_spmd(_NC, in_maps, core_ids=list(range(8)))
    _LAST_RES = res
    outs = [res.results[c]["out"] for c in range(8)]
    o0 = outs[0] + outs[1] + outs[2] + outs[3]
    o1 = outs[4] + outs[5] + outs[6] + outs[7]
    return np.stack([o0, o1]).astype(np.float32)


# revision 58
# speedup vs baseline: 1.8898x; 1.0177x over previous
"""TRN2 Bass/Tile kernel: causal self-attention with RoPE (bf16 pipeline).

Sharding across 8 NeuronCores: batch (2) x head-groups (4 groups of 4 heads).
Per core, for its batch and 4 heads (2 head-pairs "hp" of 2 heads each):

- Phase A: Q/K/V projections in bf16. RoPE is applied as
  q_rope = q*cos + rot(q*sin) where rot is a fixed pair-swap/sign
  permutation executed as a single [128x128] matmul on the PE (the
  interleaved cos/sin tables are pair-equal so rot commutes with them).
- Phase B: causal attention in scores^T orientation ([keys, q] tiles).
  Diagonal key-tiles are restricted to their live q-range and masked with
  one [128,128] triangle on DVE; fully-masked regions are never computed.
  Softmax denominators come from ones-columns in the augmented V (extra
  PSUM rows are free: matmul cost depends only on the moving free size).
- Phase C: output projection partials, summed on the host.

All matmuls bf16 with fp32 PSUM accumulation.
"""
import numpy as np
import ml_dtypes
import concourse.bass as bass
from concourse import bacc
import concourse.mybir as mybir
import concourse.tile as tile
from concourse.bass_utils import run_bass_kernel_spmd

B, S, D = 2, 2048, 1024
H, DK = 16, 64
THETA = 10000.0
ST = 512              # q-band width
NSC = S // 128        # 16 key chunks of 128
f32 = mybir.dt.float32
bf16 = mybir.dt.bfloat16
AF = mybir.ActivationFunctionType
Alu = mybir.AluOpType

# v_aug layout per key-chunk, per head pair:
# A head [v(64) | one] = 65 cols -> psum rows 0:64 attn, row 64 denom
# B head [one | zeros(63) | v(64)] = 128 cols -> row 0 denom, rows 64:128 attn
VA = 65
VB = 128
VHP = VA + VB         # 193
VSC = 2 * VHP         # 386

# consts layout: [tri(256 = 2 copies) | mrotT(128) | onesA(128) | onesB(128)]
C_TRI = 0
C_ROT = 256
C_ONA = 384
C_ONB = 512
C_W = 640

_NC = None
_CONSTS = None
_LAST_RES = None


def _build():
    nc = bacc.Bacc()
    xT = nc.dram_tensor("xT", [D, S], bf16, kind="ExternalInput")
    pw = nc.dram_tensor("pw", [D, 768], bf16, kind="ExternalInput")
    woT = nc.dram_tensor("woT", [256, D], bf16, kind="ExternalInput")
    cossin = nc.dram_tensor("cossin", [128, 2 * S], bf16, kind="ExternalInput")
    consts = nc.dram_tensor("consts", [128, C_W + VSC], bf16,
                            kind="ExternalInput")
    out = nc.dram_tensor("out", [S, D], bf16, kind="ExternalOutput")
    import os
    dbg = os.environ.get("K_DEBUG", "") == "1"
    LOOK = int(os.environ.get("K_LOOK", "3"))
    HOLD = os.environ.get("K_HOLD", "0") == "1"
    if dbg:
        qdbg = nc.dram_tensor("qdbg", [128, 2 * S], bf16, kind="ExternalOutput")
        kdbg = nc.dram_tensor("kdbg", [128, 2 * S], bf16, kind="ExternalOutput")
        vdbg = nc.dram_tensor("vdbg", [128, NSC * VSC], bf16,
                              kind="ExternalOutput")
        cdbg = nc.dram_tensor("cdbg", [128, 2 * S], bf16, kind="ExternalOutput")
        bdbg = nc.dram_tensor("bdbg", [128, 2 * ST], f32, kind="ExternalOutput")

    with tile.TileContext(nc) as tc:
        with tc.tile_pool(name="persist", bufs=1) as pp, \
             tc.tile_pool(name="rope", bufs=4) as rp, \
             tc.tile_pool(name="wtp", bufs=6) as wtp, \
             tc.tile_pool(name="rcp", bufs=4) as rcp, \
             tc.tile_pool(name="obp", bufs=3) as obp, \
             tc.tile_pool(name="psS", bufs=2, space="PSUM") as psS, \
             tc.tile_pool(name="psW", bufs=2, space="PSUM") as psW, \
             tc.tile_pool(name="psA", bufs=1, space="PSUM") as psA, \
             tc.tile_pool(name="psB", bufs=1, space="PSUM") as psB:
            qT = [pp.tile([128, S], bf16, tag=f"qT{i}", name=f"qT{i}") for i in range(2)]
            kT = [pp.tile([128, S], bf16, tag=f"kT{i}", name=f"kT{i}") for i in range(2)]
            v_aug = pp.tile([128, NSC * VSC], bf16, tag="vaug")
            concatT = [pp.tile([128, S], bf16, tag=f"cT{i}", name=f"cT{i}") for i in range(2)]
            woT_sb = pp.tile([128, 2, D], bf16, tag="woT")
            cs_sb = pp.tile([128, 2, S], bf16, tag="cs")
            co_sb = pp.tile([128, C_W + VSC], bf16, tag="consts")
            pw_sb = pp.tile([128, 8, 768], bf16, tag="pw")
            xs = pp.tile([128, 8, S], bf16, tag="xs")

            # input DMAs, in need-order (SP queue drains FIFO)
            pw_r = pw[:].rearrange("(k p) m -> p k m", p=128)
            xs_r = xT[:].rearrange("(k p) m -> p k m", p=128)
            va_c = v_aug[:].rearrange("p (c r) -> p c r", c=NSC)
            nc.sync.dma_start(pw_sb[:, :, 0:256], pw_r[:, :, 0:256])
            nc.scalar.dma_start(xs[:, :, 0:ST], xs_r[:, :, 0:ST])
            nc.sync.dma_start(co_sb[:], consts[:])
            nc.vector.tensor_copy(va_c[:, 0, :], co_sb[:, C_W:C_W + VSC])
            nc.sync.dma_start(pw_sb[:, :, 256:768], pw_r[:, :, 256:768])
            nc.scalar.dma_start(cs_sb[:],
                                cossin[:].rearrange("p (c s) -> p c s", c=2))
            nc.scalar.dma_start(xs[:, :, ST:2 * ST], xs_r[:, :, ST:2 * ST])
            nc.scalar.dma_start(xs[:, :, 2 * ST:3 * ST],
                                xs_r[:, :, 2 * ST:3 * ST])
            nc.scalar.dma_start(xs[:, :, 3 * ST:4 * ST],
                                xs_r[:, :, 3 * ST:4 * ST])
            nc.sync.dma_start(woT_sb[:],
                              woT[:].rearrange("(k p) m -> p k m", p=128))
            # replicate the ones/zeros pattern to the other 15 key-chunks on
            # the otherwise-idle Pool engine (SBUF->SBUF)
            for c in range(1, NSC):
                nc.gpsimd.tensor_copy(va_c[:, c, :], va_c[:, 0, :])

            if dbg:
                bdbg_sb = pp.tile([128, 2 * ST], f32, tag="bdbg")
                nc.vector.memset(bdbg_sb[:], 0.0)
            va_sc = v_aug[:].rearrange("p (c h r) -> p c h r", c=NSC, r=VHP)
            tri2 = co_sb[:, C_TRI:C_TRI + 256].rearrange(
                "p (h e) -> p h e", h=2)
            mrotT = co_sb[:, C_ROT:C_ROT + 128]
            onesA = co_sb[0:1, C_ONA:C_ONA + 128]
            onesB = co_sb[0:1, C_ONB:C_ONB + 128]

            def proj_chunks(st):
                """Projection work for band st as a list of issue-chunks so
                band(st-1) can interleave them into its PE stream: 4 Q/K
                units, one rots+adds chunk, 4 V chunks."""
                sl = slice(st * ST, (st + 1) * ST)
                units = []

                def mk_qk(hp, coff):
                    def f():
                        dstT = qT if coff == 0 else kT
                        ps = psW.tile([128, ST], f32, tag="w", name="ps")
                        o = coff + 128 * hp
                        for kt in range(8):
                            nc.tensor.matmul(ps[:], pw_sb[:, kt, o:o + 128],
                                             xs[:, kt, sl],
                                             start=(kt == 0), stop=(kt == 7))
                        t1 = rp.tile([128, ST], bf16, tag="t1", name="t1")
                        ts = rp.tile([128, ST], bf16, tag="ts", name="ts")
                        nc.vector.tensor_tensor(t1[:], ps[:], cs_sb[:, 0, sl],
                                                Alu.mult)
                        nc.vector.tensor_tensor(ts[:], ps[:], cs_sb[:, 1, sl],
                                                Alu.mult)
                        units.append((t1, ts, dstT, hp))
                    return f

                def rots():
                    for t1, ts, dstT, hp in units:
                        rot = psW.tile([128, ST], f32, tag="w", name="rot")
                        nc.tensor.matmul(rot[:], mrotT, ts[:],
                                         start=True, stop=True)
                        nc.vector.tensor_tensor(dstT[hp][:, sl], t1[:],
                                                rot[:], Alu.add)

                def mk_v(scl):
                    def f():
                        sc = st * 4 + scl
                        vp = psW.tile([128, ST], f32, tag="w", name="vp")
                        lo = st * ST + scl * 128
                        for kt in range(8):
                            nc.tensor.matmul(vp[:, 0:256],
                                             xs[:, kt, lo:lo + 128],
                                             pw_sb[:, kt, 512:768],
                                             start=(kt == 0), stop=(kt == 7))
                        vr = vp[:, 0:256].rearrange("p (g t e) -> p g t e",
                                                    g=2, t=2)
                        nc.vector.tensor_copy(va_sc[:, sc, :, 0:64],
                                              vr[:, :, 0, :])
                        nc.vector.tensor_copy(va_sc[:, sc, :, 129:193],
                                              vr[:, :, 1, :])
                    return f

                return [mk_qk(0, 0), mk_qk(1, 0), mk_qk(0, 256),
                        mk_qk(1, 256), rots,
                        mk_v(0), mk_v(1), mk_v(2), mk_v(3)]

            def band(b, feed=(), tail_chunks=()):
                feed = list(feed)
                ntile = 2 * (4 * b + 4)
                nfeed = len(feed)
                tctr = 0

                def pace(last=False):
                    nonlocal tctr
                    tctr += 1
                    if last:
                        return
                    want = tctr * nfeed // ntile
                    while feed and (nfeed - len(feed)) < want:
                        feed.pop(0)()
                qb = b * ST
                nkt = 4 * b + 4
                # diagonal tiles first, ascending j: tile j=0 claims the full
                # PSUM bank with a full-width start (the start bit arms zeroing
                # at 2KB-bank granularity, so sub-range starts are illegal);
                # later diagonal tiles accumulate their live sub-range only
                tiles = [4 * b, 4 * b + 1, 4 * b + 2, 4 * b + 3] \
                    + list(range(0, 4 * b))
                for hp in range(2):
                    pvA = psA.tile([128, ST], f32, tag="pva")
                    pvB = psB.tile([128, ST], f32, tag="pvb")
                    wts = {}

                    def emit_pv(i):
                        kt = tiles[i]
                        j = kt - 4 * b
                        last = (i == nkt - 1)
                        lo = 128 * j if j > 0 else 0
                        st_ = (i == 0)
                        wt = wts.pop(i)
                        nc.tensor.matmul(pvA[0:VA, lo:512],
                                         va_sc[:, kt, hp, 0:VA],
                                         wt[:, lo:512],
                                         start=st_, stop=last,
                                         skip_group_check=True)
                        nc.tensor.matmul(pvB[:, lo:512],
                                         va_sc[:, kt, hp, VA:VHP],
                                         wt[:, 512 + lo:1024],
                                         start=st_, stop=last,
                                         skip_group_check=True)

                    for i, kt in enumerate(tiles):
                        j = kt - 4 * b
                        qlo = 128 * j if j >= 0 else 0
                        ksl = slice(kt * 128, (kt + 1) * 128)
                        scp = psS.tile([128, 2 * ST], f32, tag="sc")
                        for h in range(2):
                            nc.tensor.matmul(
                                scp[:, 512 * h + qlo:512 * h + 512],
                                kT[hp][64 * h:64 * h + 64, ksl],
                                qT[hp][64 * h:64 * h + 64,
                                       qb + qlo:qb + 512],
                                start=True, stop=True)
                        wt = wtp.tile([128, 2 * ST], bf16, tag="wt")
                        scv = scp[:].rearrange("p (h q) -> p h q", h=2)
                        wtv = wt[:].rearrange("p (h q) -> p h q", h=2)
                        nc.scalar.activation(wtv[:, :, qlo:512],
                                             scv[:, :, qlo:512],
                                             AF.Exp, scale=0.125)
                        if j >= 0:
                            nc.vector.tensor_tensor(
                                wtv[:, :, qlo:qlo + 128],
                                wtv[:, :, qlo:qlo + 128], tri2, Alu.mult)
                        wts[i] = wt
                        if i >= LOOK:
                            emit_pv(i - LOOK)
                        pace(last=HOLD and (i >= nkt - 2))
                    for i in range(max(0, nkt - LOOK), nkt):
                        emit_pv(i)

                    # softmax normalization: denominator rows -> SBUF bf16,
                    # broadcast across partitions via a PE matmul with ones
                    # vectors, one full-tile reciprocal, then one
                    # psum-x-sbuf multiply per head half
                    dnA = rcp.tile([1, ST], bf16, tag="r")
                    dnB = rcp.tile([1, ST], bf16, tag="r")
                    nc.scalar.copy(dnA[:], pvA[64:65, :])
                    nc.vector.tensor_copy(dnB[:], pvB[0:1, :])
                    bcp = psW.tile([128, ST], f32, tag="w")
                    nc.tensor.matmul(bcp[:], onesA, dnA[:],
                                     start=True, stop=False)
                    nc.tensor.matmul(bcp[:], onesB, dnB[:],
                                     start=False, stop=True)
                    if feed:
                        feed.pop(0)()
                    bc = rcp.tile([128, ST], f32, tag="bc")
                    nc.vector.reciprocal_approx_fast(bc[:], bcp[:])
                    tail_c = tail_chunks if hp == 1 else []
                    for ci in range(2):
                        csl = slice(ci * 256, (ci + 1) * 256)
                        qsl = slice(qb + ci * 256, qb + (ci + 1) * 256)
                        nc.vector.tensor_tensor(concatT[hp][0:64, qsl],
                                                pvA[0:64, csl], bc[0:64, csl],
                                                Alu.mult)
                        nc.vector.tensor_tensor(concatT[hp][64:128, qsl],
                                                pvB[64:128, csl],
                                                bc[64:128, csl],
                                                Alu.mult)
                        for f in tail_c[2 * ci:2 * ci + 2]:
                            f()
                    if hp == 1:
                        for f in feed:
                            f()
                        feed = []
                    if dbg and b == 0 and hp == 0:
                        nc.vector.tensor_copy(bdbg_sb[0:65, 0:ST],
                                              pvA[0:65, :])
                        nc.vector.tensor_copy(bdbg_sb[:, ST:2 * ST], pvB[:])
                        nc.sync.dma_start(bdbg[:], bdbg_sb[:])

            def phase_c_chunks(b, narrow=False, alternate=False,
                               tailq=False):
                def mk(sc, narrow=narrow):
                    def f():
                        ssl = slice(sc * 128, (sc + 1) * 128)
                        ob = obp.tile([128, D], bf16, tag="ob", name="ob")
                        if narrow:
                            ops = [psW.tile([128, ST], f32, tag="w", name="op")
                                   for _ in range(2)]
                        else:
                            opw = psS.tile([128, D], f32, tag="sc", name="op")
                            ops = [opw[:, 0:512], opw[:, 512:1024]]
                        for dh in range(2):
                            dsl = slice(dh * 512, (dh + 1) * 512)
                            o = ops[dh][:] if narrow else ops[dh]
                            nc.tensor.matmul(o, concatT[0][:, ssl],
                                             woT_sb[:, 0, dsl],
                                             start=True, stop=False)
                            nc.tensor.matmul(o, concatT[1][:, ssl],
                                             woT_sb[:, 1, dsl],
                                             start=False, stop=True)
                        o0 = ops[0][:] if narrow else ops[0]
                        o1 = ops[1][:] if narrow else ops[1]
                        nc.scalar.copy(ob[:, 0:512], o0)
                        nc.sync.dma_start(out[ssl, 0:512], ob[:, 0:512])
                        nc.vector.tensor_copy(ob[:, 512:1024], o1)
                        dq = nc.gpsimd if tailq else nc.sync
                        dq.dma_start(out[ssl, 512:1024], ob[:, 512:1024])
                    return f
                if alternate:
                    return [mk(sc, narrow=(sc % 2 == 1))
                            for sc in range(4 * b, 4 * b + 4)]
                return [mk(sc) for sc in range(4 * b, 4 * b + 4)]

            for f in proj_chunks(0):
                f()
            for st in range(4):
                pj = proj_chunks(st + 1) if st < 3 else []
                if st == 1:
                    cc = phase_c_chunks(0)
                elif st == 3:
                    cc = phase_c_chunks(1, narrow=True) \
                        + phase_c_chunks(2, narrow=True)
                else:
                    cc = []
                feed = []
                while pj or cc:
                    if pj:
                        feed.append(pj.pop(0))
                    if cc:
                        feed.append(cc.pop(0))
                band(st, feed=feed,
                     tail_chunks=phase_c_chunks(3, alternate=True,
                                                tailq=True)
                     if st == 3 else ())
            if dbg:
                for i in range(2):
                    nc.sync.dma_start(qdbg[:, i * S:(i + 1) * S], qT[i][:])
                    nc.sync.dma_start(kdbg[:, i * S:(i + 1) * S], kT[i][:])
                    nc.sync.dma_start(cdbg[:, i * S:(i + 1) * S], concatT[i][:])
                nc.sync.dma_start(vdbg[:], v_aug[:])
    nc.finalize()
    return nc


def _rope_tables():
    inv_freq = 1.0 / (THETA ** (np.arange(0, DK, 2, dtype=np.float64) / DK))
    t = np.arange(S, dtype=np.float64)
    freqs = np.outer(t, inv_freq)
    emb = np.stack((freqs, freqs), axis=-1).reshape(S, DK)
    return np.cos(emb).astype(np.float32), np.sin(emb).astype(np.float32)


def _host_consts():
    consts = np.zeros((128, C_W), np.float32)  # vpat appended below
    p = np.arange(128)
    tri = (p[None, :] >= p[:, None])
    consts[:, C_TRI:C_TRI + 128] = tri
    consts[:, C_TRI + 128:C_TRI + 256] = tri
    # rot lhsT: lhsT[2i+1, 2i] = -1, lhsT[2i, 2i+1] = +1
    m = np.zeros((128, 128), np.float32)
    i2 = np.arange(0, 128, 2)
    m[i2 + 1, i2] = -1.0
    m[i2, i2 + 1] = 1.0
    consts[:, C_ROT:C_ROT + 128] = m
    consts[0, C_ONA:C_ONA + 64] = 1.0
    consts[0, C_ONB + 64:C_ONB + 128] = 1.0

    vpat_np = np.zeros((128, VSC), np.float32)
    for r in range(2):
        base = r * VHP
        vpat_np[:, base + 64] = 1.0   # A ones column
        vpat_np[:, base + VA] = 1.0   # B ones column
    return (np.concatenate([consts, vpat_np], axis=1)
            .astype(ml_dtypes.bfloat16),)


def kernel(x, token_positions, W_q, W_k, W_v, W_o):
    global _NC, _CONSTS
    if _NC is None:
        _NC = _build()
    x = np.asarray(x, dtype=np.float32)
    token_positions = np.asarray(token_positions)
    W_q = np.asarray(W_q, dtype=np.float32)
    W_k = np.asarray(W_k, dtype=np.float32)
    W_v = np.asarray(W_v, dtype=np.float32)
    W_o = np.asarray(W_o, dtype=np.float32)

    if _CONSTS is None:
        _CONSTS = (*_rope_tables(), *_host_consts())
    cos_t, sin_t, consts_np = _CONSTS

    in_maps = []
    for c in range(8):
        b, g = divmod(c, 4)
        rows = slice(256 * g, 256 * (g + 1))
        pw_np = np.concatenate(
            [W_q[rows].T, W_k[rows].T, W_v[rows].T], axis=1
        ).astype(ml_dtypes.bfloat16)
        woT_np = W_o[:, rows].T.astype(ml_dtypes.bfloat16)
        pos = np.asarray(token_positions[b], dtype=np.int64)
        cosT = np.tile(cos_t[pos].T, (2, 1))
        sinT = np.tile(sin_t[pos].T, (2, 1))
        cossin_np = np.concatenate([cosT, sinT], axis=1).astype(
            ml_dtypes.bfloat16)
        xT_np = x[b].T.astype(ml_dtypes.bfloat16)
        in_maps.append({
            "xT": np.ascontiguousarray(xT_np),
            "pw": np.ascontiguousarray(pw_np),
            "woT": np.ascontiguousarray(woT_np),
            "cossin": np.ascontiguousarray(cossin_np),
            "consts": consts_np,
        })

    global _LAST_RES
    res = run_bass_kernel_spmd(_NC, in_maps, core_ids=list(range(8)))
    _LAST_RES = res
    outs = [res.results[c]["out"] for c in range(8)]
    o0 = outs[0] + outs[1] + outs[2] + outs[3]
    o1 = outs[4] + outs[5] + outs[6] + outs[7]
    return np.stack([o0, o1]).astype(np.float32)


# revision 66
# speedup vs baseline: 1.8982x; 1.0045x over previous
"""TRN2 Bass/Tile kernel: causal self-attention with RoPE (bf16 pipeline).

Sharding across 8 NeuronCores: batch (2) x head-groups (4 groups of 4 heads).
Per core, for its batch and 4 heads (2 head-pairs "hp" of 2 heads each):

- Phase A: Q/K/V projections in bf16. RoPE is applied as
  q_rope = q*cos + rot(q*sin) where rot is a fixed pair-swap/sign
  permutation executed as a single [128x128] matmul on the PE (the
  interleaved cos/sin tables are pair-equal so rot commutes with them).
- Phase B: causal attention in scores^T orientation ([keys, q] tiles).
  Diagonal key-tiles are restricted to their live q-range and masked with
  one [128,128] triangle on DVE; fully-masked regions are never computed.
  Softmax denominators come from ones-columns in the augmented V (extra
  PSUM rows are free: matmul cost depends only on the moving free size).
- Phase C: output projection partials, summed on the host.

All matmuls bf16 with fp32 PSUM accumulation.
"""
import numpy as np
import ml_dtypes
import concourse.bass as bass
from concourse import bacc
import concourse.mybir as mybir
import concourse.tile as tile
from concourse.bass_utils import run_bass_kernel_spmd

B, S, D = 2, 2048, 1024
H, DK = 16, 64
THETA = 10000.0
ST = 512              # q-band width
NSC = S // 128        # 16 key chunks of 128
f32 = mybir.dt.float32
bf16 = mybir.dt.bfloat16
AF = mybir.ActivationFunctionType
Alu = mybir.AluOpType

# v_aug layout per key-chunk, per head pair:
# A head [v(64) | one] = 65 cols -> psum rows 0:64 attn, row 64 denom
# B head [one | zeros(63) | v(64)] = 128 cols -> row 0 denom, rows 64:128 attn
VA = 65
VB = 128
VHP = VA + VB         # 193
VSC = 2 * VHP         # 386

# consts layout: [tri(256 = 2 copies) | mrotT(128) | onesA(128) | onesB(128)]
C_TRI = 0
C_ROT = 256
C_ONA = 384
C_ONB = 512
C_W = 640

_NC = None
_CONSTS = None
_LAST_RES = None


def _build():
    nc = bacc.Bacc()
    xT = nc.dram_tensor("xT", [D, S], bf16, kind="ExternalInput")
    pw = nc.dram_tensor("pw", [D, 768], bf16, kind="ExternalInput")
    woT = nc.dram_tensor("woT", [256, D], bf16, kind="ExternalInput")
    cossin = nc.dram_tensor("cossin", [128, 2 * S], bf16, kind="ExternalInput")
    consts = nc.dram_tensor("consts", [128, C_W + VSC], bf16,
                            kind="ExternalInput")
    out = nc.dram_tensor("out", [S, D], bf16, kind="ExternalOutput")
    import os
    dbg = os.environ.get("K_DEBUG", "") == "1"
    LOOK = int(os.environ.get("K_LOOK", "3"))
    HOLD = os.environ.get("K_HOLD", "0") == "1"
    if dbg:
        qdbg = nc.dram_tensor("qdbg", [128, 2 * S], bf16, kind="ExternalOutput")
        kdbg = nc.dram_tensor("kdbg", [128, 2 * S], bf16, kind="ExternalOutput")
        vdbg = nc.dram_tensor("vdbg", [128, NSC * VSC], bf16,
                              kind="ExternalOutput")
        cdbg = nc.dram_tensor("cdbg", [128, 2 * S], bf16, kind="ExternalOutput")
        bdbg = nc.dram_tensor("bdbg", [128, 2 * ST], f32, kind="ExternalOutput")

    with tile.TileContext(nc) as tc:
        with tc.tile_pool(name="persist", bufs=1) as pp, \
             tc.tile_pool(name="rope", bufs=4) as rp, \
             tc.tile_pool(name="wtp", bufs=6) as wtp, \
             tc.tile_pool(name="rcp", bufs=4) as rcp, \
             tc.tile_pool(name="obp", bufs=3) as obp, \
             tc.tile_pool(name="psS", bufs=2, space="PSUM") as psS, \
             tc.tile_pool(name="psW", bufs=2, space="PSUM") as psW, \
             tc.tile_pool(name="psA", bufs=1, space="PSUM") as psA, \
             tc.tile_pool(name="psB", bufs=1, space="PSUM") as psB:
            qT = [pp.tile([128, S], bf16, tag=f"qT{i}", name=f"qT{i}") for i in range(2)]
            kT = [pp.tile([128, S], bf16, tag=f"kT{i}", name=f"kT{i}") for i in range(2)]
            v_aug = pp.tile([128, NSC * VSC], bf16, tag="vaug")
            concatT = [pp.tile([128, S], bf16, tag=f"cT{i}", name=f"cT{i}") for i in range(2)]
            woT_sb = pp.tile([128, 2, D], bf16, tag="woT")
            cs_sb = pp.tile([128, 2, S], bf16, tag="cs")
            co_sb = pp.tile([128, C_W + VSC], bf16, tag="consts")
            pw_sb = pp.tile([128, 8, 768], bf16, tag="pw")
            xs = pp.tile([128, 8, S], bf16, tag="xs")

            # input DMAs, in need-order (SP queue drains FIFO)
            pw_r = pw[:].rearrange("(k p) m -> p k m", p=128)
            xs_r = xT[:].rearrange("(k p) m -> p k m", p=128)
            va_c = v_aug[:].rearrange("p (c r) -> p c r", c=NSC)
            nc.sync.dma_start(pw_sb[:, :, 0:256], pw_r[:, :, 0:256])
            nc.scalar.dma_start(xs[:, :, 0:ST], xs_r[:, :, 0:ST])
            nc.sync.dma_start(co_sb[:], consts[:])
            nc.vector.tensor_copy(va_c[:, 0, :], co_sb[:, C_W:C_W + VSC])
            nc.sync.dma_start(pw_sb[:, :, 256:768], pw_r[:, :, 256:768])
            nc.scalar.dma_start(cs_sb[:],
                                cossin[:].rearrange("p (c s) -> p c s", c=2))
            nc.scalar.dma_start(xs[:, :, ST:2 * ST], xs_r[:, :, ST:2 * ST])
            nc.scalar.dma_start(xs[:, :, 2 * ST:3 * ST],
                                xs_r[:, :, 2 * ST:3 * ST])
            nc.scalar.dma_start(xs[:, :, 3 * ST:4 * ST],
                                xs_r[:, :, 3 * ST:4 * ST])
            nc.sync.dma_start(woT_sb[:],
                              woT[:].rearrange("(k p) m -> p k m", p=128))
            # replicate the ones/zeros pattern to the other 15 key-chunks on
            # the otherwise-idle Pool engine (SBUF->SBUF)
            for c in range(1, NSC):
                nc.gpsimd.tensor_copy(va_c[:, c, :], va_c[:, 0, :])

            if dbg:
                bdbg_sb = pp.tile([128, 2 * ST], f32, tag="bdbg")
                nc.vector.memset(bdbg_sb[:], 0.0)
            va_sc = v_aug[:].rearrange("p (c h r) -> p c h r", c=NSC, r=VHP)
            tri2 = co_sb[:, C_TRI:C_TRI + 256].rearrange(
                "p (h e) -> p h e", h=2)
            mrotT = co_sb[:, C_ROT:C_ROT + 128]
            onesA = co_sb[0:1, C_ONA:C_ONA + 128]
            onesB = co_sb[0:1, C_ONB:C_ONB + 128]

            def proj_chunks(st):
                """Projection work for band st as a list of issue-chunks so
                band(st-1) can interleave them into its PE stream: 4 Q/K
                units, one rots+adds chunk, 4 V chunks."""
                sl = slice(st * ST, (st + 1) * ST)
                units = []

                def mk_qk(hp, coff):
                    def f():
                        dstT = qT if coff == 0 else kT
                        ps = psW.tile([128, ST], f32, tag="w", name="ps")
                        o = coff + 128 * hp
                        for kt in range(8):
                            nc.tensor.matmul(ps[:], pw_sb[:, kt, o:o + 128],
                                             xs[:, kt, sl],
                                             start=(kt == 0), stop=(kt == 7))
                        t1 = rp.tile([128, ST], bf16, tag="t1", name="t1")
                        ts = rp.tile([128, ST], bf16, tag="ts", name="ts")
                        nc.vector.tensor_tensor(t1[:], ps[:], cs_sb[:, 0, sl],
                                                Alu.mult)
                        nc.vector.tensor_tensor(ts[:], ps[:], cs_sb[:, 1, sl],
                                                Alu.mult)
                        units.append((t1, ts, dstT, hp))
                    return f

                def rots():
                    while units:
                        t1, ts, dstT, hp = units.pop(0)
                        rot = psW.tile([128, ST], f32, tag="w", name="rot")
                        nc.tensor.matmul(rot[:], mrotT, ts[:],
                                         start=True, stop=True)
                        nc.vector.tensor_tensor(dstT[hp][:, sl], t1[:],
                                                rot[:], Alu.add)

                def mk_v(scl):
                    def f():
                        sc = st * 4 + scl
                        vp = psW.tile([128, ST], f32, tag="w", name="vp")
                        lo = st * ST + scl * 128
                        for kt in range(8):
                            nc.tensor.matmul(vp[:, 0:256],
                                             xs[:, kt, lo:lo + 128],
                                             pw_sb[:, kt, 512:768],
                                             start=(kt == 0), stop=(kt == 7))
                        vr = vp[:, 0:256].rearrange("p (g t e) -> p g t e",
                                                    g=2, t=2)
                        nc.vector.tensor_copy(va_sc[:, sc, :, 0:64],
                                              vr[:, :, 0, :])
                        nc.vector.tensor_copy(va_sc[:, sc, :, 129:193],
                                              vr[:, :, 1, :])
                    return f

                return [mk_qk(0, 0), mk_qk(1, 0), mk_qk(0, 256), rots,
                        mk_qk(1, 256), rots,
                        mk_v(0), mk_v(1), mk_v(2), mk_v(3)]

            def band(b, feed=(), tail_chunks=()):
                feed = list(feed)
                ntile = 2 * (4 * b + 4)
                nfeed = len(feed)
                tctr = 0

                def pace(last=False):
                    nonlocal tctr
                    tctr += 1
                    if last:
                        return
                    want = tctr * nfeed // ntile
                    while feed and (nfeed - len(feed)) < want:
                        feed.pop(0)()
                qb = b * ST
                nkt = 4 * b + 4
                # diagonal tiles first, ascending j: tile j=0 claims the full
                # PSUM bank with a full-width start (the start bit arms zeroing
                # at 2KB-bank granularity, so sub-range starts are illegal);
                # later diagonal tiles accumulate their live sub-range only
                tiles = [4 * b, 4 * b + 1, 4 * b + 2, 4 * b + 3] \
                    + list(range(0, 4 * b))
                for hp in range(2):
                    pvA = psA.tile([128, ST], f32, tag="pva")
                    pvB = psB.tile([128, ST], f32, tag="pvb")
                    wts = {}

                    def emit_pv(i):
                        kt = tiles[i]
                        j = kt - 4 * b
                        last = (i == nkt - 1)
                        lo = 128 * j if j > 0 else 0
                        st_ = (i == 0)
                        wt = wts.pop(i)
                        nc.tensor.matmul(pvA[0:VA, lo:512],
                                         va_sc[:, kt, hp, 0:VA],
                                         wt[:, lo:512],
                                         start=st_, stop=last,
                                         skip_group_check=True)
                        nc.tensor.matmul(pvB[:, lo:512],
                                         va_sc[:, kt, hp, VA:VHP],
                                         wt[:, 512 + lo:1024],
                                         start=st_, stop=last,
                                         skip_group_check=True)

                    for i, kt in enumerate(tiles):
                        j = kt - 4 * b
                        qlo = 128 * j if j >= 0 else 0
                        ksl = slice(kt * 128, (kt + 1) * 128)
                        scp = psS.tile([128, 2 * ST], f32, tag="sc")
                        for h in range(2):
                            nc.tensor.matmul(
                                scp[:, 512 * h + qlo:512 * h + 512],
                                kT[hp][64 * h:64 * h + 64, ksl],
                                qT[hp][64 * h:64 * h + 64,
                                       qb + qlo:qb + 512],
                                start=True, stop=True)
                        wt = wtp.tile([128, 2 * ST], bf16, tag="wt")
                        scv = scp[:].rearrange("p (h q) -> p h q", h=2)
                        wtv = wt[:].rearrange("p (h q) -> p h q", h=2)
                        nc.scalar.activation(wtv[:, :, qlo:512],
                                             scv[:, :, qlo:512],
                                             AF.Exp, scale=0.125)
                        if j >= 0:
                            nc.vector.tensor_tensor(
                                wtv[:, :, qlo:qlo + 128],
                                wtv[:, :, qlo:qlo + 128], tri2, Alu.mult)
                        wts[i] = wt
                        if i >= LOOK:
                            emit_pv(i - LOOK)
                        pace(last=HOLD and (i >= nkt - 2))
                    for i in range(max(0, nkt - LOOK), nkt):
                        emit_pv(i)

                    # softmax normalization: denominator rows -> SBUF bf16,
                    # broadcast across partitions via a PE matmul with ones
                    # vectors, one full-tile reciprocal, then one
                    # psum-x-sbuf multiply per head half
                    dnA = rcp.tile([1, ST], bf16, tag="r")
                    dnB = rcp.tile([1, ST], bf16, tag="r")
                    nc.scalar.copy(dnA[:], pvA[64:65, :])
                    nc.vector.tensor_copy(dnB[:], pvB[0:1, :])
                    bcp = psW.tile([128, ST], f32, tag="w")
                    nc.tensor.matmul(bcp[:], onesA, dnA[:],
                                     start=True, stop=False)
                    nc.tensor.matmul(bcp[:], onesB, dnB[:],
                                     start=False, stop=True)
                    if feed:
                        feed.pop(0)()
                    bc = rcp.tile([128, ST], f32, tag="bc")
                    nc.vector.reciprocal_approx_fast(bc[:], bcp[:])
                    tail_c = tail_chunks if hp == 1 else []
                    for ci in range(2):
                        csl = slice(ci * 256, (ci + 1) * 256)
                        qsl = slice(qb + ci * 256, qb + (ci + 1) * 256)
                        nc.vector.tensor_tensor(concatT[hp][0:64, qsl],
                                                pvA[0:64, csl], bc[0:64, csl],
                                                Alu.mult)
                        nc.vector.tensor_tensor(concatT[hp][64:128, qsl],
                                                pvB[64:128, csl],
                                                bc[64:128, csl],
                                                Alu.mult)
                        for f in tail_c[2 * ci:2 * ci + 2]:
                            f()
                    if hp == 1:
                        for f in feed:
                            f()
                        feed = []
                    if dbg and b == 0 and hp == 0:
                        nc.vector.tensor_copy(bdbg_sb[0:65, 0:ST],
                                              pvA[0:65, :])
                        nc.vector.tensor_copy(bdbg_sb[:, ST:2 * ST], pvB[:])
                        nc.sync.dma_start(bdbg[:], bdbg_sb[:])

            def phase_c_chunks(b, narrow=False, alternate=False,
                               tailq=False):
                def mk(sc, narrow=narrow):
                    def f():
                        ssl = slice(sc * 128, (sc + 1) * 128)
                        ob = obp.tile([128, D], bf16, tag="ob", name="ob")
                        if narrow:
                            ops = [psW.tile([128, ST], f32, tag="w", name="op")
                                   for _ in range(2)]
                        else:
                            opw = psS.tile([128, D], f32, tag="sc", name="op")
                            ops = [opw[:, 0:512], opw[:, 512:1024]]
                        for dh in range(2):
                            dsl = slice(dh * 512, (dh + 1) * 512)
                            o = ops[dh][:] if narrow else ops[dh]
                            nc.tensor.matmul(o, concatT[0][:, ssl],
                                             woT_sb[:, 0, dsl],
                                             start=True, stop=False)
                            nc.tensor.matmul(o, concatT[1][:, ssl],
                                             woT_sb[:, 1, dsl],
                                             start=False, stop=True)
                        o0 = ops[0][:] if narrow else ops[0]
                        o1 = ops[1][:] if narrow else ops[1]
                        nc.scalar.copy(ob[:, 0:512], o0)
                        nc.sync.dma_start(out[ssl, 0:512], ob[:, 0:512])
                        nc.vector.tensor_copy(ob[:, 512:1024], o1)
                        dq = nc.gpsimd if tailq else nc.sync
                        dq.dma_start(out[ssl, 512:1024], ob[:, 512:1024])
                    return f
                if alternate:
                    return [mk(sc, narrow=(sc % 2 == 1))
                            for sc in range(4 * b, 4 * b + 4)]
                return [mk(sc) for sc in range(4 * b, 4 * b + 4)]

            for f in proj_chunks(0):
                f()
            for st in range(4):
                pj = proj_chunks(st + 1) if st < 3 else []
                if st == 1:
                    cc = phase_c_chunks(0)
                elif st == 3:
                    cc = phase_c_chunks(1, narrow=True) \
                        + phase_c_chunks(2, narrow=True)
                else:
                    cc = []
                feed = []
                while pj or cc:
                    if pj:
                        feed.append(pj.pop(0))
                    if cc:
                        feed.append(cc.pop(0))
                band(st, feed=feed,
                     tail_chunks=phase_c_chunks(3, alternate=True,
                                                tailq=True)
                     if st == 3 else ())
            if dbg:
                for i in range(2):
                    nc.sync.dma_start(qdbg[:, i * S:(i + 1) * S], qT[i][:])
                    nc.sync.dma_start(kdbg[:, i * S:(i + 1) * S], kT[i][:])
                    nc.sync.dma_start(cdbg[:, i * S:(i + 1) * S], concatT[i][:])
                nc.sync.dma_start(vdbg[:], v_aug[:])
    nc.finalize()
    return nc


def _rope_tables():
    inv_freq = 1.0 / (THETA ** (np.arange(0, DK, 2, dtype=np.float64) / DK))
    t = np.arange(S, dtype=np.float64)
    freqs = np.outer(t, inv_freq)
    emb = np.stack((freqs, freqs), axis=-1).reshape(S, DK)
    return np.cos(emb).astype(np.float32), np.sin(emb).astype(np.float32)


def _host_consts():
    consts = np.zeros((128, C_W), np.float32)  # vpat appended below
    p = np.arange(128)
    tri = (p[None, :] >= p[:, None])
    consts[:, C_TRI:C_TRI + 128] = tri
    consts[:, C_TRI + 128:C_TRI + 256] = tri
    # rot lhsT: lhsT[2i+1, 2i] = -1, lhsT[2i, 2i+1] = +1
    m = np.zeros((128, 128), np.float32)
    i2 = np.arange(0, 128, 2)
    m[i2 + 1, i2] = -1.0
    m[i2, i2 + 1] = 1.0
    consts[:, C_ROT:C_ROT + 128] = m
    consts[0, C_ONA:C_ONA + 64] = 1.0
    consts[0, C_ONB + 64:C_ONB + 128] = 1.0

    vpat_np = np.zeros((128, VSC), np.float32)
    for r in range(2):
        base = r * VHP
        vpat_np[:, base + 64] = 1.0   # A ones column
        vpat_np[:, base + VA] = 1.0   # B ones column
    return (np.concatenate([consts, vpat_np], axis=1)
            .astype(ml_dtypes.bfloat16),)


def kernel(x, token_positions, W_q, W_k, W_v, W_o):
    global _NC, _CONSTS
    if _NC is None:
        _NC = _build()
    x = np.asarray(x, dtype=np.float32)
    token_positions = np.asarray(token_positions)
    W_q = np.asarray(W_q, dtype=np.float32)
    W_k = np.asarray(W_k, dtype=np.float32)
    W_v = np.asarray(W_v, dtype=np.float32)
    W_o = np.asarray(W_o, dtype=np.float32)

    if _CONSTS is None:
        _CONSTS = (*_rope_tables(), *_host_consts())
    cos_t, sin_t, consts_np = _CONSTS

    in_maps = []
    for c in range(8):
        b, g = divmod(c, 4)
        rows = slice(256 * g, 256 * (g + 1))
        pw_np = np.concatenate(
            [W_q[rows].T, W_k[rows].T, W_v[rows].T], axis=1
        ).astype(ml_dtypes.bfloat16)
        woT_np = W_o[:, rows].T.astype(ml_dtypes.bfloat16)
        pos = np.asarray(token_positions[b], dtype=np.int64)
        cosT = np.tile(cos_t[pos].T, (2, 1))
        sinT = np.tile(sin_t[pos].T, (2, 1))
        cossin_np = np.concatenate([cosT, sinT], axis=1).astype(
            ml_dtypes.bfloat16)
        xT_np = x[b].T.astype(ml_dtypes.bfloat16)
        in_maps.append({
            "xT": np.ascontiguousarray(xT_np),
            "pw": np.ascontiguousarray(pw_np),
            "woT": np.ascontiguousarray(woT_np),
            "cossin": np.ascontiguousarray(cossin_np),
            "consts": consts_np,
        })

    global _LAST_RES
    res = run_bass_kernel_spmd(_NC, in_maps, core_ids=list(range(8)))
    _LAST_RES = res
    outs = [res.results[c]["out"] for c in range(8)]
    o0 = outs[0] + outs[1] + outs[2] + outs[3]
    o1 = outs[4] + outs[5] + outs[6] + outs[7]
    return np.stack([o0, o1]).astype(np.float32)


# revision 91
# speedup vs baseline: 1.9290x; 1.0162x over previous
"""TRN2 Bass/Tile kernel: causal self-attention with RoPE (bf16 pipeline).

Sharding across 8 NeuronCores: batch (2) x head-groups (4 groups of 4 heads).
Per core, for its batch and 4 heads (2 head-pairs "hp" of 2 heads each):

- Phase A: Q/K/V projections in bf16. RoPE is applied as
  q_rope = q*cos + rot(q*sin) where rot is a fixed pair-swap/sign
  permutation executed as a single [128x128] matmul on the PE (the
  interleaved cos/sin tables are pair-equal so rot commutes with them).
- Phase B: causal attention in scores^T orientation ([keys, q] tiles).
  Diagonal key-tiles are restricted to their live q-range and masked with
  one [128,128] triangle on DVE; fully-masked regions are never computed.
  Softmax denominators come from ones-columns in the augmented V (extra
  PSUM rows are free: matmul cost depends only on the moving free size).
- Phase C: output projection partials, summed on the host.

All matmuls bf16 with fp32 PSUM accumulation. The per-band pipeline
software-interleaves the next band's projection chunks and the previous
band's output-projection chunks into each band's instruction stream
(pace()/force-feed), so the PE stays busy through exp/normalization
latency. PSUM: 2x[128,1024] score tiles, 2x[128,512] work tiles,
2 PV accumulators (8 banks exactly).
"""
import numpy as np
import ml_dtypes
import concourse.bass as bass
from concourse import bacc
import concourse.mybir as mybir
import concourse.tile as tile
from concourse.bass_utils import run_bass_kernel_spmd

B, S, D = 2, 2048, 1024
H, DK = 16, 64
THETA = 10000.0
ST = 512              # q-band width
NSC = S // 128        # 16 key chunks of 128
f32 = mybir.dt.float32
bf16 = mybir.dt.bfloat16
AF = mybir.ActivationFunctionType
Alu = mybir.AluOpType

# v_aug layout per key-chunk, per head pair:
# A head [v(64) | one] = 65 cols -> psum rows 0:64 attn, row 64 denom
# B head [one | zeros(63) | v(64)] = 128 cols -> row 0 denom, rows 64:128 attn
VA = 65
VB = 128
VHP = VA + VB         # 193
VSC = 2 * VHP         # 386

# consts layout: [tri(256 = 2 copies) | mrotT(128) | onesA(128) | onesB(128)]
C_TRI = 0
C_ROT = 256
C_ONA = 384
C_ONB = 512
C_W = 640

_NC = None
_CONSTS = None
_LAST_RES = None


def _build():
    nc = bacc.Bacc()
    xT = nc.dram_tensor("xT", [D, S], bf16, kind="ExternalInput")
    pw = nc.dram_tensor("pw", [D, 768], bf16, kind="ExternalInput")
    woT = nc.dram_tensor("woT", [256, D], bf16, kind="ExternalInput")
    cossin = nc.dram_tensor("cossin", [64, 2 * S], bf16, kind="ExternalInput")
    consts = nc.dram_tensor("consts", [128, C_W + VSC], bf16,
                            kind="ExternalInput")
    out = nc.dram_tensor("out", [S, D], bf16, kind="ExternalOutput")
    import os
    dbg = os.environ.get("K_DEBUG", "") == "1"
    LOOK = int(os.environ.get("K_LOOK", "3"))
    HOLD = os.environ.get("K_HOLD", "0") == "1"
    if dbg:
        qdbg = nc.dram_tensor("qdbg", [128, 2 * S], bf16, kind="ExternalOutput")
        kdbg = nc.dram_tensor("kdbg", [128, 2 * S], bf16, kind="ExternalOutput")
        vdbg = nc.dram_tensor("vdbg", [128, NSC * VSC], bf16,
                              kind="ExternalOutput")
        cdbg = nc.dram_tensor("cdbg", [128, 2 * S], bf16, kind="ExternalOutput")
        bdbg = nc.dram_tensor("bdbg", [128, 2 * ST], f32, kind="ExternalOutput")

    with tile.TileContext(nc) as tc:
        with tc.tile_pool(name="persist", bufs=1) as pp, \
             tc.tile_pool(name="rope", bufs=4) as rp, \
             tc.tile_pool(name="wtp", bufs=6) as wtp, \
             tc.tile_pool(name="rcp", bufs=4) as rcp, \
             tc.tile_pool(name="obp", bufs=3) as obp, \
             tc.tile_pool(name="psS", bufs=2, space="PSUM") as psS, \
             tc.tile_pool(name="psW", bufs=2, space="PSUM") as psW, \
             tc.tile_pool(name="psA", bufs=1, space="PSUM") as psA, \
             tc.tile_pool(name="psB", bufs=1, space="PSUM") as psB:
            qT = [pp.tile([128, S], bf16, tag=f"qT{i}", name=f"qT{i}") for i in range(2)]
            kT = [pp.tile([128, S], bf16, tag=f"kT{i}", name=f"kT{i}") for i in range(2)]
            v_aug = pp.tile([128, NSC * VSC], bf16, tag="vaug")
            concatT = [pp.tile([128, S], bf16, tag=f"cT{i}", name=f"cT{i}") for i in range(2)]
            woT_sb = pp.tile([128, 2, D], bf16, tag="woT")
            cs_sb = pp.tile([128, 2, S], bf16, tag="cs")
            co_sb = pp.tile([128, C_W + VSC], bf16, tag="consts")
            pw_sb = pp.tile([128, 8, 768], bf16, tag="pw")
            xs = pp.tile([128, 8, S], bf16, tag="xs")

            # input DMAs, in need-order (SP queue drains FIFO)
            pw_r = pw[:].rearrange("(k p) m -> p k m", p=128)
            xs_r = xT[:].rearrange("(k p) m -> p k m", p=128)
            va_c = v_aug[:].rearrange("p (c r) -> p c r", c=NSC)
            nc.sync.dma_start(pw_sb[:, :, 0:256], pw_r[:, :, 0:256])
            nc.scalar.dma_start(xs[:, :, 0:ST], xs_r[:, :, 0:ST])
            nc.sync.dma_start(co_sb[:], consts[:])
            nc.vector.tensor_copy(va_c[:, 0, :], co_sb[:, C_W:C_W + VSC])
            nc.sync.dma_start(pw_sb[:, :, 256:768], pw_r[:, :, 256:768])
            nc.scalar.dma_start(cs_sb[0:64, :, :],
                                cossin[:].rearrange("p (c s) -> p c s", c=2))
            nc.vector.tensor_copy(cs_sb[64:128, :, :], cs_sb[0:64, :, :])
            nc.scalar.dma_start(xs[:, :, ST:2 * ST], xs_r[:, :, ST:2 * ST])
            nc.scalar.dma_start(xs[:, :, 2 * ST:3 * ST],
                                xs_r[:, :, 2 * ST:3 * ST])
            nc.scalar.dma_start(xs[:, :, 3 * ST:4 * ST],
                                xs_r[:, :, 3 * ST:4 * ST])
            nc.sync.dma_start(woT_sb[:],
                              woT[:].rearrange("(k p) m -> p k m", p=128))
            # replicate the ones/zeros pattern to the other 15 key-chunks on
            # the otherwise-idle Pool engine (SBUF->SBUF)
            for c in range(1, NSC):
                nc.gpsimd.tensor_copy(va_c[:, c, :], va_c[:, 0, :])

            if dbg:
                bdbg_sb = pp.tile([128, 2 * ST], f32, tag="bdbg")
                nc.vector.memset(bdbg_sb[:], 0.0)
            va_sc = v_aug[:].rearrange("p (c h r) -> p c h r", c=NSC, r=VHP)
            tri2 = co_sb[:, C_TRI:C_TRI + 256].rearrange(
                "p (h e) -> p h e", h=2)
            mrotT = co_sb[:, C_ROT:C_ROT + 128]
            onesA = co_sb[0:1, C_ONA:C_ONA + 128]
            onesB = co_sb[0:1, C_ONB:C_ONB + 128]

            def proj_chunks(st):
                """Projection work for band st as a list of issue-chunks so
                band(st-1) can interleave them into its PE stream: 4 Q/K
                units, one rots+adds chunk, 4 V chunks."""
                sl = slice(st * ST, (st + 1) * ST)
                units = []

                def mk_qk(hp, coff):
                    def f():
                        dstT = qT if coff == 0 else kT
                        ps = psW.tile([128, ST], f32, tag="w", name="ps")
                        o = coff + 128 * hp
                        for kt in range(8):
                            nc.tensor.matmul(ps[:], pw_sb[:, kt, o:o + 128],
                                             xs[:, kt, sl],
                                             start=(kt == 0), stop=(kt == 7))
                        t1 = rp.tile([128, ST], bf16, tag="t1", name="t1")
                        ts = rp.tile([128, ST], bf16, tag="ts", name="ts")
                        nc.vector.tensor_tensor(t1[:], ps[:], cs_sb[:, 0, sl],
                                                Alu.mult)
                        nc.vector.tensor_tensor(ts[:], ps[:], cs_sb[:, 1, sl],
                                                Alu.mult)
                        units.append((t1, ts, dstT, hp))
                    return f

                def rots():
                    while units:
                        t1, ts, dstT, hp = units.pop(0)
                        rot = psW.tile([128, ST], f32, tag="w", name="rot")
                        nc.tensor.matmul(rot[:], mrotT, ts[:],
                                         start=True, stop=True)
                        nc.vector.tensor_tensor(dstT[hp][:, sl], t1[:],
                                                rot[:], Alu.add)

                def mk_v(scl):
                    def f():
                        sc = st * 4 + scl
                        vp = psW.tile([128, ST], f32, tag="w", name="vp")
                        lo = st * ST + scl * 128
                        for kt in range(8):
                            nc.tensor.matmul(vp[:, 0:256],
                                             xs[:, kt, lo:lo + 128],
                                             pw_sb[:, kt, 512:768],
                                             start=(kt == 0), stop=(kt == 7))
                        vr = vp[:, 0:256].rearrange("p (g t e) -> p g t e",
                                                    g=2, t=2)
                        nc.vector.tensor_copy(va_sc[:, sc, :, 0:64],
                                              vr[:, :, 0, :])
                        nc.vector.tensor_copy(va_sc[:, sc, :, 129:193],
                                              vr[:, :, 1, :])
                    return f

                return [mk_qk(0, 0), mk_qk(1, 0), mk_qk(0, 256), rots,
                        mk_qk(1, 256), rots,
                        mk_v(0), mk_v(1), mk_v(2), mk_v(3)]

            def band(b, feed=(), tail_chunks=()):
                feed = list(feed)
                ntile = 2 * (4 * b + 4)
                nfeed = len(feed)
                tctr = 0

                def pace(last=False):
                    nonlocal tctr
                    tctr += 1
                    if last:
                        return
                    want = tctr * nfeed // ntile
                    while feed and (nfeed - len(feed)) < want:
                        feed.pop(0)()

                def pace2():
                    pass
                qb = b * ST
                nkt = 4 * b + 4
                # diagonal tiles first, ascending j: tile j=0 claims the full
                # PSUM bank with a full-width start (the start bit arms zeroing
                # at 2KB-bank granularity, so sub-range starts are illegal);
                # later diagonal tiles accumulate their live sub-range only
                tiles = [4 * b, 4 * b + 1, 4 * b + 2, 4 * b + 3] \
                    + list(range(0, 4 * b))
                for hp in range(2):
                    pvA = psA.tile([128, ST], f32, tag="pva")
                    pvB = psB.tile([128, ST], f32, tag="pvb")
                    wts = {}

                    def emit_pv(i):
                        kt = tiles[i]
                        j = kt - 4 * b
                        last = (i == nkt - 1)
                        lo = 128 * j if j > 0 else 0
                        st_ = (i == 0)
                        wt = wts.pop(i)
                        nc.tensor.matmul(pvA[0:VA, lo:512],
                                         va_sc[:, kt, hp, 0:VA],
                                         wt[:, lo:512],
                                         start=st_, stop=last,
                                         skip_group_check=True)
                        nc.tensor.matmul(pvB[:, lo:512],
                                         va_sc[:, kt, hp, VA:VHP],
                                         wt[:, 512 + lo:1024],
                                         start=st_, stop=last,
                                         skip_group_check=True)

                    for i, kt in enumerate(tiles):
                        j = kt - 4 * b
                        qlo = 128 * j if j >= 0 else 0
                        ksl = slice(kt * 128, (kt + 1) * 128)
                        if i % 2 == 1:
                            pace2()
                        scp = psS.tile([128, 2 * ST], f32, tag="sc")
                        for h in range(2):
                            nc.tensor.matmul(
                                scp[:, 512 * h + qlo:512 * h + 512],
                                kT[hp][64 * h:64 * h + 64, ksl],
                                qT[hp][64 * h:64 * h + 64,
                                       qb + qlo:qb + 512],
                                start=True, stop=True)
                        wt = wtp.tile([128, 2 * ST], bf16, tag="wt")
                        scv = scp[:].rearrange("p (h q) -> p h q", h=2)
                        wtv = wt[:].rearrange("p (h q) -> p h q", h=2)
                        nc.scalar.activation(wtv[:, :, qlo:512],
                                             scv[:, :, qlo:512],
                                             AF.Exp, scale=0.125)
                        if j >= 0:
                            nc.vector.tensor_tensor(
                                wtv[:, :, qlo:qlo + 128],
                                wtv[:, :, qlo:qlo + 128], tri2, Alu.mult)
                        wts[i] = wt
                        pace(last=HOLD and (i >= nkt - 2))
                        if i >= LOOK:
                            emit_pv(i - LOOK)
                    for i in range(max(0, nkt - LOOK), nkt):
                        emit_pv(i)

                    # softmax normalization: denominator rows -> SBUF bf16,
                    # broadcast across partitions via a PE matmul with ones
                    # vectors, one full-tile reciprocal, then one
                    # psum-x-sbuf multiply per head half
                    if feed:
                        feed.pop(0)()
                    dnA = rcp.tile([1, ST], bf16, tag="r")
                    dnB = rcp.tile([1, ST], bf16, tag="r")
                    nc.scalar.copy(dnA[:], pvA[64:65, :])
                    nc.vector.tensor_copy(dnB[:], pvB[0:1, :])
                    bcp = psW.tile([128, ST], f32, tag="w")
                    nc.tensor.matmul(bcp[:], onesA, dnA[:],
                                     start=True, stop=False)
                    nc.tensor.matmul(bcp[:], onesB, dnB[:],
                                     start=False, stop=True)
                    if feed:
                        feed.pop(0)()
                    bc = rcp.tile([128, ST], f32, tag="bc")
                    nc.vector.reciprocal_approx_fast(bc[:], bcp[:])
                    tail_c = tail_chunks if hp == 1 else []
                    for ci in range(2):
                        csl = slice(ci * 256, (ci + 1) * 256)
                        qsl = slice(qb + ci * 256, qb + (ci + 1) * 256)
                        nc.vector.tensor_tensor(concatT[hp][0:64, qsl],
                                                pvA[0:64, csl], bc[0:64, csl],
                                                Alu.mult)
                        nc.vector.tensor_tensor(concatT[hp][64:128, qsl],
                                                pvB[64:128, csl],
                                                bc[64:128, csl],
                                                Alu.mult)
                        for f in tail_c[2 * ci:2 * ci + 2]:
                            f()
                    if hp == 1:
                        for f in feed:
                            f()
                        feed = []
                    if dbg and b == 0 and hp == 0:
                        nc.vector.tensor_copy(bdbg_sb[0:65, 0:ST],
                                              pvA[0:65, :])
                        nc.vector.tensor_copy(bdbg_sb[:, ST:2 * ST], pvB[:])
                        nc.sync.dma_start(bdbg[:], bdbg_sb[:])

            def phase_c_chunks(b, narrow=False, alternate=False,
                               tailq=False):
                def mk(sc, narrow=narrow):
                    def f():
                        ssl = slice(sc * 128, (sc + 1) * 128)
                        ob = obp.tile([128, D], bf16, tag="ob", name="ob")
                        if narrow:
                            ops = [psW.tile([128, ST], f32, tag="w", name="op")
                                   for _ in range(2)]
                        else:
                            opw = psS.tile([128, D], f32, tag="sc", name="op")
                            ops = [opw[:, 0:512], opw[:, 512:1024]]
                        for dh in range(2):
                            dsl = slice(dh * 512, (dh + 1) * 512)
                            o = ops[dh][:] if narrow else ops[dh]
                            nc.tensor.matmul(o, concatT[0][:, ssl],
                                             woT_sb[:, 0, dsl],
                                             start=True, stop=False)
                            nc.tensor.matmul(o, concatT[1][:, ssl],
                                             woT_sb[:, 1, dsl],
                                             start=False, stop=True)
                        o0 = ops[0][:] if narrow else ops[0]
                        o1 = ops[1][:] if narrow else ops[1]
                        nc.scalar.copy(ob[:, 0:512], o0)
                        nc.sync.dma_start(out[ssl, 0:512], ob[:, 0:512])
                        nc.vector.tensor_copy(ob[:, 512:1024], o1)
                        dq = nc.gpsimd if tailq else nc.sync
                        dq.dma_start(out[ssl, 512:1024], ob[:, 512:1024])
                    return f
                if alternate:
                    return [mk(sc, narrow=(sc % 2 == 1))
                            for sc in range(4 * b, 4 * b + 4)]
                return [mk(sc) for sc in range(4 * b, 4 * b + 4)]

            for f in proj_chunks(0):
                f()
            for st in range(4):
                pj = proj_chunks(st + 1) if st < 3 else []
                if st == 1:
                    cc = phase_c_chunks(0)
                elif st == 3:
                    cc = phase_c_chunks(1, narrow=True) \
                        + phase_c_chunks(2, narrow=True)
                else:
                    cc = []
                feed = []
                while pj or cc:
                    if pj:
                        feed.append(pj.pop(0))
                    if cc:
                        feed.append(cc.pop(0))
                band(st, feed=feed,
                     tail_chunks=phase_c_chunks(3, alternate=True,
                                                tailq=True)
                     if st == 3 else ())
            if dbg:
                for i in range(2):
                    nc.sync.dma_start(qdbg[:, i * S:(i + 1) * S], qT[i][:])
                    nc.sync.dma_start(kdbg[:, i * S:(i + 1) * S], kT[i][:])
                    nc.sync.dma_start(cdbg[:, i * S:(i + 1) * S], concatT[i][:])
                nc.sync.dma_start(vdbg[:], v_aug[:])
    nc.finalize()
    return nc


def _rope_tables():
    inv_freq = 1.0 / (THETA ** (np.arange(0, DK, 2, dtype=np.float64) / DK))
    t = np.arange(S, dtype=np.float64)
    freqs = np.outer(t, inv_freq)
    emb = np.stack((freqs, freqs), axis=-1).reshape(S, DK)
    return np.cos(emb).astype(np.float32), np.sin(emb).astype(np.float32)


def _host_consts():
    consts = np.zeros((128, C_W), np.float32)  # vpat appended below
    p = np.arange(128)
    tri = (p[None, :] >= p[:, None])
    consts[:, C_TRI:C_TRI + 128] = tri
    consts[:, C_TRI + 128:C_TRI + 256] = tri
    # rot lhsT: lhsT[2i+1, 2i] = -1, lhsT[2i, 2i+1] = +1
    m = np.zeros((128, 128), np.float32)
    i2 = np.arange(0, 128, 2)
    m[i2 + 1, i2] = -1.0
    m[i2, i2 + 1] = 1.0
    consts[:, C_ROT:C_ROT + 128] = m
    consts[0, C_ONA:C_ONA + 64] = 1.0
    consts[0, C_ONB + 64:C_ONB + 128] = 1.0

    vpat_np = np.zeros((128, VSC), np.float32)
    for r in range(2):
        base = r * VHP
        vpat_np[:, base + 64] = 1.0   # A ones column
        vpat_np[:, base + VA] = 1.0   # B ones column
    return (np.concatenate([consts, vpat_np], axis=1)
            .astype(ml_dtypes.bfloat16),)


def kernel(x, token_positions, W_q, W_k, W_v, W_o):
    global _NC, _CONSTS
    if _NC is None:
        _NC = _build()
    x = np.asarray(x, dtype=np.float32)
    token_positions = np.asarray(token_positions)
    W_q = np.asarray(W_q, dtype=np.float32)
    W_k = np.asarray(W_k, dtype=np.float32)
    W_v = np.asarray(W_v, dtype=np.float32)
    W_o = np.asarray(W_o, dtype=np.float32)

    if _CONSTS is None:
        _CONSTS = (*_rope_tables(), *_host_consts())
    cos_t, sin_t, consts_np = _CONSTS

    in_maps = []
    for c in range(8):
        b, g = divmod(c, 4)
        rows = slice(256 * g, 256 * (g + 1))
        pw_np = np.concatenate(
            [W_q[rows].T, W_k[rows].T, W_v[rows].T], axis=1
        ).astype(ml_dtypes.bfloat16)
        woT_np = W_o[:, rows].T.astype(ml_dtypes.bfloat16)
        pos = np.asarray(token_positions[b], dtype=np.int64)
        cossin_np = np.concatenate(
            [cos_t[pos].T, sin_t[pos].T], axis=1).astype(ml_dtypes.bfloat16)
        xT_np = x[b].T.astype(ml_dtypes.bfloat16)
        in_maps.append({
            "xT": np.ascontiguousarray(xT_np),
            "pw": np.ascontiguousarray(pw_np),
            "woT": np.ascontiguousarray(woT_np),
            "cossin": np.ascontiguousarray(cossin_np),
            "consts": consts_np,
        })

    global _LAST_RES
    res = run_bass_kernel_spmd(_NC, in_maps, core_ids=list(range(8)))
    _LAST_RES = res
    outs = [res.results[c]["out"] for c in range(8)]
    o0 = outs[0] + outs[1] + outs[2] + outs[3]
    o1 = outs[4] + outs[5] + outs[6] + outs[7]
    return np.stack([o0, o1]).astype(np.float32)


# revision 96
# speedup vs baseline: 1.9401x; 1.0058x over previous
"""TRN2 Bass/Tile kernel: causal self-attention with RoPE (bf16 pipeline).

Sharding across 8 NeuronCores: batch (2) x head-groups (4 groups of 4 heads).
Per core, for its batch and 4 heads (2 head-pairs "hp" of 2 heads each):

- Phase A: Q/K/V projections in bf16. RoPE is applied as
  q_rope = q*cos + rot(q*sin) where rot is a fixed pair-swap/sign
  permutation executed as a single [128x128] matmul on the PE (the
  interleaved cos/sin tables are pair-equal so rot commutes with them).
- Phase B: causal attention in scores^T orientation ([keys, q] tiles).
  Diagonal key-tiles are restricted to their live q-range and masked with
  one [128,128] triangle on DVE; fully-masked regions are never computed.
  Softmax denominators come from ones-columns in the augmented V (extra
  PSUM rows are free: matmul cost depends only on the moving free size).
- Phase C: output projection partials, summed on the host.

All matmuls bf16 with fp32 PSUM accumulation. The per-band pipeline
software-interleaves the next band's projection chunks and the previous
band's output-projection chunks into each band's instruction stream
(pace()/force-feed), so the PE stays busy through exp/normalization
latency. PSUM: 2x[128,1024] score tiles, 2x[128,512] work tiles,
2 PV accumulators (8 banks exactly).
"""
import numpy as np
import ml_dtypes
import concourse.bass as bass
from concourse import bacc
import concourse.mybir as mybir
import concourse.tile as tile
from concourse.bass_utils import run_bass_kernel_spmd

B, S, D = 2, 2048, 1024
H, DK = 16, 64
THETA = 10000.0
ST = 512              # q-band width
NSC = S // 128        # 16 key chunks of 128
f32 = mybir.dt.float32
bf16 = mybir.dt.bfloat16
AF = mybir.ActivationFunctionType
Alu = mybir.AluOpType

# v_aug layout per key-chunk, per head pair:
# A head [v(64) | one] = 65 cols -> psum rows 0:64 attn, row 64 denom
# B head [one | zeros(63) | v(64)] = 128 cols -> row 0 denom, rows 64:128 attn
VA = 65
VB = 128
VHP = VA + VB         # 193
VSC = 2 * VHP         # 386

# consts layout: [tri(256 = 2 copies) | mrotT(128) | onesA(128) | onesB(128)]
C_TRI = 0
C_ROT = 256
C_ONA = 384
C_ONB = 512
C_W = 640

_NC = None
_CONSTS = None
_LAST_RES = None


def _build():
    nc = bacc.Bacc()
    xT = nc.dram_tensor("xT", [D, S], bf16, kind="ExternalInput")
    pw = nc.dram_tensor("pw", [D, 768], bf16, kind="ExternalInput")
    woT = nc.dram_tensor("woT", [256, D], bf16, kind="ExternalInput")
    cossin = nc.dram_tensor("cossin", [64, 2 * S], bf16, kind="ExternalInput")
    consts = nc.dram_tensor("consts", [128, C_W + VSC], bf16,
                            kind="ExternalInput")
    out = nc.dram_tensor("out", [S, D], bf16, kind="ExternalOutput")
    import os
    dbg = os.environ.get("K_DEBUG", "") == "1"
    LOOK = int(os.environ.get("K_LOOK", "3"))
    HOLD = os.environ.get("K_HOLD", "0") == "1"
    if dbg:
        qdbg = nc.dram_tensor("qdbg", [128, 2 * S], bf16, kind="ExternalOutput")
        kdbg = nc.dram_tensor("kdbg", [128, 2 * S], bf16, kind="ExternalOutput")
        vdbg = nc.dram_tensor("vdbg", [128, NSC * VSC], bf16,
                              kind="ExternalOutput")
        cdbg = nc.dram_tensor("cdbg", [128, 2 * S], bf16, kind="ExternalOutput")
        bdbg = nc.dram_tensor("bdbg", [128, 2 * ST], f32, kind="ExternalOutput")

    with tile.TileContext(nc) as tc:
        with tc.tile_pool(name="persist", bufs=1) as pp, \
             tc.tile_pool(name="rope", bufs=4) as rp, \
             tc.tile_pool(name="wtp", bufs=6) as wtp, \
             tc.tile_pool(name="rcp", bufs=4) as rcp, \
             tc.tile_pool(name="obp", bufs=3) as obp, \
             tc.tile_pool(name="psS", bufs=2, space="PSUM") as psS, \
             tc.tile_pool(name="psW", bufs=2, space="PSUM") as psW, \
             tc.tile_pool(name="psA", bufs=1, space="PSUM") as psA, \
             tc.tile_pool(name="psB", bufs=1, space="PSUM") as psB:
            qT = [pp.tile([128, S], bf16, tag=f"qT{i}", name=f"qT{i}") for i in range(2)]
            kT = [pp.tile([128, S], bf16, tag=f"kT{i}", name=f"kT{i}") for i in range(2)]
            v_aug = pp.tile([128, NSC * VSC], bf16, tag="vaug")
            concatT = [pp.tile([128, S], bf16, tag=f"cT{i}", name=f"cT{i}") for i in range(2)]
            woT_sb = pp.tile([128, 2, D], bf16, tag="woT")
            cs_sb = pp.tile([128, 2, S], bf16, tag="cs")
            co_sb = pp.tile([128, C_W + VSC], bf16, tag="consts")
            pw_sb = pp.tile([128, 8, 768], bf16, tag="pw")
            xs = pp.tile([128, 8, S], bf16, tag="xs")

            # input DMAs, in need-order (SP queue drains FIFO)
            pw_r = pw[:].rearrange("(k p) m -> p k m", p=128)
            xs_r = xT[:].rearrange("(k p) m -> p k m", p=128)
            va_c = v_aug[:].rearrange("p (c r) -> p c r", c=NSC)
            nc.sync.dma_start(pw_sb[:, :, 0:256], pw_r[:, :, 0:256])
            nc.scalar.dma_start(xs[:, :, 0:ST], xs_r[:, :, 0:ST])
            nc.sync.dma_start(co_sb[:], consts[:])
            nc.vector.tensor_copy(va_c[:, 0, :], co_sb[:, C_W:C_W + VSC])
            nc.sync.dma_start(pw_sb[:, :, 256:768], pw_r[:, :, 256:768])
            nc.scalar.dma_start(cs_sb[0:64, :, :],
                                cossin[:].rearrange("p (c s) -> p c s", c=2))
            nc.vector.tensor_copy(cs_sb[64:128, :, :], cs_sb[0:64, :, :])
            nc.scalar.dma_start(xs[:, :, ST:2 * ST], xs_r[:, :, ST:2 * ST])
            nc.scalar.dma_start(xs[:, :, 2 * ST:3 * ST],
                                xs_r[:, :, 2 * ST:3 * ST])
            nc.scalar.dma_start(xs[:, :, 3 * ST:4 * ST],
                                xs_r[:, :, 3 * ST:4 * ST])
            nc.sync.dma_start(woT_sb[:],
                              woT[:].rearrange("(k p) m -> p k m", p=128))
            # replicate the ones/zeros pattern to the other 15 key-chunks on
            # the otherwise-idle Pool engine (SBUF->SBUF)
            for c in range(1, NSC):
                nc.gpsimd.tensor_copy(va_c[:, c, :], va_c[:, 0, :])

            if dbg:
                bdbg_sb = pp.tile([128, 2 * ST], f32, tag="bdbg")
                nc.vector.memset(bdbg_sb[:], 0.0)
            va_sc = v_aug[:].rearrange("p (c h r) -> p c h r", c=NSC, r=VHP)
            tri2 = co_sb[:, C_TRI:C_TRI + 256].rearrange(
                "p (h e) -> p h e", h=2)
            mrotT = co_sb[:, C_ROT:C_ROT + 128]
            onesA = co_sb[0:1, C_ONA:C_ONA + 128]
            onesB = co_sb[0:1, C_ONB:C_ONB + 128]

            def proj_chunks(st):
                """Projection work for band st as a list of issue-chunks so
                band(st-1) can interleave them into its PE stream: 4 Q/K
                units, one rots+adds chunk, 4 V chunks."""
                sl = slice(st * ST, (st + 1) * ST)
                units = []

                def mk_qk(hp, coff):
                    def f():
                        dstT = qT if coff == 0 else kT
                        ps = psW.tile([128, ST], f32, tag="w", name="ps")
                        o = coff + 128 * hp
                        for kt in range(8):
                            nc.tensor.matmul(ps[:], pw_sb[:, kt, o:o + 128],
                                             xs[:, kt, sl],
                                             start=(kt == 0), stop=(kt == 7))
                        t1 = rp.tile([128, ST], bf16, tag="t1", name="t1")
                        ts = rp.tile([128, ST], bf16, tag="ts", name="ts")
                        nc.vector.tensor_tensor(t1[:], ps[:], cs_sb[:, 0, sl],
                                                Alu.mult)
                        nc.vector.tensor_tensor(ts[:], ps[:], cs_sb[:, 1, sl],
                                                Alu.mult)
                        units.append((t1, ts, dstT, hp))
                    return f

                def rots():
                    while units:
                        t1, ts, dstT, hp = units.pop(0)
                        rot = psW.tile([128, ST], f32, tag="w", name="rot")
                        nc.tensor.matmul(rot[:], mrotT, ts[:],
                                         start=True, stop=True)
                        nc.vector.tensor_tensor(dstT[hp][:, sl], t1[:],
                                                rot[:], Alu.add)

                def mk_v(scl):
                    def f():
                        sc = st * 4 + scl
                        vp = psW.tile([128, ST], f32, tag="w", name="vp")
                        lo = st * ST + scl * 128
                        for kt in range(8):
                            nc.tensor.matmul(vp[:, 0:256],
                                             xs[:, kt, lo:lo + 128],
                                             pw_sb[:, kt, 512:768],
                                             start=(kt == 0), stop=(kt == 7))
                        vr = vp[:, 0:256].rearrange("p (g t e) -> p g t e",
                                                    g=2, t=2)
                        nc.vector.tensor_copy(va_sc[:, sc, :, 0:64],
                                              vr[:, :, 0, :])
                        nc.vector.tensor_copy(va_sc[:, sc, :, 129:193],
                                              vr[:, :, 1, :])
                    return f

                return [mk_qk(0, 0), mk_qk(1, 0), mk_qk(0, 256), rots,
                        mk_qk(1, 256), rots,
                        mk_v(0), mk_v(1), mk_v(2), mk_v(3)]

            def band(b, feed=(), tail_chunks=()):
                feed = list(feed)
                ntile = 2 * (4 * b + 4)
                nfeed = len(feed)
                tctr = 0

                def pace(last=False):
                    nonlocal tctr
                    tctr += 1
                    if last:
                        return
                    want = tctr * nfeed // ntile
                    while feed and (nfeed - len(feed)) < want:
                        feed.pop(0)()

                def pace2():
                    pass
                qb = b * ST
                nkt = 4 * b + 4
                # diagonal tiles first, ascending j: tile j=0 claims the full
                # PSUM bank with a full-width start (the start bit arms zeroing
                # at 2KB-bank granularity, so sub-range starts are illegal);
                # later diagonal tiles accumulate their live sub-range only
                tiles = [4 * b, 4 * b + 1, 4 * b + 2, 4 * b + 3] \
                    + list(range(0, 4 * b))
                for hp in range(2):
                    pvA = psA.tile([128, ST], f32, tag="pva")
                    pvB = psB.tile([128, ST], f32, tag="pvb")
                    wts = {}

                    def emit_pv(i):
                        kt = tiles[i]
                        j = kt - 4 * b
                        last = (i == nkt - 1)
                        lo = 128 * j if j > 0 else 0
                        st_ = (i == 0)
                        wt = wts.pop(i)
                        nc.tensor.matmul(pvA[0:VA, lo:512],
                                         va_sc[:, kt, hp, 0:VA],
                                         wt[:, lo:512],
                                         start=st_, stop=last,
                                         skip_group_check=True)
                        nc.tensor.matmul(pvB[:, lo:512],
                                         va_sc[:, kt, hp, VA:VHP],
                                         wt[:, 512 + lo:1024],
                                         start=st_, stop=last,
                                         skip_group_check=True)

                    for i, kt in enumerate(tiles):
                        j = kt - 4 * b
                        qlo = 128 * j if j >= 0 else 0
                        ksl = slice(kt * 128, (kt + 1) * 128)
                        if i % 2 == 1:
                            pace2()
                        scp = psS.tile([128, 2 * ST], f32, tag="sc")
                        for h in range(2):
                            nc.tensor.matmul(
                                scp[:, 512 * h + qlo:512 * h + 512],
                                kT[hp][64 * h:64 * h + 64, ksl],
                                qT[hp][64 * h:64 * h + 64,
                                       qb + qlo:qb + 512],
                                start=True, stop=True)
                        wt = wtp.tile([128, 2 * ST], bf16, tag="wt")
                        scv = scp[:].rearrange("p (h q) -> p h q", h=2)
                        wtv = wt[:].rearrange("p (h q) -> p h q", h=2)
                        nc.scalar.activation(wtv[:, :, qlo:512],
                                             scv[:, :, qlo:512],
                                             AF.Exp, scale=0.125)
                        if j >= 0:
                            nc.vector.tensor_tensor(
                                wtv[:, :, qlo:qlo + 128],
                                wtv[:, :, qlo:qlo + 128], tri2, Alu.mult)
                        wts[i] = wt
                        pace(last=HOLD and (i >= nkt - 2))
                        if i >= LOOK:
                            emit_pv(i - LOOK)
                    for i in range(max(0, nkt - LOOK), nkt):
                        emit_pv(i)

                    # softmax normalization: denominator rows -> SBUF bf16,
                    # broadcast across partitions via a PE matmul with ones
                    # vectors, one full-tile reciprocal, then one
                    # psum-x-sbuf multiply per head half
                    if feed:
                        feed.pop(0)()
                    dnA = rcp.tile([1, ST], bf16, tag="r")
                    dnB = rcp.tile([1, ST], bf16, tag="r")
                    nc.scalar.copy(dnA[:], pvA[64:65, :])
                    nc.vector.tensor_copy(dnB[:], pvB[0:1, :])
                    bcp = psW.tile([128, ST], f32, tag="w")
                    nc.tensor.matmul(bcp[:], onesA, dnA[:],
                                     start=True, stop=False)
                    nc.tensor.matmul(bcp[:], onesB, dnB[:],
                                     start=False, stop=True)
                    if feed:
                        feed.pop(0)()
                    bc = rcp.tile([128, ST], f32, tag="bc")
                    nc.vector.reciprocal_approx_fast(bc[:], bcp[:])
                    tail_c = tail_chunks if hp == 1 else []
                    for ci in range(2):
                        csl = slice(ci * 256, (ci + 1) * 256)
                        qsl = slice(qb + ci * 256, qb + (ci + 1) * 256)
                        nc.vector.tensor_tensor(concatT[hp][0:64, qsl],
                                                pvA[0:64, csl], bc[0:64, csl],
                                                Alu.mult)
                        nc.vector.tensor_tensor(concatT[hp][64:128, qsl],
                                                pvB[64:128, csl],
                                                bc[64:128, csl],
                                                Alu.mult)
                        for f in tail_c[2 * ci:2 * ci + 2]:
                            f()
                    if hp == 1:
                        for f in feed:
                            f()
                        feed = []
                    if dbg and b == 0 and hp == 0:
                        nc.vector.tensor_copy(bdbg_sb[0:65, 0:ST],
                                              pvA[0:65, :])
                        nc.vector.tensor_copy(bdbg_sb[:, ST:2 * ST], pvB[:])
                        nc.sync.dma_start(bdbg[:], bdbg_sb[:])

            def phase_c_chunks(b, narrow=False, alternate=False,
                               tailq=False):
                def mk(sc, narrow=narrow):
                    def f():
                        ssl = slice(sc * 128, (sc + 1) * 128)
                        ob = obp.tile([128, D], bf16, tag="ob", name="ob")
                        if narrow:
                            ops = [psW.tile([128, ST], f32, tag="w", name="op")
                                   for _ in range(2)]
                        else:
                            opw = psS.tile([128, D], f32, tag="sc", name="op")
                            ops = [opw[:, 0:512], opw[:, 512:1024]]
                        for dh in range(2):
                            dsl = slice(dh * 512, (dh + 1) * 512)
                            o = ops[dh][:] if narrow else ops[dh]
                            nc.tensor.matmul(o, concatT[0][:, ssl],
                                             woT_sb[:, 0, dsl],
                                             start=True, stop=False)
                            nc.tensor.matmul(o, concatT[1][:, ssl],
                                             woT_sb[:, 1, dsl],
                                             start=False, stop=True)
                        o0 = ops[0][:] if narrow else ops[0]
                        o1 = ops[1][:] if narrow else ops[1]
                        nc.scalar.copy(ob[:, 0:512], o0)
                        nc.sync.dma_start(out[ssl, 0:512], ob[:, 0:512])
                        nc.vector.tensor_copy(ob[:, 512:1024], o1)
                        dq = nc.gpsimd if (tailq and sc < 15) else nc.sync
                        dq.dma_start(out[ssl, 512:1024], ob[:, 512:1024])
                    return f
                if alternate:
                    return [mk(sc, narrow=(sc % 2 == 1))
                            for sc in range(4 * b, 4 * b + 4)]
                return [mk(sc) for sc in range(4 * b, 4 * b + 4)]

            for f in proj_chunks(0):
                f()
            for st in range(4):
                pj = proj_chunks(st + 1) if st < 3 else []
                if st == 1:
                    cc = phase_c_chunks(0)
                elif st == 3:
                    cc = phase_c_chunks(1, narrow=True) \
                        + phase_c_chunks(2, narrow=True)
                else:
                    cc = []
                feed = []
                while pj or cc:
                    if pj:
                        feed.append(pj.pop(0))
                    if cc:
                        feed.append(cc.pop(0))
                band(st, feed=feed,
                     tail_chunks=phase_c_chunks(3, alternate=True,
                                                tailq=True)
                     if st == 3 else ())
            if dbg:
                for i in range(2):
                    nc.sync.dma_start(qdbg[:, i * S:(i + 1) * S], qT[i][:])
                    nc.sync.dma_start(kdbg[:, i * S:(i + 1) * S], kT[i][:])
                    nc.sync.dma_start(cdbg[:, i * S:(i + 1) * S], concatT[i][:])
                nc.sync.dma_start(vdbg[:], v_aug[:])
    nc.finalize()
    return nc


def _rope_tables():
    inv_freq = 1.0 / (THETA ** (np.arange(0, DK, 2, dtype=np.float64) / DK))
    t = np.arange(S, dtype=np.float64)
    freqs = np.outer(t, inv_freq)
    emb = np.stack((freqs, freqs), axis=-1).reshape(S, DK)
    return np.cos(emb).astype(np.float32), np.sin(emb).astype(np.float32)


def _host_consts():
    consts = np.zeros((128, C_W), np.float32)  # vpat appended below
    p = np.arange(128)
    tri = (p[None, :] >= p[:, None])
    consts[:, C_TRI:C_TRI + 128] = tri
    consts[:, C_TRI + 128:C_TRI + 256] = tri
    # rot lhsT: lhsT[2i+1, 2i] = -1, lhsT[2i, 2i+1] = +1
    m = np.zeros((128, 128), np.float32)
    i2 = np.arange(0, 128, 2)
    m[i2 + 1, i2] = -1.0
    m[i2, i2 + 1] = 1.0
    consts[:, C_ROT:C_ROT + 128] = m
    consts[0, C_ONA:C_ONA + 64] = 1.0
    consts[0, C_ONB + 64:C_ONB + 128] = 1.0

    vpat_np = np.zeros((128, VSC), np.float32)
    for r in range(2):
        base = r * VHP
        vpat_np[:, base + 64] = 1.0   # A ones column
        vpat_np[:, base + VA] = 1.0   # B ones column
    return (np.concatenate([consts, vpat_np], axis=1)
            .astype(ml_dtypes.bfloat16),)


def kernel(x, token_positions, W_q, W_k, W_v, W_o):
    global _NC, _CONSTS
    if _NC is None:
        _NC = _build()
    x = np.asarray(x, dtype=np.float32)
    token_positions = np.asarray(token_positions)
    W_q = np.asarray(W_q, dtype=np.float32)
    W_k = np.asarray(W_k, dtype=np.float32)
    W_v = np.asarray(W_v, dtype=np.float32)
    W_o = np.asarray(W_o, dtype=np.float32)

    if _CONSTS is None:
        _CONSTS = (*_rope_tables(), *_host_consts())
    cos_t, sin_t, consts_np = _CONSTS

    in_maps = []
    for c in range(8):
        b, g = divmod(c, 4)
        rows = slice(256 * g, 256 * (g + 1))
        pw_np = np.concatenate(
            [W_q[rows].T, W_k[rows].T, W_v[rows].T], axis=1
        ).astype(ml_dtypes.bfloat16)
        woT_np = W_o[:, rows].T.astype(ml_dtypes.bfloat16)
        pos = np.asarray(token_positions[b], dtype=np.int64)
        cossin_np = np.concatenate(
            [cos_t[pos].T, sin_t[pos].T], axis=1).astype(ml_dtypes.bfloat16)
        xT_np = x[b].T.astype(ml_dtypes.bfloat16)
        in_maps.append({
            "xT": np.ascontiguousarray(xT_np),
            "pw": np.ascontiguousarray(pw_np),
            "woT": np.ascontiguousarray(woT_np),
            "cossin": np.ascontiguousarray(cossin_np),
            "consts": consts_np,
        })

    global _LAST_RES
    res = run_bass_kernel_spmd(_NC, in_maps, core_ids=list(range(8)))
    _LAST_RES = res
    outs = [res.results[c]["out"] for c in range(8)]
    o0 = outs[0] + outs[1] + outs[2] + outs[3]
    o1 = outs[4] + outs[5] + outs[6] + outs[7]
    return np.stack([o0, o1]).astype(np.float32)


# revision 105
# speedup vs baseline: 1.9404x; 1.0001x over previous
"""TRN2 Bass/Tile kernel: causal self-attention with RoPE (bf16 pipeline).

Sharding across 8 NeuronCores: batch (2) x head-groups (4 groups of 4 heads).
Per core, for its batch and 4 heads (2 head-pairs "hp" of 2 heads each):

- Phase A: Q/K/V projections in bf16. RoPE is applied as
  q_rope = q*cos + rot(q*sin) where rot is a fixed pair-swap/sign
  permutation executed as a single [128x128] matmul on the PE (the
  interleaved cos/sin tables are pair-equal so rot commutes with them).
- Phase B: causal attention in scores^T orientation ([keys, q] tiles).
  Diagonal key-tiles are restricted to their live q-range and masked with
  one [128,128] triangle on DVE; fully-masked regions are never computed.
  Softmax denominators come from ones-columns in the augmented V (extra
  PSUM rows are free: matmul cost depends only on the moving free size).
- Phase C: output projection partials, summed on the host.

All matmuls bf16 with fp32 PSUM accumulation. The per-band pipeline
software-interleaves the next band's projection chunks and the previous
band's output-projection chunks into each band's instruction stream
(pace()/force-feed), so the PE stays busy through exp/normalization
latency. PSUM: 2x[128,1024] score tiles, 2x[128,512] work tiles,
2 PV accumulators (8 banks exactly).
"""
import numpy as np
import ml_dtypes
import concourse.bass as bass
from concourse import bacc
import concourse.mybir as mybir
import concourse.tile as tile
from concourse.bass_utils import run_bass_kernel_spmd

B, S, D = 2, 2048, 1024
H, DK = 16, 64
THETA = 10000.0
ST = 512              # q-band width
NSC = S // 128        # 16 key chunks of 128
f32 = mybir.dt.float32
bf16 = mybir.dt.bfloat16
AF = mybir.ActivationFunctionType
Alu = mybir.AluOpType

# v_aug layout per key-chunk, per head pair:
# A head [v(64) | one] = 65 cols -> psum rows 0:64 attn, row 64 denom
# B head [one | zeros(63) | v(64)] = 128 cols -> row 0 denom, rows 64:128 attn
VA = 65
VB = 128
VHP = VA + VB         # 193
VSC = 2 * VHP         # 386

# consts layout: [tri(256 = 2 copies) | mrotT(128) | onesA(128) | onesB(128)]
C_TRI = 0
C_ROT = 256
C_ONA = 384
C_ONB = 512
C_W = 640

_NC = None
_CONSTS = None
_LAST_RES = None


def _build():
    nc = bacc.Bacc()
    xT = nc.dram_tensor("xT", [D, S], bf16, kind="ExternalInput")
    pw = nc.dram_tensor("pw", [D, 768], bf16, kind="ExternalInput")
    woT = nc.dram_tensor("woT", [256, D], bf16, kind="ExternalInput")
    cossin = nc.dram_tensor("cossin", [64, 2 * S], bf16, kind="ExternalInput")
    consts = nc.dram_tensor("consts", [128, C_W + VSC], bf16,
                            kind="ExternalInput")
    out = nc.dram_tensor("out", [S, D], bf16, kind="ExternalOutput")
    import os
    dbg = os.environ.get("K_DEBUG", "") == "1"
    LOOK = int(os.environ.get("K_LOOK", "3"))
    HOLD = os.environ.get("K_HOLD", "0") == "1"
    if dbg:
        qdbg = nc.dram_tensor("qdbg", [128, 2 * S], bf16, kind="ExternalOutput")
        kdbg = nc.dram_tensor("kdbg", [128, 2 * S], bf16, kind="ExternalOutput")
        vdbg = nc.dram_tensor("vdbg", [128, NSC * VSC], bf16,
                              kind="ExternalOutput")
        cdbg = nc.dram_tensor("cdbg", [128, 2 * S], bf16, kind="ExternalOutput")
        bdbg = nc.dram_tensor("bdbg", [128, 2 * ST], f32, kind="ExternalOutput")

    with tile.TileContext(nc) as tc:
        with tc.tile_pool(name="persist", bufs=1) as pp, \
             tc.tile_pool(name="rope", bufs=4) as rp, \
             tc.tile_pool(name="wtp", bufs=6) as wtp, \
             tc.tile_pool(name="rcp", bufs=4) as rcp, \
             tc.tile_pool(name="obp", bufs=3) as obp, \
             tc.tile_pool(name="psS", bufs=2, space="PSUM") as psS, \
             tc.tile_pool(name="psW", bufs=2, space="PSUM") as psW, \
             tc.tile_pool(name="psA", bufs=1, space="PSUM") as psA, \
             tc.tile_pool(name="psB", bufs=1, space="PSUM") as psB:
            qT = [pp.tile([128, S], bf16, tag=f"qT{i}", name=f"qT{i}") for i in range(2)]
            kT = [pp.tile([128, S], bf16, tag=f"kT{i}", name=f"kT{i}") for i in range(2)]
            v_aug = pp.tile([128, NSC * VSC], bf16, tag="vaug")
            concatT = [pp.tile([128, S], bf16, tag=f"cT{i}", name=f"cT{i}") for i in range(2)]
            woT_sb = pp.tile([128, 2, D], bf16, tag="woT")
            cs_sb = pp.tile([128, 2, S], bf16, tag="cs")
            co_sb = pp.tile([128, C_W + VSC], bf16, tag="consts")
            pw_sb = pp.tile([128, 8, 768], bf16, tag="pw")
            xs = pp.tile([128, 8, S], bf16, tag="xs")

            # input DMAs, in need-order (SP queue drains FIFO)
            pw_r = pw[:].rearrange("(k p) m -> p k m", p=128)
            xs_r = xT[:].rearrange("(k p) m -> p k m", p=128)
            va_c = v_aug[:].rearrange("p (c r) -> p c r", c=NSC)
            nc.sync.dma_start(pw_sb[:, :, 0:256], pw_r[:, :, 0:256])
            nc.scalar.dma_start(xs[:, :, 0:ST], xs_r[:, :, 0:ST])
            nc.sync.dma_start(co_sb[:], consts[:])
            nc.vector.tensor_copy(va_c[:, 0, :], co_sb[:, C_W:C_W + VSC])
            nc.sync.dma_start(pw_sb[:, :, 256:768], pw_r[:, :, 256:768])
            nc.scalar.dma_start(cs_sb[0:64, :, :],
                                cossin[:].rearrange("p (c s) -> p c s", c=2))
            nc.vector.tensor_copy(cs_sb[64:128, :, :], cs_sb[0:64, :, :])
            nc.scalar.dma_start(xs[:, :, ST:2 * ST], xs_r[:, :, ST:2 * ST])
            nc.scalar.dma_start(xs[:, :, 2 * ST:3 * ST],
                                xs_r[:, :, 2 * ST:3 * ST])
            nc.scalar.dma_start(xs[:, :, 3 * ST:4 * ST],
                                xs_r[:, :, 3 * ST:4 * ST])
            nc.sync.dma_start(woT_sb[:],
                              woT[:].rearrange("(k p) m -> p k m", p=128))
            # replicate the ones/zeros pattern to the other 15 key-chunks on
            # the otherwise-idle Pool engine (SBUF->SBUF)
            for c in range(1, NSC):
                nc.gpsimd.tensor_copy(va_c[:, c, :], va_c[:, 0, :])

            if dbg:
                bdbg_sb = pp.tile([128, 2 * ST], f32, tag="bdbg")
                nc.vector.memset(bdbg_sb[:], 0.0)
            va_sc = v_aug[:].rearrange("p (c h r) -> p c h r", c=NSC, r=VHP)
            tri2 = co_sb[:, C_TRI:C_TRI + 256].rearrange(
                "p (h e) -> p h e", h=2)
            mrotT = co_sb[:, C_ROT:C_ROT + 128]
            onesA = co_sb[0:1, C_ONA:C_ONA + 128]
            onesB = co_sb[0:1, C_ONB:C_ONB + 128]

            def proj_chunks(st):
                """Projection work for band st as a list of issue-chunks so
                band(st-1) can interleave them into its PE stream: 4 Q/K
                units, one rots+adds chunk, 4 V chunks."""
                sl = slice(st * ST, (st + 1) * ST)
                units = []

                def mk_qk(hp, coff):
                    def f():
                        dstT = qT if coff == 0 else kT
                        ps = psW.tile([128, ST], f32, tag="w", name="ps")
                        o = coff + 128 * hp
                        for kt in range(8):
                            nc.tensor.matmul(ps[:], pw_sb[:, kt, o:o + 128],
                                             xs[:, kt, sl],
                                             start=(kt == 0), stop=(kt == 7))
                        t1 = rp.tile([128, ST], bf16, tag="t1", name="t1")
                        ts = rp.tile([128, ST], bf16, tag="ts", name="ts")
                        nc.vector.tensor_tensor(t1[:], ps[:], cs_sb[:, 0, sl],
                                                Alu.mult)
                        nc.vector.tensor_tensor(ts[:], ps[:], cs_sb[:, 1, sl],
                                                Alu.mult)
                        units.append((t1, ts, dstT, hp))
                    return f

                def rots():
                    while units:
                        t1, ts, dstT, hp = units.pop(0)
                        rot = psW.tile([128, ST], f32, tag="w", name="rot")
                        nc.tensor.matmul(rot[:], mrotT, ts[:],
                                         start=True, stop=True)
                        nc.vector.tensor_tensor(dstT[hp][:, sl], t1[:],
                                                rot[:], Alu.add)

                def mk_v(scl):
                    def f():
                        sc = st * 4 + scl
                        vp = psW.tile([128, ST], f32, tag="w", name="vp")
                        lo = st * ST + scl * 128
                        for kt in range(8):
                            nc.tensor.matmul(vp[:, 0:256],
                                             xs[:, kt, lo:lo + 128],
                                             pw_sb[:, kt, 512:768],
                                             start=(kt == 0), stop=(kt == 7))
                        vr = vp[:, 0:256].rearrange("p (g t e) -> p g t e",
                                                    g=2, t=2)
                        nc.vector.tensor_copy(va_sc[:, sc, :, 0:64],
                                              vr[:, :, 0, :])
                        nc.vector.tensor_copy(va_sc[:, sc, :, 129:193],
                                              vr[:, :, 1, :])
                    return f

                return [mk_qk(0, 0), mk_qk(1, 0), mk_qk(0, 256), rots,
                        mk_qk(1, 256), rots,
                        mk_v(0), mk_v(1), mk_v(2), mk_v(3)]

            def band(b, feed=(), tail_chunks=()):
                feed = list(feed)
                ntile = 2 * (4 * b + 4)
                nfeed = len(feed)
                tctr = 0

                def pace(last=False):
                    nonlocal tctr
                    tctr += 1
                    if last:
                        return
                    want = tctr * nfeed // ntile
                    while feed and (nfeed - len(feed)) < want:
                        feed.pop(0)()

                def pace2():
                    pass
                qb = b * ST
                nkt = 4 * b + 4
                # diagonal tiles first, ascending j: tile j=0 claims the full
                # PSUM bank with a full-width start (the start bit arms zeroing
                # at 2KB-bank granularity, so sub-range starts are illegal);
                # later diagonal tiles accumulate their live sub-range only
                tiles = [4 * b, 4 * b + 1, 4 * b + 2, 4 * b + 3] \
                    + list(range(0, 4 * b))
                for hp in range(2):
                    pvA = psA.tile([128, ST], f32, tag="pva")
                    pvB = psB.tile([128, ST], f32, tag="pvb")
                    wts = {}

                    def emit_pv(i):
                        kt = tiles[i]
                        j = kt - 4 * b
                        last = (i == nkt - 1)
                        lo = 128 * j if j > 0 else 0
                        st_ = (i == 0)
                        wt = wts.pop(i)
                        nc.tensor.matmul(pvA[0:VA, lo:512],
                                         va_sc[:, kt, hp, 0:VA],
                                         wt[:, lo:512],
                                         start=st_, stop=last,
                                         skip_group_check=True)
                        nc.tensor.matmul(pvB[:, lo:512],
                                         va_sc[:, kt, hp, VA:VHP],
                                         wt[:, 512 + lo:1024],
                                         start=st_, stop=last,
                                         skip_group_check=True)

                    for i, kt in enumerate(tiles):
                        j = kt - 4 * b
                        qlo = 128 * j if j >= 0 else 0
                        ksl = slice(kt * 128, (kt + 1) * 128)
                        if i % 2 == 1:
                            pace2()
                        scp = psS.tile([128, 2 * ST], f32, tag="sc")
                        for h in range(2):
                            nc.tensor.matmul(
                                scp[:, 512 * h + qlo:512 * h + 512],
                                kT[hp][64 * h:64 * h + 64, ksl],
                                qT[hp][64 * h:64 * h + 64,
                                       qb + qlo:qb + 512],
                                start=True, stop=True)
                        wt = wtp.tile([128, 2 * ST], bf16, tag="wt")
                        scv = scp[:].rearrange("p (h q) -> p h q", h=2)
                        wtv = wt[:].rearrange("p (h q) -> p h q", h=2)
                        nc.scalar.activation(wtv[:, :, qlo:512],
                                             scv[:, :, qlo:512],
                                             AF.Exp, scale=0.125)
                        if j >= 0:
                            nc.vector.tensor_tensor(
                                wtv[:, :, qlo:qlo + 128],
                                wtv[:, :, qlo:qlo + 128], tri2, Alu.mult)
                        wts[i] = wt
                        pace(last=HOLD and (i >= nkt - 2))
                        if i >= LOOK:
                            emit_pv(i - LOOK)
                    for i in range(max(0, nkt - LOOK), nkt):
                        emit_pv(i)

                    # softmax normalization: denominator rows -> SBUF bf16,
                    # broadcast across partitions via a PE matmul with ones
                    # vectors, one full-tile reciprocal, then one
                    # psum-x-sbuf multiply per head half
                    if feed:
                        feed.pop(0)()
                    dnA = rcp.tile([1, ST], bf16, tag="r")
                    dnB = rcp.tile([1, ST], bf16, tag="r")
                    nc.scalar.copy(dnA[:], pvA[64:65, :])
                    nc.vector.tensor_copy(dnB[:], pvB[0:1, :])
                    bcp = psW.tile([128, ST], f32, tag="w")
                    nc.tensor.matmul(bcp[:], onesA, dnA[:],
                                     start=True, stop=False)
                    nc.tensor.matmul(bcp[:], onesB, dnB[:],
                                     start=False, stop=True)
                    if feed:
                        feed.pop(0)()
                    bc = rcp.tile([128, ST], f32, tag="bc")
                    nc.vector.reciprocal_approx_fast(bc[:], bcp[:])
                    tail_c = tail_chunks if hp == 1 else []
                    for ci in range(2):
                        csl = slice(ci * 256, (ci + 1) * 256)
                        qsl = slice(qb + ci * 256, qb + (ci + 1) * 256)
                        nc.vector.tensor_tensor(concatT[hp][0:64, qsl],
                                                pvA[0:64, csl], bc[0:64, csl],
                                                Alu.mult)
                        nc.vector.tensor_tensor(concatT[hp][64:128, qsl],
                                                pvB[64:128, csl],
                                                bc[64:128, csl],
                                                Alu.mult)
                        for f in tail_c[2 * ci:2 * ci + 2]:
                            f()
                    if hp == 1:
                        for f in feed:
                            f()
                        feed = []
                    if dbg and b == 0 and hp == 0:
                        nc.vector.tensor_copy(bdbg_sb[0:65, 0:ST],
                                              pvA[0:65, :])
                        nc.vector.tensor_copy(bdbg_sb[:, ST:2 * ST], pvB[:])
                        nc.sync.dma_start(bdbg[:], bdbg_sb[:])

            def phase_c_chunks(b, narrow=False, alternate=False,
                               tailq=False):
                def mk(sc, narrow=narrow):
                    def f():
                        ssl = slice(sc * 128, (sc + 1) * 128)
                        ob = obp.tile([128, D], bf16, tag="ob", name="ob")
                        if narrow:
                            ops = [psW.tile([128, ST], f32, tag="w", name="op")
                                   for _ in range(2)]
                        else:
                            opw = psS.tile([128, D], f32, tag="sc", name="op")
                            ops = [opw[:, 0:512], opw[:, 512:1024]]
                        for dh in range(2):
                            dsl = slice(dh * 512, (dh + 1) * 512)
                            o = ops[dh][:] if narrow else ops[dh]
                            nc.tensor.matmul(o, concatT[0][:, ssl],
                                             woT_sb[:, 0, dsl],
                                             start=True, stop=False)
                            nc.tensor.matmul(o, concatT[1][:, ssl],
                                             woT_sb[:, 1, dsl],
                                             start=False, stop=True)
                        o0 = ops[0][:] if narrow else ops[0]
                        o1 = ops[1][:] if narrow else ops[1]
                        nc.vector.tensor_copy(ob[:, 512:1024], o1)
                        dq = nc.gpsimd if (tailq and sc < 15) else nc.sync
                        dq.dma_start(out[ssl, 512:1024], ob[:, 512:1024])
                        nc.scalar.copy(ob[:, 0:512], o0)
                        nc.sync.dma_start(out[ssl, 0:512], ob[:, 0:512])
                    return f
                if alternate:
                    return [mk(sc, narrow=(sc % 2 == 1))
                            for sc in range(4 * b, 4 * b + 4)]
                return [mk(sc) for sc in range(4 * b, 4 * b + 4)]

            for f in proj_chunks(0):
                f()
            for st in range(4):
                pj = proj_chunks(st + 1) if st < 3 else []
                if st == 1:
                    cc = phase_c_chunks(0)
                elif st == 3:
                    cc = phase_c_chunks(1, narrow=True) \
                        + phase_c_chunks(2, narrow=True)
                else:
                    cc = []
                feed = []
                while pj or cc:
                    if pj:
                        feed.append(pj.pop(0))
                    if cc:
                        feed.append(cc.pop(0))
                band(st, feed=feed,
                     tail_chunks=phase_c_chunks(3, alternate=True,
                                                tailq=True)
                     if st == 3 else ())
            if dbg:
                for i in range(2):
                    nc.sync.dma_start(qdbg[:, i * S:(i + 1) * S], qT[i][:])
                    nc.sync.dma_start(kdbg[:, i * S:(i + 1) * S], kT[i][:])
                    nc.sync.dma_start(cdbg[:, i * S:(i + 1) * S], concatT[i][:])
                nc.sync.dma_start(vdbg[:], v_aug[:])
    nc.finalize()
    return nc


def _rope_tables():
    inv_freq = 1.0 / (THETA ** (np.arange(0, DK, 2, dtype=np.float64) / DK))
    t = np.arange(S, dtype=np.float64)
    freqs = np.outer(t, inv_freq)
    emb = np.stack((freqs, freqs), axis=-1).reshape(S, DK)
    return np.cos(emb).astype(np.float32), np.sin(emb).astype(np.float32)


def _host_consts():
    consts = np.zeros((128, C_W), np.float32)  # vpat appended below
    p = np.arange(128)
    tri = (p[None, :] >= p[:, None])
    consts[:, C_TRI:C_TRI + 128] = tri
    consts[:, C_TRI + 128:C_TRI + 256] = tri
    # rot lhsT: lhsT[2i+1, 2i] = -1, lhsT[2i, 2i+1] = +1
    m = np.zeros((128, 128), np.float32)
    i2 = np.arange(0, 128, 2)
    m[i2 + 1, i2] = -1.0
    m[i2, i2 + 1] = 1.0
    consts[:, C_ROT:C_ROT + 128] = m
    consts[0, C_ONA:C_ONA + 64] = 1.0
    consts[0, C_ONB + 64:C_ONB + 128] = 1.0

    vpat_np = np.zeros((128, VSC), np.float32)
    for r in range(2):
        base = r * VHP
        vpat_np[:, base + 64] = 1.0   # A ones column
        vpat_np[:, base + VA] = 1.0   # B ones column
    return (np.concatenate([consts, vpat_np], axis=1)
            .astype(ml_dtypes.bfloat16),)


def kernel(x, token_positions, W_q, W_k, W_v, W_o):
    global _NC, _CONSTS
    if _NC is None:
        _NC = _build()
    x = np.asarray(x, dtype=np.float32)
    token_positions = np.asarray(token_positions)
    W_q = np.asarray(W_q, dtype=np.float32)
    W_k = np.asarray(W_k, dtype=np.float32)
    W_v = np.asarray(W_v, dtype=np.float32)
    W_o = np.asarray(W_o, dtype=np.float32)

    if _CONSTS is None:
        _CONSTS = (*_rope_tables(), *_host_consts())
    cos_t, sin_t, consts_np = _CONSTS

    in_maps = []
    for c in range(8):
        b, g = divmod(c, 4)
        rows = slice(256 * g, 256 * (g + 1))
        pw_np = np.concatenate(
            [W_q[rows].T, W_k[rows].T, W_v[rows].T], axis=1
        ).astype(ml_dtypes.bfloat16)
        woT_np = W_o[:, rows].T.astype(ml_dtypes.bfloat16)
        pos = np.asarray(token_positions[b], dtype=np.int64)
        cossin_np = np.concatenate(
            [cos_t[pos].T, sin_t[pos].T], axis=1).astype(ml_dtypes.bfloat16)
        xT_np = x[b].T.astype(ml_dtypes.bfloat16)
        in_maps.append({
            "xT": np.ascontiguousarray(xT_np),
            "pw": np.ascontiguousarray(pw_np),
            "woT": np.ascontiguousarray(woT_np),
            "cossin": np.ascontiguousarray(cossin_np),
            "consts": consts_np,
        })

    global _LAST_RES
    res = run_bass_kernel_spmd(_NC, in_maps, core_ids=list(range(8)))
    _LAST_RES = res
    outs = [res.results[c]["out"] for c in range(8)]
    o0 = outs[0] + outs[1] + outs[2] + outs[3]
    o1 = outs[4] + outs[5] + outs[6] + outs[7]
    return np.stack([o0, o1]).astype(np.float32)
